# revision 1
# baseline (speedup 1.0000x reference)
"""nn_AttentionC Trainium2 kernel (8 NeuronCores, SPMD).

Sharding: h-axis (64) split into 8 chunks of 8 rows, one per core; each core's
x slab is host-padded to [b2, t10, h10, w66] fp16 tokens (conv zero-padding
baked in). Only cross-core traffic: AllReduce of per-(b,head) [48,48] q/k
gram matrices (110 KB).

Per core (matmul data fp16, PSUM fp32):
  qkv 1x1 conv on PE -> padded slabs; depthwise 3x3x3 = 27 diag-matmul taps
  accumulated in PSUM on PE; q~,k~ transposed on PE -> [q;k] grams on PE ->
  AllReduce -> batched norm/softmax on DVE/ACT -> block-diag attn @ v on PE
  -> proj 1x1 conv on PE -> fp32 out.
"""
import numpy as np

DIM = 192
HEADS = 8
HD = DIM // HEADS  # 24
B, T, H, W = 2, 8, 8, 64  # per-core owned h rows = 8
HP, WP, TP = 10, 66, 10
SLAB = HP * WP  # 660
SLAB_AL = 664  # allocated slab cols (tap AP needs 2 spare elements)
NTOK = B * T * H * W  # 8192 owned tokens per core
NCORES = 8
C3 = 3 * DIM
NPADTOK = B * TP * SLAB  # 13200

_CACHE = {}

MTILES = [(0, 128), (128, 128), (256, 128), (384, 128), (512, 64)]
KTILES = [(0, 128), (128, 64)]
TAPS = [(dt, dh, dw) for dt in (-1, 0, 1) for dh in (-1, 0, 1)
        for dw in (-1, 0, 1)]


def _build():
    import concourse.bacc as bacc
    import concourse.mybir as mybir
    import concourse.tile as tile
    from concourse import masks

    F32 = mybir.dt.float32
    F16 = mybir.dt.float16
    AL = mybir.AluOpType
    AF = mybir.ActivationFunctionType
    AX = mybir.AxisListType

    nc = bacc.Bacc("TRN2", target_bir_lowering=False, debug=False,
                   num_devices=NCORES)

    x16 = nc.dram_tensor("x16", [DIM, NPADTOK], F16, kind="ExternalInput").ap()
    qkvwT = nc.dram_tensor("qkvwT", [DIM, C3], F16, kind="ExternalInput").ap()
    qkvb = nc.dram_tensor("qkvb", [128, 5], F32, kind="ExternalInput").ap()
    dww = nc.dram_tensor("dww", [128, 5 * 27], F32, kind="ExternalInput").ap()
    dwb = nc.dram_tensor("dwb", [128, 5], F32, kind="ExternalInput").ap()
    # proj weight transposed, [192 k, 192 m] fp16
    projwT = nc.dram_tensor("projwT", [DIM, DIM], F16, kind="ExternalInput").ap()
    projb = nc.dram_tensor("projb", [128, 2], F32, kind="ExternalInput").ap()
    temp = nc.dram_tensor("temp", [16, 1], F32, kind="ExternalInput").ap()
    out = nc.dram_tensor("out", [DIM, NTOK], F32, kind="ExternalOutput").ap()

    gram_in = nc.dram_tensor("gram_in", [16, 48, 48], F32).ap()
    gram_out = nc.dram_tensor("gram_out", [16, 48, 48], F32,
                              addr_space="Shared").ap()
    attn_dram = nc.dram_tensor("attn_dram", [16, HD, HD], F16).ap()

    with tile.TileContext(nc) as tc:
        with (
            tc.tile_pool(name="wp", bufs=1) as wp,
            tc.tile_pool(name="xp", bufs=4) as xp,
            tc.tile_pool(name="qslab", bufs=5) as slp,
            tc.tile_pool(name="qk", bufs=1) as qkpool,
            tc.tile_pool(name="ev", bufs=4) as ev,
            tc.tile_pool(name="small", bufs=1) as sp,
            tc.tile_pool(name="ps", bufs=4, space="PSUM") as psp,
            tc.tile_pool(name="pst", bufs=2, space="PSUM") as pst,
            tc.tile_pool(name="psg", bufs=1, space="PSUM") as psg,
        ):
            # ---------------- weights ----------------
            wq = []
            for ko, kc in KTILES:
                t = wp.tile([kc, C3], F16, tag=f"wq{ko}")
                nc.sync.dma_start(out=t[:], in_=qkvwT[ko:ko + kc, :])
                wq.append(t)
            qkvb_s = wp.tile([128, 5], F32, tag="qkvb")
            nc.sync.dma_start(out=qkvb_s[:], in_=qkvb)
            dww_s = wp.tile([128, 5 * 27], F32, tag="dww")
            nc.sync.dma_start(out=dww_s[:], in_=dww)
            dwb_s = wp.tile([128, 5], F32, tag="dwb")
            nc.sync.dma_start(out=dwb_s[:], in_=dwb)
            wproj = []
            for ki, (ko, kc) in enumerate(KTILES):
                t = wp.tile([kc, DIM], F16, tag=f"wproj{ki}")
                nc.sync.dma_start(out=t[:], in_=projwT[ko:ko + kc, :])
                wproj.append(t)
            projb_s = wp.tile([128, 2], F32, tag="projb")
            nc.sync.dma_start(out=projb_s[:], in_=projb)
            temp_s = wp.tile([16, 1], F32, tag="temp")
            nc.sync.dma_start(out=temp_s[:], in_=temp)

            ident16 = wp.tile([128, 128], F16, tag="ident16")
            masks.make_identity(nc, ident16[:])

            diags = {}
            for mi, (mo, mc) in enumerate(MTILES):
                for tap in range(27):
                    d = wp.tile([mc, mc], F16, tag=f"dg{mi}_{tap}")
                    nc.vector.tensor_scalar_mul(
                        d[:], ident16[:mc, :mc],
                        dww_s[:mc, 27 * mi + tap:27 * mi + tap + 1])
                    diags[(mi, tap)] = d

            # dw outputs: q~,k~ (384 ch) in 3 tiles, v (192 ch) in 2 tiles
            qk_t = [qkpool.tile([128, NTOK], F16, tag=f"qk{i}", name=f"qk{i}")
                    for i in range(3)]
            v_t = [qkpool.tile([vc, NTOK], F16, tag=f"v{i}", name=f"v{i}")
                   for i, vc in enumerate([128, 64])]

            # ---------------- qkv conv + depthwise ----------------
            def qkv_slab(b, t_, slabs):
                    xt = []
                    for ko, kc in KTILES:
                        xx = xp.tile([kc, SLAB], F16, tag=f"x{ko}")
                        off = (b * TP + t_) * SLAB
                        nc.sync.dma_start(out=xx[:],
                                          in_=x16[ko:ko + kc, off:off + SLAB])
                        xt.append(xx)
                    mts = []
                    for mi, (mo, mc) in enumerate(MTILES):
                        sl = slp.tile([mc, SLAB_AL], F16, tag=f"sl{mi}")
                        nc.vector.memset(sl[:, SLAB:SLAB_AL], 0.0)
                        for half in range(2):
                            ps = psp.tile([128, 512], F32, tag="mm")
                            for ki, (ko, kc) in enumerate(KTILES):
                                nc.tensor.matmul(
                                    ps[:mc, :330],
                                    wq[ki][:, mo:mo + mc],
                                    xt[ki][:, 330 * half:330 * (half + 1)],
                                    start=(ki == 0), stop=(ki == 1))
                            nc.scalar.activation(
                                sl[:, 330 * half:330 * (half + 1)],
                                ps[:mc, :330], AF.Identity,
                                bias=qkvb_s[:mc, mi:mi + 1], scale=1.0)
                        mts.append(sl)
                    slabs[t_] = mts

            def dw_chunk(b, t_o, slabs):
                  for half in range(2):
                    chunk = (b * T + t_o) * 2 + half
                    co = 256 * chunk
                    NS = 4 * WP  # 264-wide contiguous padded stream
                    keep = [(ti, tap) for ti, tap in enumerate(TAPS)
                            if not ((t_o == 0 and tap[0] == -1) or
                                    (t_o == T - 1 and tap[0] == 1))]
                    for mi, (mo, mc) in enumerate(MTILES):
                        ps = psp.tile([128, 512], F32, tag="mm")
                        for j, (ti, (dt, dh, dw)) in enumerate(keep):
                            src = slabs[t_o + 1 + dt][mi]
                            off = (4 * half + 1 + dh) * WP + (1 + dw)
                            nc.tensor.matmul(
                                ps[:mc, :NS], diags[(mi, ti)][:],
                                src[:, off:off + NS],
                                start=(j == 0), stop=(j == len(keep) - 1))
                        pv = ps[:, :NS].rearrange(
                            "p (h w) -> p h w", h=4)[:, :, 0:64]
                        if mi < 3:
                            nc.vector.tensor_scalar(
                                qk_t[mi][:, co:co + 256].rearrange(
                                    "p (h w) -> p h w", h=4),
                                pv[:mc], dwb_s[:mc, mi:mi + 1], None, AL.add)
                        elif mi == 3:
                            nc.vector.tensor_scalar(
                                v_t[0][:, co:co + 256].rearrange(
                                    "p (h w) -> p h w", h=4),
                                pv[:, :, :], dwb_s[:, mi:mi + 1], None, AL.add)
                        else:
                            nc.vector.tensor_scalar(
                                v_t[1][:, co:co + 256].rearrange(
                                    "p (h w) -> p h w", h=4),
                                pv[:64], dwb_s[:64, mi:mi + 1], None, AL.add)

            gps = [psg.tile([48, 384], F32, tag=f"gram{i}", name=f"gram{i}") for i in range(2)]
            for b in range(B):
                slabs = {}
                for t_ in (1, 2, 3):
                    qkv_slab(b, t_, slabs)
                for t_o in range(T):
                    dw_chunk(b, t_o, slabs)
                    if t_o + 4 <= T:
                        qkv_slab(b, t_o + 4, slabs)
                # transposes + grams for this batch, then its AllReduce --
                # batch 0's collective overlaps batch 1's depthwise work
                for c64 in range(32 * b, 32 * (b + 1)):
                    qkT = ev.tile([128, 384], F16, tag="qkT")
                    for i in range(3):
                        tps = pst.tile([128, 128], F16, tag="trps")
                        nc.tensor.transpose(
                            tps[:], qk_t[i][:, 128 * c64:128 * (c64 + 1)],
                            ident16[:])
                        nc.vector.tensor_copy(qkT[:, 128 * i:128 * (i + 1)],
                                              tps[:])
                    for h in range(HEADS):
                        z = qkT[:, 48 * h:48 * (h + 1)]
                        nc.tensor.matmul(
                            gps[b][:, 48 * h:48 * (h + 1)], z, z,
                            start=(c64 % 32 == 0 and h == 0),
                            stop=(c64 % 32 == 31 and h == HEADS - 1))
                gs = ev.tile([48, 384], F32, tag="gs")
                nc.vector.tensor_copy(gs[:], gps[b][:])
                nc.sync.dma_start(
                    out=gram_in[8 * b:8 * (b + 1)].rearrange(
                        "g c d -> c g d"),
                    in_=gs[:].rearrange("c (g d) -> c g d", g=8))
                nc.gpsimd.collective_compute(
                    "AllReduce", AL.add,
                    replica_groups=[list(range(NCORES))],
                    ins=[gram_in[8 * b:8 * (b + 1)]],
                    outs=[gram_out[8 * b:8 * (b + 1)]])

            # ---------------- norms + softmax (batched [16, .]) -------------
            qq_f = sp.tile([16, 576], F32, tag="qqf")
            kk_f = sp.tile([16, 576], F32, tag="kkf")
            qk_f = sp.tile([16, 576], F32, tag="qkf")
            nc.sync.dma_start(
                out=qq_f[:].rearrange("p (c d) -> p c d", c=24),
                in_=gram_out[:, 0:24, 0:24])
            nc.sync.dma_start(
                out=kk_f[:].rearrange("p (c d) -> p c d", c=24),
                in_=gram_out[:, 24:48, 24:48])
            nc.sync.dma_start(
                out=qk_f[:].rearrange("p (c d) -> p c d", c=24),
                in_=gram_out[:, 0:24, 24:48])

            dm = sp.tile([16, 576], F32, tag="dm")
            nc.gpsimd.memset(dm[:], 0.0)
            nc.gpsimd.affine_select(
                out=dm[:], in_=dm[:], compare_op=AL.not_equal, fill=1.0,
                base=0, pattern=[[1, 24], [-1, 24]], channel_multiplier=0)

            def diag_rsqrt(src, tag):
                t1 = sp.tile([16, 576], F32, tag=tag + "a")
                nc.vector.tensor_mul(t1[:], src[:], dm[:])
                n2 = sp.tile([16, 24], F32, tag=tag + "b")
                nc.vector.tensor_reduce(
                    n2[:], t1[:].rearrange("p (c d) -> p c d", c=24),
                    axis=AX.X, op=AL.add)
                nrm = sp.tile([16, 24], F32, tag=tag + "c")
                nc.scalar.sqrt(nrm[:], n2[:])
                nc.vector.tensor_scalar_max(nrm[:], nrm[:], 1e-12)
                r = sp.tile([16, 24], F32, tag=tag + "d")
                nc.vector.reciprocal(r[:], nrm[:])
                return r

            rq = diag_rsqrt(qq_f, "rq")
            rk = diag_rsqrt(kk_f, "rk")

            a1 = sp.tile([16, 576], F32, tag="a1")
            nc.vector.tensor_mul(
                a1[:].rearrange("p (c d) -> p c d", c=24),
                qk_f[:].rearrange("p (c d) -> p c d", c=24),
                rq[:].rearrange("p (c one) -> p c one", one=1).broadcast_to(
                    (16, 24, 24)))
            nc.vector.tensor_mul(
                a1[:].rearrange("p (c d) -> p c d", c=24),
                a1[:].rearrange("p (c d) -> p c d", c=24),
                rk[:].rearrange("p (one d) -> p one d", one=1).broadcast_to(
                    (16, 24, 24)))
            nc.vector.tensor_scalar_mul(a1[:], a1[:], temp_s[:])

            mx = sp.tile([16, 24], F32, tag="mx")
            nc.vector.tensor_reduce(
                mx[:], a1[:].rearrange("p (c d) -> p c d", c=24),
                axis=AX.X, op=AL.max)
            nc.vector.tensor_sub(
                a1[:].rearrange("p (c d) -> p c d", c=24),
                a1[:].rearrange("p (c d) -> p c d", c=24),
                mx[:].rearrange("p (c one) -> p c one", one=1).broadcast_to(
                    (16, 24, 24)))
            ex = sp.tile([16, 576], F32, tag="ex")
            nc.scalar.activation(ex[:], a1[:], AF.Exp)
            sm = sp.tile([16, 24], F32, tag="sm")
            nc.vector.tensor_reduce(
                sm[:], ex[:].rearrange("p (c d) -> p c d", c=24),
                axis=AX.X, op=AL.add)
            rs = sp.tile([16, 24], F32, tag="rs")
            nc.vector.reciprocal(rs[:], sm[:])
            at16 = sp.tile([16, 576], F16, tag="at16")
            nc.vector.tensor_mul(
                at16[:].rearrange("p (c d) -> p c d", c=24),
                ex[:].rearrange("p (c d) -> p c d", c=24),
                rs[:].rearrange("p (c one) -> p c one", one=1).broadcast_to(
                    (16, 24, 24)))
            nc.sync.dma_start(
                out=attn_dram[:],
                in_=at16[:].rearrange("p (c d) -> p c d", c=24))

            # block-diag attn^T per batch, split into k-tiles 128+64
            bd = []
            for b in range(B):
                bts = []
                for ki, (ko, kc) in enumerate(KTILES):
                    bdt = sp.tile([kc, DIM], F16, tag=f"bd{b}_{ki}")
                    nc.vector.memset(bdt[:], 0.0)
                    bts.append(bdt)
                for h in range(HEADS):
                    src_a = attn_dram[8 * b + h].rearrange("c d -> d c")
                    r0, r1 = HD * h, HD * (h + 1)
                    if r1 <= 128:
                        nc.sync.dma_start(
                            out=bts[0][r0:r1, r0:r1], in_=src_a)
                    elif r0 >= 128:
                        nc.sync.dma_start(
                            out=bts[1][r0 - 128:r1 - 128, r0:r1], in_=src_a)
                    else:  # h == 5: rows 120..144 straddle the k-tile split
                        nc.sync.dma_start(
                            out=bts[0][r0:128, r0:r1], in_=src_a[0:128 - r0])
                        nc.sync.dma_start(
                            out=bts[1][0:r1 - 128, r0:r1],
                            in_=src_a[128 - r0:HD])
                bd.append(bts)

            # ---------------- attn@v + proj ----------------
            for chunk in range(B * T):
                b = chunk // T
                co = 512 * chunk
                aos = [ev.tile([kc, 512], F16, tag=f"ao{ki}", name=f"ao{ki}")
                       for ki, (ko, kc) in enumerate(KTILES)]
                for mi, (mo, mc) in enumerate(KTILES):
                    ps = psp.tile([128, 512], F32, tag="mm")
                    for ki in range(2):
                        nc.tensor.matmul(
                            ps[:mc, :], bd[b][ki][:, mo:mo + mc],
                            v_t[ki][:, co:co + 512],
                            start=(ki == 0), stop=(ki == 1))
                    nc.vector.tensor_copy(aos[mi][:, :], ps[:mc, :])
                for mi, (mo, mc) in enumerate(KTILES):
                    ps = psp.tile([128, 512], F32, tag="mm")
                    for ki in range(2):
                        nc.tensor.matmul(
                            ps[:mc, :], wproj[ki][:, mo:mo + mc],
                            aos[ki][:, :],
                            start=(ki == 0), stop=(ki == 1))
                    of = ev.tile([128, 512], F32, tag="of")
                    nc.vector.tensor_scalar(
                        of[:mc, :], ps[:mc, :], projb_s[:mc, mi:mi + 1],
                        None, AL.add)
                    nc.sync.dma_start(out=out[mo:mo + mc, co:co + 512],
                                      in_=of[:mc, :])
    nc.compile()
    return nc


def _prep_inputs(x, qkv_w, qkv_b, dw_w, dw_b, temperature, proj_w, proj_b):
    """Host-side prep: per-core padded fp16 slabs + shared weights."""
    x = np.asarray(x, np.float32)
    b_, c_, t_, h_, w_ = x.shape  # 2, 192, 8, 64, 64
    qkv_w2 = np.asarray(qkv_w, np.float32).reshape(C3, DIM)
    dw_w2 = np.asarray(dw_w, np.float32).reshape(C3, 27)
    proj_w2 = np.asarray(proj_w, np.float32).reshape(DIM, DIM)
    # permute qkv channels: [q_h0, k_h0, q_h1, k_h1, ..., v] so each head's
    # (q,k) columns are adjacent after transpose (contiguous gram operands)
    perm = []
    for h in range(HEADS):
        perm.extend(range(HD * h, HD * (h + 1)))          # q_h
        perm.extend(range(DIM + HD * h, DIM + HD * (h + 1)))  # k_h
    perm.extend(range(2 * DIM, 3 * DIM))                  # v unchanged
    perm = np.array(perm)
    qkv_w2 = qkv_w2[perm]
    dw_w2 = dw_w2[perm]
    qkv_b = np.asarray(qkv_b, np.float32)[perm]
    dw_b = np.asarray(dw_b, np.float32)[perm]

    wqT = np.ascontiguousarray(qkv_w2.T).astype(np.float16)  # [192, 576]
    qkvb_h = np.zeros((128, 5), np.float32)
    dwb_h = np.zeros((128, 5), np.float32)
    dww_h = np.zeros((128, 5 * 27), np.float32)
    for mi, (mo, mc) in enumerate(MTILES):
        qkvb_h[:mc, mi] = np.asarray(qkv_b, np.float32)[mo:mo + mc]
        dwb_h[:mc, mi] = np.asarray(dw_b, np.float32)[mo:mo + mc]
        dww_h[:mc, 27 * mi:27 * (mi + 1)] = dw_w2[mo:mo + mc]
    # proj lhsT with contraction padded 192->2x96 (no padding needed: 96*2)
    projwT_h = np.ascontiguousarray(proj_w2.T).astype(np.float16)  # [192,192]
    projb_h = np.zeros((128, 2), np.float32)
    projb_h[:128, 0] = np.asarray(proj_b, np.float32)[0:128]
    projb_h[:64, 1] = np.asarray(proj_b, np.float32)[128:192]
    temp_h = np.tile(np.asarray(temperature, np.float32).reshape(HEADS),
                     2).reshape(16, 1)  # g = b*8+h

    in_maps = []
    for i in range(NCORES):
        # padded slab [b, t10, h10, w66], h rows 8i-1 .. 8i+9 clamped->zero
        xs = np.zeros((b_, TP, HP, WP, c_), np.float32)
        hlo, hhi = 8 * i - 1, 8 * i + 9
        slo, shi = max(0, hlo), min(h_, hhi)
        # x [b,c,t,h,w] -> [b,t,h,w,c]
        xt = x[:, :, :, slo:shi, :].transpose(0, 2, 3, 4, 1)
        xs[:, 1:9, (slo - hlo):(slo - hlo) + (shi - slo), 1:65, :] = xt
        x16 = np.ascontiguousarray(
            xs.reshape(b_ * TP * SLAB, c_).T).astype(np.float16)
        in_maps.append({
            "x16": x16, "qkvwT": wqT, "qkvb": qkvb_h, "dww": dww_h,
            "dwb": dwb_h, "projwT": projwT_h, "projb": projb_h,
            "temp": temp_h,
        })
    return in_maps


def _get_runner():
    """Build once; return a persistent sharded-jit callable (the per-call
    closure in bass2jax.run_bass_via_pjrt defeats jax's jit cache)."""
    if "runner" in _CACHE:
        return _CACHE["runner"]
    import jax
    for flag, val in [("jax_compilation_cache_dir", "/tmp/jax_kernel_cache"),
                      ("jax_persistent_cache_min_compile_time_secs", 1.0),
                      ("jax_persistent_cache_min_entry_size_bytes", 0)]:
        try:
            jax.config.update(flag, val)
        except Exception:
            pass
    import jax.numpy as jnp
    from jax.sharding import Mesh, PartitionSpec
    from jax.experimental.shard_map import shard_map
    import concourse.mybir as mybir
    from concourse import bass2jax

    nc = _build()
    bass2jax.install_neuronx_cc_hook()

    partition_name = (nc.partition_id_tensor.name
                      if nc.partition_id_tensor else None)
    in_names, out_names, out_avals, zero_shapes = [], [], [], []
    for alloc in nc.m.functions[0].allocations:
        if not isinstance(alloc, mybir.MemoryLocationSet):
            continue
        name = alloc.memorylocations[0].name
        if alloc.kind == "ExternalInput":
            if name != partition_name:
                in_names.append(name)
        elif alloc.kind == "ExternalOutput":
            shape = tuple(alloc.tensor_shape)
            dtype = mybir.dt.np(alloc.dtype)
            out_names.append(name)
            out_avals.append(jax.core.ShapedArray(shape, dtype))
            zero_shapes.append((shape, dtype))
    n_params = len(in_names)
    all_names = in_names + out_names
    if partition_name is not None:
        all_names.append(partition_name)

    def _body(*args):
        operands = list(args)
        if partition_name is not None:
            operands.append(bass2jax.partition_id_tensor())
        outs = bass2jax._bass_exec_p.bind(
            *operands, out_avals=tuple(out_avals), in_names=tuple(all_names),
            out_names=tuple(out_names), lowering_input_output_aliases=(),
            sim_require_finite=True, sim_require_nnan=True, nc=nc)
        return tuple(outs)

    devices = jax.devices()[:NCORES]
    mesh = Mesh(np.asarray(devices), ("core",))
    n_outs = len(out_names)
    sharded = jax.jit(
        shard_map(_body, mesh=mesh,
                  in_specs=(PartitionSpec("core"),) * (n_params + n_outs),
                  out_specs=(PartitionSpec("core"),) * n_outs,
                  check_rep=False),
        donate_argnums=tuple(range(n_params, n_params + n_outs)),
        keep_unused=True)

    def run(in_maps):
        concat_in = [np.concatenate([in_maps[c][nm] for c in range(NCORES)],
                                    axis=0) for nm in in_names]
        concat_zeros = [np.zeros((NCORES * s[0], *s[1:]), dt)
                        for s, dt in zero_shapes]
        out_arrs = sharded(*concat_in, *concat_zeros)
        return [
            {nm: np.asarray(out_arrs[i]).reshape(NCORES, *out_avals[i].shape)[c]
             for i, nm in enumerate(out_names)}
            for c in range(NCORES)]

    _CACHE["runner"] = run
    return run


def kernel(x, qkv_w, qkv_b, dw_w, dw_b, temperature, proj_w, proj_b):
    run = _get_runner()
    in_maps = _prep_inputs(x, qkv_w, qkv_b, dw_w, dw_b, temperature,
                           proj_w, proj_b)
    results = run(in_maps)
    b_, c_, t_, h_, w_ = np.asarray(x).shape
    outf = np.empty((b_, c_, t_, h_, w_), np.float32)
    for i in range(NCORES):
        o = results[i]["out"].reshape(c_, b_, t_, H, w_)
        outf[:, :, :, 8 * i:8 * i + 8, :] = o.transpose(1, 0, 2, 3, 4)
    return outf



# revision 10
# speedup vs baseline: 1.3431x; 1.3431x over previous
"""nn_AttentionC Trainium2 kernel (8 NeuronCores, SPMD) — fp8 DoubleRow.

Sharding: h-axis (64) split into 8 chunks of 8 rows, one per core; each core's
x slab is host-padded to [b2, t10, h10, w66] tokens (conv zero-padding baked
in). Only cross-core traffic: AllReduce of per-(b,head) [48,48] q/k gram
matrices (110 KB).

Precision scheme (validated vs fp32 reference, rel_err ~2e-3):
  - weights everywhere: fp8e4m3 hi+lo (pre-scaled x32; coherent quant noise
    does not average out, so 2 words needed)
  - qkv input x: fp8 hi+lo from host (v-path needs the lo word)
  - q,k dw slabs + gram operands: single fp8 (x8 / x32 pre-scale) — random
    per-element noise washes out in the 32k-token gram contraction
  - v path slabs: fp8 hi+lo (x8 pre-scale); v output, attn, proj: fp16
All fp8 matmuls use DoubleRow perf mode (2 K-tiles/instr, 0.5 cyc/row):
  qkv: (whi|wlo)@xhi [+ whi@xlo for v tiles], contraction 192 split 96+96
  dw:  per tap (whi|wlo)-diag @ stride-0-paired window; v adds whi@xlo with
       taps paired two-per-instr via manually built two-offset APs
  gram: two 128-token tiles per instr.
"""
import numpy as np

DIM = 192
HEADS = 8
HD = DIM // HEADS  # 24
B, T, H, W = 2, 8, 8, 64  # per-core owned h rows = 8
HP, WP, TP = 10, 66, 10
SLAB = HP * WP  # 660
SLAB_AL = 664  # allocated slab stride (aligned)
NTOK = B * T * H * W  # 8192 owned tokens per core
NCORES = 8
C3 = 3 * DIM
NPADTOK = B * TP * SLAB  # 13200
NPT = TP * SLAB_AL  # 6640 padded cols per (b) big slab tile

_CACHE = {}

MTILES = [(0, 128), (128, 128), (256, 128), (384, 128), (512, 64)]
QK_MIS = (0, 1, 2)
V_MIS = (3, 4)
TAPS = [(dt, dh, dw) for dt in (-1, 0, 1) for dh in (-1, 0, 1)
        for dw in (-1, 0, 1)]


def _tap_idx(dt, i):
    return (dt + 1) * 9 + i


# xlo-pass tap pairings (pairs of tap indices; second None = zero k-tile)
def _xlo_pairs(kind):
    if kind == "int":
        prs = [(_tap_idx(-1, i), _tap_idx(0, i)) for i in range(9)]
        prs += [(_tap_idx(1, 2 * k), _tap_idx(1, 2 * k + 1)) for k in range(4)]
        prs += [(_tap_idx(1, 8), None)]
    elif kind == "e0":
        prs = [(_tap_idx(0, i), _tap_idx(1, i)) for i in range(9)]
    else:  # e7
        prs = [(_tap_idx(-1, i), _tap_idx(0, i)) for i in range(9)]
    return prs


# ordered unique pair keys across all chunk kinds
_PAIR_KEYS = []
for _kind in ("int", "e0", "e7"):
    for _p in _xlo_pairs(_kind):
        if _p not in _PAIR_KEYS:
            _PAIR_KEYS.append(_p)
_PAIR_POS = {p: i for i, p in enumerate(_PAIR_KEYS)}
NPAIRS = len(_PAIR_KEYS)


def _win_off(t_slab, dh, dw, half):
    """Column offset of a 264-wide tap window in the per-b big slab tile."""
    return SLAB_AL * t_slab + WP * (1 + dh + 4 * half) + (1 + dw)


def _build():
    import concourse.bacc as bacc
    import concourse.mybir as mybir
    import concourse.tile as tile
    from concourse import masks
    from concourse.ap import AP as APc

    F32 = mybir.dt.float32
    F16 = mybir.dt.float16
    F8 = mybir.dt.float8e4
    AL = mybir.AluOpType
    AF = mybir.ActivationFunctionType
    AX = mybir.AxisListType
    PM = mybir.MatmulPerfMode

    NW = 264  # tap window width (4 h-rows x 66)

    def pair_ap(full, off_a, off_b, n=NW):
        """[p, 2, n] AP over `full` (a [p, X] AP) with windows at two offsets."""
        base = full[:, off_a:off_a + n]
        return APc(base.tensor, base.offset,
                   [list(d) for d in base.ap[:1]] + [[off_b - off_a, 2]] +
                   [list(d) for d in base.ap[1:]])

    nc = bacc.Bacc("TRN2", target_bir_lowering=False, debug=False,
                   num_devices=NCORES)

    # ---- dram inputs ----
    x8hi = nc.dram_tensor("x8hi", [96, 2 * NPADTOK], F8,
                          kind="ExternalInput").ap()
    x8lo = nc.dram_tensor("x8lo", [96, 2 * NPADTOK], F8,
                          kind="ExternalInput").ap()
    wq8hi = nc.dram_tensor("wq8hi", [96, 2 * C3], F8,
                           kind="ExternalInput").ap()
    wq8lo = nc.dram_tensor("wq8lo", [96, 2 * C3], F8,
                           kind="ExternalInput").ap()
    qkvb8 = nc.dram_tensor("qkvb8", [128, 5], F32, kind="ExternalInput").ap()
    # prebuilt diag tiles: per mtile, 27 x [mc, 2mc] (whi|wlo)
    dgd = [nc.dram_tensor(f"dg{mi}", [mc, 27 * 2 * mc], F8,
                          kind="ExternalInput").ap()
           for mi, (mo, mc) in enumerate(MTILES)]
    # xlo-pass pair diags for v mtiles: NPAIRS x [mc, 2mc] (whi_a|whi_b)
    dgp = {mi: nc.dram_tensor(f"dgp{mi}", [MTILES[mi][1],
                                           NPAIRS * 2 * MTILES[mi][1]], F8,
                              kind="ExternalInput").ap()
           for mi in V_MIS}
    dwbqk = nc.dram_tensor("dwbqk", [128, 3], F32, kind="ExternalInput").ap()
    dwbv = nc.dram_tensor("dwbv", [128, 2], F32, kind="ExternalInput").ap()
    projwT = nc.dram_tensor("projwT", [DIM, DIM], F16,
                            kind="ExternalInput").ap()
    projb = nc.dram_tensor("projb", [128, 2], F32, kind="ExternalInput").ap()
    temp = nc.dram_tensor("temp", [16, 1], F32, kind="ExternalInput").ap()
    out = nc.dram_tensor("out", [DIM, NTOK], F32, kind="ExternalOutput").ap()

    gram_in = nc.dram_tensor("gram_in", [16, 48, 48], F32).ap()
    gram_out = nc.dram_tensor("gram_out", [16, 48, 48], F32,
                              addr_space="Shared").ap()
    attn_dram = nc.dram_tensor("attn_dram", [16, HD, HD], F16).ap()

    KTILES = [(0, 128), (128, 64)]  # fp16 contraction split (proj/attn@v)

    with tile.TileContext(nc) as tc:
        with (
            tc.tile_pool(name="wp", bufs=1) as wp,
            tc.tile_pool(name="xp", bufs=3) as xp,
            tc.tile_pool(name="slb", bufs=1) as slb,
            tc.tile_pool(name="qk", bufs=1) as qkpool,
            tc.tile_pool(name="ev", bufs=4) as ev,
            tc.tile_pool(name="evh", bufs=2) as evh,
            tc.tile_pool(name="small", bufs=1) as sp,
            tc.tile_pool(name="ps", bufs=4, space="PSUM") as psp,
            tc.tile_pool(name="pst", bufs=2, space="PSUM") as pst,
            tc.tile_pool(name="psg", bufs=1, space="PSUM") as psg,
        ):
            # ---------------- weights ----------------
            wqhi = wp.tile([96, 2 * C3], F8, tag="wqhi")
            nc.sync.dma_start(out=wqhi[:], in_=wq8hi)
            wqlo = wp.tile([96, 2 * C3], F8, tag="wqlo")
            nc.sync.dma_start(out=wqlo[:], in_=wq8lo)
            qkvb_s = wp.tile([128, 5], F32, tag="qkvb")
            nc.sync.dma_start(out=qkvb_s[:], in_=qkvb8)
            dg_s = []
            for mi, (mo, mc) in enumerate(MTILES):
                t = wp.tile([mc, 27 * 2 * mc], F8, tag=f"dg{mi}")
                nc.sync.dma_start(out=t[:], in_=dgd[mi])
                dg_s.append(t)
            dgp_s = {}
            for mi in V_MIS:
                mc = MTILES[mi][1]
                t = wp.tile([mc, NPAIRS * 2 * mc], F8, tag=f"dgp{mi}")
                nc.sync.dma_start(out=t[:], in_=dgp[mi])
                dgp_s[mi] = t
            dwbqk_s = wp.tile([128, 3], F32, tag="dwbqk")
            nc.sync.dma_start(out=dwbqk_s[:], in_=dwbqk)
            dwbv_s = wp.tile([128, 2], F32, tag="dwbv")
            nc.sync.dma_start(out=dwbv_s[:], in_=dwbv)
            wproj = []
            for ki, (ko, kc) in enumerate(KTILES):
                t = wp.tile([kc, DIM], F16, tag=f"wproj{ki}")
                nc.sync.dma_start(out=t[:], in_=projwT[ko:ko + kc, :])
                wproj.append(t)
            projb_s = wp.tile([128, 2], F32, tag="projb")
            nc.sync.dma_start(out=projb_s[:], in_=projb)
            temp_s = wp.tile([16, 1], F32, tag="temp")
            nc.sync.dma_start(out=temp_s[:], in_=temp)

            ident8 = wp.tile([128, 128], F8, tag="ident8")
            masks.make_identity(nc, ident8[:])

            def dg_ap(mi, ti):
                mc = MTILES[mi][1]
                return dg_s[mi][:, ti * 2 * mc:(ti + 1) * 2 * mc].rearrange(
                    "p (two m) -> p two m", two=2)

            def dgp_ap(mi, pos):
                mc = MTILES[mi][1]
                return dgp_s[mi][:, pos * 2 * mc:(pos + 1) * 2 * mc].rearrange(
                    "p (two m) -> p two m", two=2)

            # dw outputs: q~,k~ (384 ch) fp8 x32, one batch at a time since
            # transposes+grams drain per batch; v (192 ch) fp16 both batches
            NTOKB = NTOK // B  # 4096
            qk_t = [qkpool.tile([128, NTOKB], F8, tag=f"qk{i}", name=f"qk{i}")
                    for i in range(3)]
            v_t = [qkpool.tile([vc, NTOK], F16, tag=f"v{i}", name=f"v{i}")
                   for i, vc in enumerate([128, 64])]

            # per-b big slab tiles (all 10 t-slabs contiguous, stride 664)
            sl_hi = [slb.tile([MTILES[mi][1], NPT], F8, tag=f"slhi{mi}",
                              name=f"slhi{mi}") for mi in range(5)]
            sl_lo = {mi: slb.tile([MTILES[mi][1], NPT], F8, tag=f"sllo{mi}",
                                  name=f"sllo{mi}") for mi in V_MIS}

            # ---------------- qkv conv ----------------
            def qkv_slab(b, t_):
                xhi = xp.tile([96, 2 * SLAB_AL], F8, tag="xhi")
                xlo = xp.tile([96, 2 * SLAB_AL], F8, tag="xlo")
                off = (b * TP + t_) * SLAB
                for src, dst in ((x8hi, xhi), (x8lo, xlo)):
                    nc.sync.dma_start(
                        out=dst[:].rearrange("p (two n) -> p two n",
                                             two=2)[:, :, 0:SLAB],
                        in_=src.rearrange("p (two n) -> p two n",
                                          two=2)[:, :, off:off + SLAB])
                xhi2 = xhi[:].rearrange("p (two n) -> p two n", two=2)
                xlo2 = xlo[:].rearrange("p (two n) -> p two n", two=2)
                for mi, (mo, mc) in enumerate(MTILES):
                    whi2 = wqhi[:].rearrange("p (two m) -> p two m",
                                             two=2)[:, :, mo:mo + mc]
                    wlo2 = wqlo[:].rearrange("p (two m) -> p two m",
                                             two=2)[:, :, mo:mo + mc]
                    for half in range(2):
                        cs = 330 * half
                        ce = cs + 330
                        ps = psp.tile([128, 512], F32, tag="mm")
                        three = mi in V_MIS
                        nc.tensor.matmul(ps[:mc, :330], whi2,
                                         xhi2[:, :, cs:ce], start=True,
                                         stop=False, perf_mode=PM.DoubleRow)
                        nc.tensor.matmul(ps[:mc, :330], wlo2,
                                         xhi2[:, :, cs:ce], start=False,
                                         stop=not three,
                                         perf_mode=PM.DoubleRow)
                        if three:
                            nc.tensor.matmul(ps[:mc, :330], whi2,
                                             xlo2[:, :, cs:ce], start=False,
                                             stop=True,
                                             perf_mode=PM.DoubleRow)
                        dst = SLAB_AL * t_ + cs
                        # hi slab: 8*(qkv) = psum/4 + 8b
                        nc.scalar.activation(
                            sl_hi[mi][:, dst:dst + 330], ps[:mc, :330],
                            AF.Identity, bias=qkvb_s[:mc, mi:mi + 1],
                            scale=0.25)
                        if three:
                            half32 = evh.tile([mc, 330], F32, tag=f"h32_{mi}")
                            nc.scalar.activation(
                                half32[:], ps[:mc, :330], AF.Identity,
                                bias=qkvb_s[:mc, mi:mi + 1], scale=0.25)
                            nc.vector.tensor_sub(
                                sl_lo[mi][:, dst:dst + 330], half32[:],
                                sl_hi[mi][:, dst:dst + 330])

            # ---------------- depthwise ----------------
            def dw_chunk(b, t_o):
                if t_o == 0:
                    kind, dts = "e0", (0, 1)
                elif t_o == T - 1:
                    kind, dts = "e7", (-1, 0)
                else:
                    kind, dts = "int", (-1, 0, 1)
                keep = [(dt + 1) * 9 + i for dt in dts for i in range(9)]
                xpairs = _xlo_pairs(kind)
                for half in range(2):
                    co = 256 * ((b * T + t_o) * 2 + half)      # v tiles
                    cob = 256 * (t_o * 2 + half)               # qk (per-batch)
                    for mi, (mo, mc) in enumerate(MTILES):
                        ps = psp.tile([128, 512], F32, tag="mm")
                        last_hi = (mi not in V_MIS)
                        for j, ti in enumerate(keep):
                            dt, dh, dw = TAPS[ti]
                            off = _win_off(t_o + 1 + dt, dh, dw, half)
                            rhs = pair_ap(sl_hi[mi][:], off, off)
                            nc.tensor.matmul(
                                ps[:mc, :NW], dg_ap(mi, ti), rhs,
                                start=(j == 0),
                                stop=(last_hi and j == len(keep) - 1),
                                perf_mode=PM.DoubleRow)
                        if not last_hi:
                            for j, (ta, tb) in enumerate(xpairs):
                                dta, dha, dwa = TAPS[ta]
                                offa = _win_off(t_o + 1 + dta, dha, dwa, half)
                                if tb is None:
                                    offb = offa
                                else:
                                    dtb, dhb, dwb_ = TAPS[tb]
                                    offb = _win_off(t_o + 1 + dtb, dhb, dwb_,
                                                    half)
                                rhs = pair_ap(sl_lo[mi][:], offa, offb)
                                nc.tensor.matmul(
                                    ps[:mc, :NW],
                                    dgp_ap(mi, _PAIR_POS[(ta, tb)]), rhs,
                                    start=False, stop=(j == len(xpairs) - 1),
                                    perf_mode=PM.DoubleRow)
                        pv = ps[:, :NW].rearrange(
                            "p (h w) -> p h w", h=4)[:, :, 0:64]
                        if mi in QK_MIS:
                            # qk8 = 32*(dw+b) = psum/8 + 32b
                            nc.scalar.activation(
                                qk_t[mi][:, cob:cob + 256].rearrange(
                                    "p (h w) -> p h w", h=4),
                                pv[:mc], AF.Identity,
                                bias=dwbqk_s[:mc, mi:mi + 1], scale=0.125)
                        elif mi == 3:
                            nc.scalar.activation(
                                v_t[0][:, co:co + 256].rearrange(
                                    "p (h w) -> p h w", h=4),
                                pv[:mc], AF.Identity,
                                bias=dwbv_s[:mc, 0:1], scale=1.0 / 256)
                        else:
                            nc.scalar.activation(
                                v_t[1][:, co:co + 256].rearrange(
                                    "p (h w) -> p h w", h=4),
                                pv[:64], AF.Identity,
                                bias=dwbv_s[:64, 1:2], scale=1.0 / 256)

            gps = [psg.tile([48, 384], F32, tag=f"gram{i}", name=f"gram{i}")
                   for i in range(2)]
            for b in range(B):
                for t_ in (1, 2, 3):
                    qkv_slab(b, t_)
                for t_o in range(T):
                    dw_chunk(b, t_o)
                    if t_o + 4 <= T:
                        qkv_slab(b, t_o + 4)
                # transposes + grams (c64 pairs), then AllReduce; batch 0's
                # collective overlaps batch 1's depthwise work
                for pr in range(16):
                    c64a = 2 * pr  # qk_t holds the current batch only
                    qkT = ev.tile([128, 768], F8, tag="qkT")
                    for s in range(2):
                        for i in range(3):
                            # fp8 transpose requires output element step 2
                            tps = pst.tile([128, 256], F8, tag="trps")
                            nc.tensor.transpose(
                                tps[:, 0:256:2],
                                qk_t[i][:, 128 * (c64a + s):
                                        128 * (c64a + s + 1)],
                                ident8[:])
                            nc.vector.tensor_copy(
                                qkT[:, 384 * s + 128 * i:
                                    384 * s + 128 * (i + 1)],
                                tps[:, 0:256:2])
                    z2 = qkT[:].rearrange("p (two m) -> p two m", two=2)
                    for h in range(HEADS):
                        z = z2[:, :, 48 * h:48 * (h + 1)]
                        nc.tensor.matmul(
                            gps[b][:, 48 * h:48 * (h + 1)], z, z,
                            start=(pr == 0 and h == 0),
                            stop=(pr == 15 and h == HEADS - 1),
                            perf_mode=PM.DoubleRow)
                gs = ev.tile([48, 384], F32, tag="gs")
                nc.vector.tensor_copy(gs[:], gps[b][:])
                nc.sync.dma_start(
                    out=gram_in[8 * b:8 * (b + 1)].rearrange(
                        "g c d -> c g d"),
                    in_=gs[:].rearrange("c (g d) -> c g d", g=8))
                nc.gpsimd.collective_compute(
                    "AllReduce", AL.add,
                    replica_groups=[list(range(NCORES))],
                    ins=[gram_in[8 * b:8 * (b + 1)]],
                    outs=[gram_out[8 * b:8 * (b + 1)]])

            # ---------------- norms + softmax (batched [16, .]) -------------
            # grams carry a 1024x scale (32q)(32k); it cancels in the
            # normalized correlation exactly.
            qq_f = sp.tile([16, 576], F32, tag="qqf")
            kk_f = sp.tile([16, 576], F32, tag="kkf")
            qk_f = sp.tile([16, 576], F32, tag="qkf")
            nc.sync.dma_start(
                out=qq_f[:].rearrange("p (c d) -> p c d", c=24),
                in_=gram_out[:, 0:24, 0:24])
            nc.sync.dma_start(
                out=kk_f[:].rearrange("p (c d) -> p c d", c=24),
                in_=gram_out[:, 24:48, 24:48])
            nc.sync.dma_start(
                out=qk_f[:].rearrange("p (c d) -> p c d", c=24),
                in_=gram_out[:, 0:24, 24:48])

            dm = sp.tile([16, 576], F32, tag="dm")
            nc.gpsimd.memset(dm[:], 0.0)
            nc.gpsimd.affine_select(
                out=dm[:], in_=dm[:], compare_op=AL.not_equal, fill=1.0,
                base=0, pattern=[[1, 24], [-1, 24]], channel_multiplier=0)

            def diag_rsqrt(src, tag):
                t1 = sp.tile([16, 576], F32, tag=tag + "a")
                nc.vector.tensor_mul(t1[:], src[:], dm[:])
                n2 = sp.tile([16, 24], F32, tag=tag + "b")
                nc.vector.tensor_reduce(
                    n2[:], t1[:].rearrange("p (c d) -> p c d", c=24),
                    axis=AX.X, op=AL.add)
                nrm = sp.tile([16, 24], F32, tag=tag + "c")
                nc.scalar.sqrt(nrm[:], n2[:])
                nc.vector.tensor_scalar_max(nrm[:], nrm[:], 1e-12)
                r = sp.tile([16, 24], F32, tag=tag + "d")
                nc.vector.reciprocal(r[:], nrm[:])
                return r

            rq = diag_rsqrt(qq_f, "rq")
            rk = diag_rsqrt(kk_f, "rk")

            a1 = sp.tile([16, 576], F32, tag="a1")
            nc.vector.tensor_mul(
                a1[:].rearrange("p (c d) -> p c d", c=24),
                qk_f[:].rearrange("p (c d) -> p c d", c=24),
                rq[:].rearrange("p (c one) -> p c one", one=1).broadcast_to(
                    (16, 24, 24)))
            nc.vector.tensor_mul(
                a1[:].rearrange("p (c d) -> p c d", c=24),
                a1[:].rearrange("p (c d) -> p c d", c=24),
                rk[:].rearrange("p (one d) -> p one d", one=1).broadcast_to(
                    (16, 24, 24)))
            nc.vector.tensor_scalar_mul(a1[:], a1[:], temp_s[:])

            mx = sp.tile([16, 24], F32, tag="mx")
            nc.vector.tensor_reduce(
                mx[:], a1[:].rearrange("p (c d) -> p c d", c=24),
                axis=AX.X, op=AL.max)
            nc.vector.tensor_sub(
                a1[:].rearrange("p (c d) -> p c d", c=24),
                a1[:].rearrange("p (c d) -> p c d", c=24),
                mx[:].rearrange("p (c one) -> p c one", one=1).broadcast_to(
                    (16, 24, 24)))
            ex = sp.tile([16, 576], F32, tag="ex")
            nc.scalar.activation(ex[:], a1[:], AF.Exp)
            sm = sp.tile([16, 24], F32, tag="sm")
            nc.vector.tensor_reduce(
                sm[:], ex[:].rearrange("p (c d) -> p c d", c=24),
                axis=AX.X, op=AL.add)
            rs = sp.tile([16, 24], F32, tag="rs")
            nc.vector.reciprocal(rs[:], sm[:])
            at16 = sp.tile([16, 576], F16, tag="at16")
            nc.vector.tensor_mul(
                at16[:].rearrange("p (c d) -> p c d", c=24),
                ex[:].rearrange("p (c d) -> p c d", c=24),
                rs[:].rearrange("p (c one) -> p c one", one=1).broadcast_to(
                    (16, 24, 24)))
            nc.sync.dma_start(
                out=attn_dram[:],
                in_=at16[:].rearrange("p (c d) -> p c d", c=24))

            # block-diag attn^T per batch, split into k-tiles 128+64
            bd = []
            for b in range(B):
                bts = []
                for ki, (ko, kc) in enumerate(KTILES):
                    bdt = sp.tile([kc, DIM], F16, tag=f"bd{b}_{ki}")
                    nc.vector.memset(bdt[:], 0.0)
                    bts.append(bdt)
                for h in range(HEADS):
                    src_a = attn_dram[8 * b + h].rearrange("c d -> d c")
                    r0, r1 = HD * h, HD * (h + 1)
                    if r1 <= 128:
                        nc.sync.dma_start(
                            out=bts[0][r0:r1, r0:r1], in_=src_a)
                    elif r0 >= 128:
                        nc.sync.dma_start(
                            out=bts[1][r0 - 128:r1 - 128, r0:r1], in_=src_a)
                    else:  # h == 5: rows 120..144 straddle the k-tile split
                        nc.sync.dma_start(
                            out=bts[0][r0:128, r0:r1], in_=src_a[0:128 - r0])
                        nc.sync.dma_start(
                            out=bts[1][0:r1 - 128, r0:r1],
                            in_=src_a[128 - r0:HD])
                bd.append(bts)

            # ---------------- attn@v + proj ----------------
            for chunk in range(B * T):
                b = chunk // T
                co = 512 * chunk
                aos = [ev.tile([kc, 512], F16, tag=f"ao{ki}", name=f"ao{ki}")
                       for ki, (ko, kc) in enumerate(KTILES)]
                for mi, (mo, mc) in enumerate(KTILES):
                    ps = psp.tile([128, 512], F32, tag="mm")
                    for ki in range(2):
                        nc.tensor.matmul(
                            ps[:mc, :], bd[b][ki][:, mo:mo + mc],
                            v_t[ki][:, co:co + 512],
                            start=(ki == 0), stop=(ki == 1))
                    nc.vector.tensor_copy(aos[mi][:, :], ps[:mc, :])
                for mi, (mo, mc) in enumerate(KTILES):
                    ps = psp.tile([128, 512], F32, tag="mm")
                    for ki in range(2):
                        nc.tensor.matmul(
                            ps[:mc, :], wproj[ki][:, mo:mo + mc],
                            aos[ki][:, :],
                            start=(ki == 0), stop=(ki == 1))
                    of = ev.tile([128, 512], F32, tag="of")
                    nc.vector.tensor_scalar(
                        of[:mc, :], ps[:mc, :], projb_s[:mc, mi:mi + 1],
                        None, AL.add)
                    nc.sync.dma_start(out=out[mo:mo + mc, co:co + 512],
                                      in_=of[:mc, :])
    nc.compile()
    return nc


def _prep_inputs(x, qkv_w, qkv_b, dw_w, dw_b, temperature, proj_w, proj_b):
    """Host-side prep: fp8 hi/lo inputs, prebuilt fp8 diag tiles, fp16 proj."""
    import ml_dtypes
    F8NP = ml_dtypes.float8_e4m3

    def q8(a):
        return np.asarray(a, np.float32).astype(F8NP)

    x = np.asarray(x, np.float32)
    b_, c_, t_, h_, w_ = x.shape  # 2, 192, 8, 64, 64
    qkv_w2 = np.asarray(qkv_w, np.float32).reshape(C3, DIM)
    dw_w2 = np.asarray(dw_w, np.float32).reshape(C3, 27)
    proj_w2 = np.asarray(proj_w, np.float32).reshape(DIM, DIM)
    # permute qkv channels: [q_h0, k_h0, q_h1, k_h1, ..., v] so each head's
    # (q,k) columns are adjacent after transpose (contiguous gram operands)
    perm = []
    for h in range(HEADS):
        perm.extend(range(HD * h, HD * (h + 1)))          # q_h
        perm.extend(range(DIM + HD * h, DIM + HD * (h + 1)))  # k_h
    perm.extend(range(2 * DIM, 3 * DIM))                  # v unchanged
    perm = np.array(perm)
    qkv_w2 = qkv_w2[perm]
    dw_w2 = dw_w2[perm]
    qkv_b = np.asarray(qkv_b, np.float32)[perm]
    dw_b = np.asarray(dw_b, np.float32)[perm]

    # qkv weights x32, hi/lo, laid out [96, 2, 576] -> [96, 1152]
    w32 = 32.0 * qkv_w2  # [576, 192]
    whi = q8(w32).astype(np.float32)
    wlo = q8(w32 - whi).astype(np.float32)

    def wq_layout(w):  # [576(out), 192(in)] -> [96, 2*576] fp8
        wt = np.ascontiguousarray(w.T)          # [192, 576]
        return np.ascontiguousarray(
            wt.reshape(2, 96, C3).transpose(1, 0, 2).reshape(96, 2 * C3)
        ).astype(F8NP)

    wq8hi_h = wq_layout(whi)
    wq8lo_h = wq_layout(wlo)

    qkvb8_h = np.zeros((128, 5), np.float32)
    for mi, (mo, mc) in enumerate(MTILES):
        qkvb8_h[:mc, mi] = 8.0 * qkv_b[mo:mo + mc]

    # dw diag tiles (whi|wlo) per tap, x32
    d32 = 32.0 * dw_w2  # [576, 27]
    dhi = q8(d32).astype(np.float32)
    dlo = q8(d32 - dhi).astype(np.float32)
    dgd_h = []
    for mi, (mo, mc) in enumerate(MTILES):
        t = np.zeros((mc, 27 * 2 * mc), np.float32)
        idx = np.arange(mc)
        for ti in range(27):
            t[idx, ti * 2 * mc + idx] = dhi[mo:mo + mc, ti]
            t[idx, ti * 2 * mc + mc + idx] = dlo[mo:mo + mc, ti]
        dgd_h.append(t.astype(F8NP))
    dgp_h = {}
    for mi in V_MIS:
        mo, mc = MTILES[mi]
        t = np.zeros((mc, NPAIRS * 2 * mc), np.float32)
        idx = np.arange(mc)
        for pos, (ta, tb) in enumerate(_PAIR_KEYS):
            t[idx, pos * 2 * mc + idx] = dhi[mo:mo + mc, ta]
            if tb is not None:
                t[idx, pos * 2 * mc + mc + idx] = dhi[mo:mo + mc, tb]
        dgp_h[mi] = t.astype(F8NP)

    dwbqk_h = np.zeros((128, 3), np.float32)
    for mi in QK_MIS:
        mo, mc = MTILES[mi]
        dwbqk_h[:mc, mi] = 32.0 * dw_b[mo:mo + mc]
    dwbv_h = np.zeros((128, 2), np.float32)
    for j, mi in enumerate(V_MIS):
        mo, mc = MTILES[mi]
        dwbv_h[:mc, j] = dw_b[mo:mo + mc]

    projwT_h = np.ascontiguousarray(proj_w2.T).astype(np.float16)
    projb_h = np.zeros((128, 2), np.float32)
    projb_h[:128, 0] = np.asarray(proj_b, np.float32)[0:128]
    projb_h[:64, 1] = np.asarray(proj_b, np.float32)[128:192]
    temp_h = np.tile(np.asarray(temperature, np.float32).reshape(HEADS),
                     2).reshape(16, 1)  # g = b*8+h

    in_maps = []
    for i in range(NCORES):
        # padded slab [b, t10, h10, w66], h rows 8i-1 .. 8i+9 clamped->zero
        xs = np.zeros((b_, TP, HP, WP, c_), np.float32)
        hlo, hhi = 8 * i - 1, 8 * i + 9
        slo, shi = max(0, hlo), min(h_, hhi)
        xt = x[:, :, :, slo:shi, :].transpose(0, 2, 3, 4, 1)
        xs[:, 1:9, (slo - hlo):(slo - hlo) + (shi - slo), 1:65, :] = xt
        xT = np.ascontiguousarray(
            xs.reshape(b_ * TP * SLAB, c_).T)  # [192, 13200] f32
        xhi = q8(xT).astype(np.float32)
        xlo_ = q8(xT - xhi)

        def x_layout(a):  # [192, NPADTOK] -> [96, 2*NPADTOK] fp8
            return np.ascontiguousarray(
                np.asarray(a, np.float32).reshape(2, 96, NPADTOK)
                .transpose(1, 0, 2).reshape(96, 2 * NPADTOK)).astype(F8NP)

        m = {
            "x8hi": x_layout(xhi), "x8lo": x_layout(xlo_),
            "wq8hi": wq8hi_h, "wq8lo": wq8lo_h, "qkvb8": qkvb8_h,
            "dwbqk": dwbqk_h, "dwbv": dwbv_h,
            "projwT": projwT_h, "projb": projb_h, "temp": temp_h,
        }
        for mi in range(5):
            m[f"dg{mi}"] = dgd_h[mi]
        for mi in V_MIS:
            m[f"dgp{mi}"] = dgp_h[mi]
        in_maps.append(m)
    return in_maps


def _get_runner():
    """Build once; return a persistent sharded-jit callable (the per-call
    closure in bass2jax.run_bass_via_pjrt defeats jax's jit cache)."""
    if "runner" in _CACHE:
        return _CACHE["runner"]
    import jax
    for flag, val in [("jax_compilation_cache_dir", "/tmp/jax_kernel_cache"),
                      ("jax_persistent_cache_min_compile_time_secs", 1.0),
                      ("jax_persistent_cache_min_entry_size_bytes", 0)]:
        try:
            jax.config.update(flag, val)
        except Exception:
            pass
    from jax.sharding import Mesh, PartitionSpec
    from jax.experimental.shard_map import shard_map
    import concourse.mybir as mybir
    from concourse import bass2jax

    nc = _build()
    bass2jax.install_neuronx_cc_hook()

    partition_name = (nc.partition_id_tensor.name
                      if nc.partition_id_tensor else None)
    in_names, out_names, out_avals, zero_shapes = [], [], [], []
    for alloc in nc.m.functions[0].allocations:
        if not isinstance(alloc, mybir.MemoryLocationSet):
            continue
        name = alloc.memorylocations[0].name
        if alloc.kind == "ExternalInput":
            if name != partition_name:
                in_names.append(name)
        elif alloc.kind == "ExternalOutput":
            shape = tuple(alloc.tensor_shape)
            dtype = mybir.dt.np(alloc.dtype)
            out_names.append(name)
            out_avals.append(jax.core.ShapedArray(shape, dtype))
            zero_shapes.append((shape, dtype))
    n_params = len(in_names)
    all_names = in_names + out_names
    if partition_name is not None:
        all_names.append(partition_name)

    def _body(*args):
        operands = list(args)
        if partition_name is not None:
            operands.append(bass2jax.partition_id_tensor())
        outs = bass2jax._bass_exec_p.bind(
            *operands, out_avals=tuple(out_avals), in_names=tuple(all_names),
            out_names=tuple(out_names), lowering_input_output_aliases=(),
            sim_require_finite=True, sim_require_nnan=True, nc=nc)
        return tuple(outs)

    devices = jax.devices()[:NCORES]
    mesh = Mesh(np.asarray(devices), ("core",))
    n_outs = len(out_names)
    sharded = jax.jit(
        shard_map(_body, mesh=mesh,
                  in_specs=(PartitionSpec("core"),) * (n_params + n_outs),
                  out_specs=(PartitionSpec("core"),) * n_outs,
                  check_rep=False),
        donate_argnums=tuple(range(n_params, n_params + n_outs)),
        keep_unused=True)

    def run(in_maps):
        concat_in = [np.concatenate([in_maps[c][nm] for c in range(NCORES)],
                                    axis=0) for nm in in_names]
        concat_zeros = [np.zeros((NCORES * s[0], *s[1:]), dt)
                        for s, dt in zero_shapes]
        out_arrs = sharded(*concat_in, *concat_zeros)
        return [
            {nm: np.asarray(out_arrs[i]).reshape(NCORES, *out_avals[i].shape)[c]
             for i, nm in enumerate(out_names)}
            for c in range(NCORES)]

    _CACHE["runner"] = run
    return run


def kernel(x, qkv_w, qkv_b, dw_w, dw_b, temperature, proj_w, proj_b):
    run = _get_runner()
    in_maps = _prep_inputs(x, qkv_w, qkv_b, dw_w, dw_b, temperature,
                           proj_w, proj_b)
    results = run(in_maps)
    b_, c_, t_, h_, w_ = np.asarray(x).shape
    outf = np.empty((b_, c_, t_, h_, w_), np.float32)
    for i in range(NCORES):
        o = results[i]["out"].reshape(c_, b_, t_, H, w_)
        outf[:, :, :, 8 * i:8 * i + 8, :] = o.transpose(1, 0, 2, 3, 4)
    return outf


# revision 13
# speedup vs baseline: 1.3498x; 1.0050x over previous
"""nn_AttentionC Trainium2 kernel (8 NeuronCores, SPMD) — fp8 DoubleRow.

Sharding: h-axis (64) split into 8 chunks of 8 rows, one per core; each core's
x slab is host-padded to [b2, t10, h10, w66] tokens (conv zero-padding baked
in). Only cross-core traffic: AllReduce of per-(b,head) [48,48] q/k gram
matrices (110 KB).

Precision scheme (validated vs fp32 reference, rel_err ~2e-3):
  - weights everywhere: fp8e4m3 hi+lo (pre-scaled x32; coherent quant noise
    does not average out, so 2 words needed)
  - qkv input x: fp8 hi+lo from host (v-path needs the lo word)
  - q,k dw slabs + gram operands: single fp8 (x8 / x32 pre-scale) — random
    per-element noise washes out in the 32k-token gram contraction
  - v path slabs: fp8 hi+lo (x8 pre-scale); v output, attn, proj: fp16
All fp8 matmuls use DoubleRow perf mode (2 K-tiles/instr, 0.5 cyc/row):
  qkv: (whi|wlo)@xhi [+ whi@xlo for v tiles], contraction 192 split 96+96
  dw:  per tap (whi|wlo)-diag @ stride-0-paired window; v adds whi@xlo with
       taps paired two-per-instr via manually built two-offset APs
  gram: two 128-token tiles per instr.
"""
import numpy as np

DIM = 192
HEADS = 8
HD = DIM // HEADS  # 24
B, T, H, W = 2, 8, 8, 64  # per-core owned h rows = 8
HP, WP, TP = 10, 66, 10
SLAB = HP * WP  # 660
SLAB_AL = 664  # allocated slab stride (aligned)
NTOK = B * T * H * W  # 8192 owned tokens per core
NCORES = 8
C3 = 3 * DIM
NPADTOK = B * TP * SLAB  # 13200
NPT = TP * SLAB_AL  # 6640 padded cols per (b) big slab tile

_CACHE = {}

MTILES = [(0, 128), (128, 128), (256, 128), (384, 128), (512, 64)]
QK_MIS = (0, 1, 2)
V_MIS = (3, 4)
TAPS = [(dt, dh, dw) for dt in (-1, 0, 1) for dh in (-1, 0, 1)
        for dw in (-1, 0, 1)]


def _tap_idx(dt, i):
    return (dt + 1) * 9 + i


# xlo-pass tap pairings (pairs of tap indices; second None = zero k-tile)
def _xlo_pairs(kind):
    if kind == "int":
        prs = [(_tap_idx(-1, i), _tap_idx(0, i)) for i in range(9)]
        prs += [(_tap_idx(1, 2 * k), _tap_idx(1, 2 * k + 1)) for k in range(4)]
        prs += [(_tap_idx(1, 8), None)]
    elif kind == "e0":
        prs = [(_tap_idx(0, i), _tap_idx(1, i)) for i in range(9)]
    else:  # e7
        prs = [(_tap_idx(-1, i), _tap_idx(0, i)) for i in range(9)]
    return prs


# ordered unique pair keys across all chunk kinds
_PAIR_KEYS = []
for _kind in ("int", "e0", "e7"):
    for _p in _xlo_pairs(_kind):
        if _p not in _PAIR_KEYS:
            _PAIR_KEYS.append(_p)
_PAIR_POS = {p: i for i, p in enumerate(_PAIR_KEYS)}
NPAIRS = len(_PAIR_KEYS)


def _win_off(t_slab, dh, dw, half):
    """Column offset of a 264-wide tap window in the per-b big slab tile."""
    return SLAB_AL * t_slab + WP * (1 + dh + 4 * half) + (1 + dw)


def _build():
    import concourse.bacc as bacc
    import concourse.mybir as mybir
    import concourse.tile as tile
    from concourse import masks
    from concourse.ap import AP as APc

    F32 = mybir.dt.float32
    F16 = mybir.dt.float16
    F8 = mybir.dt.float8e4
    AL = mybir.AluOpType
    AF = mybir.ActivationFunctionType
    AX = mybir.AxisListType
    PM = mybir.MatmulPerfMode

    NW = 264  # tap window width (4 h-rows x 66)

    def pair_ap(full, off_a, off_b, n=NW):
        """[p, 2, n] AP over `full` (a [p, X] AP) with windows at two offsets."""
        base = full[:, off_a:off_a + n]
        return APc(base.tensor, base.offset,
                   [list(d) for d in base.ap[:1]] + [[off_b - off_a, 2]] +
                   [list(d) for d in base.ap[1:]])

    nc = bacc.Bacc("TRN2", target_bir_lowering=False, debug=False,
                   num_devices=NCORES)

    # ---- dram inputs ----
    x8hi = nc.dram_tensor("x8hi", [96, 2 * NPADTOK], F8,
                          kind="ExternalInput").ap()
    x8lo = nc.dram_tensor("x8lo", [96, 2 * NPADTOK], F8,
                          kind="ExternalInput").ap()
    wq8hi = nc.dram_tensor("wq8hi", [96, 2 * C3], F8,
                           kind="ExternalInput").ap()
    wq8lo = nc.dram_tensor("wq8lo", [96, 2 * C3], F8,
                           kind="ExternalInput").ap()
    qkvb8 = nc.dram_tensor("qkvb8", [128, 5], F32, kind="ExternalInput").ap()
    # prebuilt diag tiles: per mtile, 27 x [mc, 2mc] (whi|wlo)
    dgd = [nc.dram_tensor(f"dg{mi}", [mc, 27 * 2 * mc], F8,
                          kind="ExternalInput").ap()
           for mi, (mo, mc) in enumerate(MTILES)]
    # xlo-pass pair diags for v mtiles: NPAIRS x [mc, 2mc] (whi_a|whi_b)
    dgp = {mi: nc.dram_tensor(f"dgp{mi}", [MTILES[mi][1],
                                           NPAIRS * 2 * MTILES[mi][1]], F8,
                              kind="ExternalInput").ap()
           for mi in V_MIS}
    dwbqk = nc.dram_tensor("dwbqk", [128, 3], F32, kind="ExternalInput").ap()
    dwbv = nc.dram_tensor("dwbv", [128, 2], F32, kind="ExternalInput").ap()
    projwT = nc.dram_tensor("projwT", [DIM, DIM], F16,
                            kind="ExternalInput").ap()
    projb = nc.dram_tensor("projb", [128, 2], F32, kind="ExternalInput").ap()
    temp = nc.dram_tensor("temp", [16, 1], F32, kind="ExternalInput").ap()
    out = nc.dram_tensor("out", [DIM, NTOK], F32, kind="ExternalOutput").ap()

    gram_in = nc.dram_tensor("gram_in", [16, 48, 48], F32).ap()
    gram_out = nc.dram_tensor("gram_out", [16, 48, 48], F32,
                              addr_space="Shared").ap()
    attn_dram = nc.dram_tensor("attn_dram", [16, HD, HD], F16).ap()

    KTILES = [(0, 128), (128, 64)]  # fp16 contraction split (proj/attn@v)

    with tile.TileContext(nc) as tc:
        with (
            tc.tile_pool(name="wp", bufs=1) as wp,
            tc.tile_pool(name="xp", bufs=3) as xp,
            tc.tile_pool(name="slb", bufs=1) as slb,
            tc.tile_pool(name="qk", bufs=1) as qkpool,
            tc.tile_pool(name="ev", bufs=4) as ev,
            tc.tile_pool(name="evh", bufs=2) as evh,
            tc.tile_pool(name="small", bufs=1) as sp,
            tc.tile_pool(name="ps", bufs=5, space="PSUM") as psp,
            tc.tile_pool(name="pst", bufs=2, space="PSUM") as pst,
            tc.tile_pool(name="psg", bufs=1, space="PSUM") as psg,
        ):
            # ---------------- weights ----------------
            # qkv weights on the SP queue (needed first); the 6MB of diag
            # tiles go on the idle Pool DGE queue so the x-slab DMAs are not
            # stuck behind them.
            wqhi = wp.tile([96, 2 * C3], F8, tag="wqhi")
            nc.sync.dma_start(out=wqhi[:], in_=wq8hi)
            wqlo = wp.tile([96, 2 * C3], F8, tag="wqlo")
            nc.sync.dma_start(out=wqlo[:], in_=wq8lo)
            qkvb_s = wp.tile([128, 5], F32, tag="qkvb")
            nc.sync.dma_start(out=qkvb_s[:], in_=qkvb8)
            dg_s = []
            for mi, (mo, mc) in enumerate(MTILES):
                t = wp.tile([mc, 27 * 2 * mc], F8, tag=f"dg{mi}")
                nc.gpsimd.dma_start(out=t[:], in_=dgd[mi])
                dg_s.append(t)
            dgp_s = {}
            for mi in V_MIS:
                mc = MTILES[mi][1]
                t = wp.tile([mc, NPAIRS * 2 * mc], F8, tag=f"dgp{mi}")
                nc.gpsimd.dma_start(out=t[:], in_=dgp[mi])
                dgp_s[mi] = t
            dwbqk_s = wp.tile([128, 3], F32, tag="dwbqk")
            nc.gpsimd.dma_start(out=dwbqk_s[:], in_=dwbqk)
            dwbv_s = wp.tile([128, 2], F32, tag="dwbv")
            nc.gpsimd.dma_start(out=dwbv_s[:], in_=dwbv)
            wproj = []
            for ki, (ko, kc) in enumerate(KTILES):
                t = wp.tile([kc, DIM], F16, tag=f"wproj{ki}")
                nc.sync.dma_start(out=t[:], in_=projwT[ko:ko + kc, :])
                wproj.append(t)
            projb_s = wp.tile([128, 2], F32, tag="projb")
            nc.sync.dma_start(out=projb_s[:], in_=projb)
            temp_s = wp.tile([16, 1], F32, tag="temp")
            nc.sync.dma_start(out=temp_s[:], in_=temp)

            ident8 = wp.tile([128, 128], F8, tag="ident8")
            masks.make_identity(nc, ident8[:])

            def dg_ap(mi, ti):
                mc = MTILES[mi][1]
                return dg_s[mi][:, ti * 2 * mc:(ti + 1) * 2 * mc].rearrange(
                    "p (two m) -> p two m", two=2)

            def dgp_ap(mi, pos):
                mc = MTILES[mi][1]
                return dgp_s[mi][:, pos * 2 * mc:(pos + 1) * 2 * mc].rearrange(
                    "p (two m) -> p two m", two=2)

            # dw outputs: q~,k~ (384 ch) fp8 x32, one batch at a time since
            # transposes+grams drain per batch; v (192 ch) fp16 both batches
            NTOKB = NTOK // B  # 4096
            qk_t = [qkpool.tile([128, NTOKB], F8, tag=f"qk{i}", name=f"qk{i}")
                    for i in range(3)]
            v_t = [qkpool.tile([vc, NTOK], F16, tag=f"v{i}", name=f"v{i}")
                   for i, vc in enumerate([128, 64])]

            # per-b big slab tiles (all 10 t-slabs contiguous, stride 664)
            sl_hi = [slb.tile([MTILES[mi][1], NPT], F8, tag=f"slhi{mi}",
                              name=f"slhi{mi}") for mi in range(5)]
            sl_lo = {mi: slb.tile([MTILES[mi][1], NPT], F8, tag=f"sllo{mi}",
                                  name=f"sllo{mi}") for mi in V_MIS}

            # ---------------- qkv conv ----------------
            def qkv_slab(b, t_):
                xhi = xp.tile([96, 2 * SLAB_AL], F8, tag="xhi")
                xlo = xp.tile([96, 2 * SLAB_AL], F8, tag="xlo")
                off = (b * TP + t_) * SLAB
                for src, dst in ((x8hi, xhi), (x8lo, xlo)):
                    nc.sync.dma_start(
                        out=dst[:].rearrange("p (two n) -> p two n",
                                             two=2)[:, :, 0:SLAB],
                        in_=src.rearrange("p (two n) -> p two n",
                                          two=2)[:, :, off:off + SLAB])
                xhi2 = xhi[:].rearrange("p (two n) -> p two n", two=2)
                xlo2 = xlo[:].rearrange("p (two n) -> p two n", two=2)
                for mi, (mo, mc) in enumerate(MTILES):
                    whi2 = wqhi[:].rearrange("p (two m) -> p two m",
                                             two=2)[:, :, mo:mo + mc]
                    wlo2 = wqlo[:].rearrange("p (two m) -> p two m",
                                             two=2)[:, :, mo:mo + mc]
                    for half in range(2):
                        cs = 330 * half
                        ce = cs + 330
                        ps = psp.tile([128, 512], F32, tag="mm")
                        three = mi in V_MIS
                        nc.tensor.matmul(ps[:mc, :330], whi2,
                                         xhi2[:, :, cs:ce], start=True,
                                         stop=False, perf_mode=PM.DoubleRow)
                        nc.tensor.matmul(ps[:mc, :330], wlo2,
                                         xhi2[:, :, cs:ce], start=False,
                                         stop=not three,
                                         perf_mode=PM.DoubleRow)
                        if three:
                            nc.tensor.matmul(ps[:mc, :330], whi2,
                                             xlo2[:, :, cs:ce], start=False,
                                             stop=True,
                                             perf_mode=PM.DoubleRow)
                        dst = SLAB_AL * t_ + cs
                        # hi slab: 8*(qkv) = psum/4 + 8b
                        nc.scalar.activation(
                            sl_hi[mi][:, dst:dst + 330], ps[:mc, :330],
                            AF.Identity, bias=qkvb_s[:mc, mi:mi + 1],
                            scale=0.25)
                        if three:
                            half32 = evh.tile([mc, 330], F32, tag=f"h32_{mi}")
                            nc.scalar.activation(
                                half32[:], ps[:mc, :330], AF.Identity,
                                bias=qkvb_s[:mc, mi:mi + 1], scale=0.25)
                            nc.vector.tensor_sub(
                                sl_lo[mi][:, dst:dst + 330], half32[:],
                                sl_hi[mi][:, dst:dst + 330])

            # ---------------- depthwise ----------------
            def dw_chunk(b, t_o):
                if t_o == 0:
                    kind, dts = "e0", (0, 1)
                elif t_o == T - 1:
                    kind, dts = "e7", (-1, 0)
                else:
                    kind, dts = "int", (-1, 0, 1)
                keep = [(dt + 1) * 9 + i for dt in dts for i in range(9)]
                xpairs = _xlo_pairs(kind)
                for half in range(2):
                    co = 256 * ((b * T + t_o) * 2 + half)      # v tiles
                    cob = 256 * (t_o * 2 + half)               # qk (per-batch)
                    for mi, (mo, mc) in enumerate(MTILES):
                        ps = psp.tile([128, 512], F32, tag="mm")
                        last_hi = (mi not in V_MIS)
                        for j, ti in enumerate(keep):
                            dt, dh, dw = TAPS[ti]
                            off = _win_off(t_o + 1 + dt, dh, dw, half)
                            rhs = pair_ap(sl_hi[mi][:], off, off)
                            nc.tensor.matmul(
                                ps[:mc, :NW], dg_ap(mi, ti), rhs,
                                start=(j == 0),
                                stop=(last_hi and j == len(keep) - 1),
                                perf_mode=PM.DoubleRow)
                        if not last_hi:
                            for j, (ta, tb) in enumerate(xpairs):
                                dta, dha, dwa = TAPS[ta]
                                offa = _win_off(t_o + 1 + dta, dha, dwa, half)
                                if tb is None:
                                    offb = offa
                                else:
                                    dtb, dhb, dwb_ = TAPS[tb]
                                    offb = _win_off(t_o + 1 + dtb, dhb, dwb_,
                                                    half)
                                rhs = pair_ap(sl_lo[mi][:], offa, offb)
                                nc.tensor.matmul(
                                    ps[:mc, :NW],
                                    dgp_ap(mi, _PAIR_POS[(ta, tb)]), rhs,
                                    start=False, stop=(j == len(xpairs) - 1),
                                    perf_mode=PM.DoubleRow)
                        pv = ps[:, :NW].rearrange(
                            "p (h w) -> p h w", h=4)[:, :, 0:64]
                        if mi in QK_MIS:
                            # qk8 = 32*(dw+b) = psum/8 + 32b
                            nc.scalar.activation(
                                qk_t[mi][:, cob:cob + 256].rearrange(
                                    "p (h w) -> p h w", h=4),
                                pv[:mc], AF.Identity,
                                bias=dwbqk_s[:mc, mi:mi + 1], scale=0.125)
                        elif mi == 3:
                            nc.scalar.activation(
                                v_t[0][:, co:co + 256].rearrange(
                                    "p (h w) -> p h w", h=4),
                                pv[:mc], AF.Identity,
                                bias=dwbv_s[:mc, 0:1], scale=1.0 / 256)
                        else:
                            nc.scalar.activation(
                                v_t[1][:, co:co + 256].rearrange(
                                    "p (h w) -> p h w", h=4),
                                pv[:64], AF.Identity,
                                bias=dwbv_s[:64, 1:2], scale=1.0 / 256)

            gps = psg.tile([48, 384], F32, tag="gram")  # reused across b
            dm = sp.tile([8, 576], F32, tag="dm")
            nc.gpsimd.memset(dm[:], 0.0)
            nc.gpsimd.affine_select(
                out=dm[:], in_=dm[:], compare_op=AL.not_equal, fill=1.0,
                base=0, pattern=[[1, 24], [-1, 24]], channel_multiplier=0)
            bd = {}

            def gram_section(b):
                # transposes + grams (c64 pairs), then AllReduce; batch 0's
                # collective overlaps batch 1's depthwise work
                for pr in range(16):
                    c64a = 2 * pr  # qk_t holds the current batch only
                    qkT = ev.tile([128, 768], F8, tag="qkT")
                    for s in range(2):
                        for i in range(3):
                            # fp8 transpose requires output element step 2
                            tps = pst.tile([128, 256], F8, tag="trps")
                            nc.tensor.transpose(
                                tps[:, 0:256:2],
                                qk_t[i][:, 128 * (c64a + s):
                                        128 * (c64a + s + 1)],
                                ident8[:])
                            nc.vector.tensor_copy(
                                qkT[:, 384 * s + 128 * i:
                                    384 * s + 128 * (i + 1)],
                                tps[:, 0:256:2])
                    z2 = qkT[:].rearrange("p (two m) -> p two m", two=2)
                    for h in range(HEADS):
                        z = z2[:, :, 48 * h:48 * (h + 1)]
                        nc.tensor.matmul(
                            gps[:, 48 * h:48 * (h + 1)], z, z,
                            start=(pr == 0 and h == 0),
                            stop=(pr == 15 and h == HEADS - 1),
                            perf_mode=PM.DoubleRow)
                gs = ev.tile([48, 384], F32, tag="gs")
                nc.vector.tensor_copy(gs[:], gps[:])
                nc.gpsimd.dma_start(
                    out=gram_in[8 * b:8 * (b + 1)].rearrange(
                        "g c d -> c g d"),
                    in_=gs[:].rearrange("c (g d) -> c g d", g=8))
                nc.gpsimd.collective_compute(
                    "AllReduce", AL.add,
                    replica_groups=[list(range(NCORES))],
                    ins=[gram_in[8 * b:8 * (b + 1)]],
                    outs=[gram_out[8 * b:8 * (b + 1)]])

            def softmax_section(b):
                # norms + softmax for one batch ([8, .]); grams carry a 1024x
                # scale (32q)(32k) which cancels exactly in the normalized
                # correlation.
                qq_f = sp.tile([8, 576], F32, tag="qqf")
                kk_f = sp.tile([8, 576], F32, tag="kkf")
                qk_f = sp.tile([8, 576], F32, tag="qkf")
                go = gram_out[8 * b:8 * (b + 1)]
                nc.gpsimd.dma_start(
                    out=qq_f[:].rearrange("p (c d) -> p c d", c=24),
                    in_=go[:, 0:24, 0:24])
                nc.gpsimd.dma_start(
                    out=kk_f[:].rearrange("p (c d) -> p c d", c=24),
                    in_=go[:, 24:48, 24:48])
                nc.gpsimd.dma_start(
                    out=qk_f[:].rearrange("p (c d) -> p c d", c=24),
                    in_=go[:, 0:24, 24:48])

                def diag_rsqrt(src, tag):
                    t1 = sp.tile([8, 576], F32, tag="tmp576")
                    nc.vector.tensor_mul(t1[:], src[:], dm[:])
                    n2 = sp.tile([8, 24], F32, tag=tag + "b")
                    nc.vector.tensor_reduce(
                        n2[:], t1[:].rearrange("p (c d) -> p c d", c=24),
                        axis=AX.X, op=AL.add)
                    nrm = sp.tile([8, 24], F32, tag=tag + "c")
                    nc.scalar.sqrt(nrm[:], n2[:])
                    nc.vector.tensor_scalar_max(nrm[:], nrm[:], 1e-12)
                    r = sp.tile([8, 24], F32, tag=tag + "d")
                    nc.vector.reciprocal(r[:], nrm[:])
                    return r

                rq = diag_rsqrt(qq_f, "rq")
                rk = diag_rsqrt(kk_f, "rk")

                a1 = sp.tile([8, 576], F32, tag="a1")
                nc.vector.tensor_mul(
                    a1[:].rearrange("p (c d) -> p c d", c=24),
                    qk_f[:].rearrange("p (c d) -> p c d", c=24),
                    rq[:].rearrange("p (c one) -> p c one",
                                    one=1).broadcast_to((8, 24, 24)))
                nc.vector.tensor_mul(
                    a1[:].rearrange("p (c d) -> p c d", c=24),
                    a1[:].rearrange("p (c d) -> p c d", c=24),
                    rk[:].rearrange("p (one d) -> p one d",
                                    one=1).broadcast_to((8, 24, 24)))
                nc.vector.tensor_scalar_mul(a1[:], a1[:], temp_s[:8])

                mx = sp.tile([8, 24], F32, tag="mx")
                nc.vector.tensor_reduce(
                    mx[:], a1[:].rearrange("p (c d) -> p c d", c=24),
                    axis=AX.X, op=AL.max)
                nc.vector.tensor_sub(
                    a1[:].rearrange("p (c d) -> p c d", c=24),
                    a1[:].rearrange("p (c d) -> p c d", c=24),
                    mx[:].rearrange("p (c one) -> p c one",
                                    one=1).broadcast_to((8, 24, 24)))
                ex = sp.tile([8, 576], F32, tag="ex")
                nc.scalar.activation(ex[:], a1[:], AF.Exp)
                sm = sp.tile([8, 24], F32, tag="sm")
                nc.vector.tensor_reduce(
                    sm[:], ex[:].rearrange("p (c d) -> p c d", c=24),
                    axis=AX.X, op=AL.add)
                rs = sp.tile([8, 24], F32, tag="rs")
                nc.vector.reciprocal(rs[:], sm[:])
                at16 = sp.tile([8, 576], F16, tag="at16")
                nc.vector.tensor_mul(
                    at16[:].rearrange("p (c d) -> p c d", c=24),
                    ex[:].rearrange("p (c d) -> p c d", c=24),
                    rs[:].rearrange("p (c one) -> p c one",
                                    one=1).broadcast_to((8, 24, 24)))
                nc.gpsimd.dma_start(
                    out=attn_dram[8 * b:8 * (b + 1)],
                    in_=at16[:].rearrange("p (c d) -> p c d", c=24))

                # block-diag attn^T, split into k-tiles 128+64
                bts = []
                for ki, (ko, kc) in enumerate(KTILES):
                    bdt = sp.tile([kc, DIM], F16, tag=f"bd{b}_{ki}")
                    nc.vector.memset(bdt[:], 0.0)
                    bts.append(bdt)
                for h in range(HEADS):
                    src_a = attn_dram[8 * b + h].rearrange("c d -> d c")
                    r0, r1 = HD * h, HD * (h + 1)
                    if r1 <= 128:
                        nc.gpsimd.dma_start(
                            out=bts[0][r0:r1, r0:r1], in_=src_a)
                    elif r0 >= 128:
                        nc.gpsimd.dma_start(
                            out=bts[1][r0 - 128:r1 - 128, r0:r1], in_=src_a)
                    else:  # h == 5: rows 120..144 straddle the k-tile split
                        nc.gpsimd.dma_start(
                            out=bts[0][r0:128, r0:r1], in_=src_a[0:128 - r0])
                        nc.gpsimd.dma_start(
                            out=bts[1][0:r1 - 128, r0:r1],
                            in_=src_a[128 - r0:HD])
                bd[b] = bts

            def av_chunks(b, lo, hi):
                for chunk in range(b * T + lo, b * T + hi):
                    co = 512 * chunk
                    aos = [ev.tile([kc, 512], F16, tag=f"ao{ki}",
                                   name=f"ao{ki}")
                           for ki, (ko, kc) in enumerate(KTILES)]
                    for mi, (mo, mc) in enumerate(KTILES):
                        ps = psp.tile([128, 512], F32, tag="mm")
                        for ki in range(2):
                            nc.tensor.matmul(
                                ps[:mc, :], bd[b][ki][:, mo:mo + mc],
                                v_t[ki][:, co:co + 512],
                                start=(ki == 0), stop=(ki == 1))
                        nc.vector.tensor_copy(aos[mi][:, :], ps[:mc, :])
                    for mi, (mo, mc) in enumerate(KTILES):
                        ps = psp.tile([128, 512], F32, tag="mm")
                        for ki in range(2):
                            nc.tensor.matmul(
                                ps[:mc, :], wproj[ki][:, mo:mo + mc],
                                aos[ki][:, :],
                                start=(ki == 0), stop=(ki == 1))
                        of = ev.tile([128, 512], F32, tag="of")
                        nc.vector.tensor_scalar(
                            of[:mc, :], ps[:mc, :], projb_s[:mc, mi:mi + 1],
                            None, AL.add)
                        nc.sync.dma_start(out=out[mo:mo + mc, co:co + 512],
                                          in_=of[:mc, :])

            for b in range(B):
                for t_ in (1, 2, 3):
                    qkv_slab(b, t_)
                for t_o in range(T):
                    dw_chunk(b, t_o)
                    if t_o + 4 <= T:
                        qkv_slab(b, t_o + 4)
                    if b == 1:
                        # hide batch 0's attention tail inside batch 1's dw
                        if t_o == 0:
                            softmax_section(0)
                        elif 1 <= t_o <= 4:
                            av_chunks(0, 2 * (t_o - 1), 2 * t_o)
                gram_section(b)
            softmax_section(1)
            av_chunks(1, 0, T)
    nc.compile()
    return nc


def _prep_inputs(x, qkv_w, qkv_b, dw_w, dw_b, temperature, proj_w, proj_b):
    """Host-side prep: fp8 hi/lo inputs, prebuilt fp8 diag tiles, fp16 proj."""
    import ml_dtypes
    F8NP = ml_dtypes.float8_e4m3

    def q8(a):
        return np.asarray(a, np.float32).astype(F8NP)

    x = np.asarray(x, np.float32)
    b_, c_, t_, h_, w_ = x.shape  # 2, 192, 8, 64, 64
    qkv_w2 = np.asarray(qkv_w, np.float32).reshape(C3, DIM)
    dw_w2 = np.asarray(dw_w, np.float32).reshape(C3, 27)
    proj_w2 = np.asarray(proj_w, np.float32).reshape(DIM, DIM)
    # permute qkv channels: [q_h0, k_h0, q_h1, k_h1, ..., v] so each head's
    # (q,k) columns are adjacent after transpose (contiguous gram operands)
    perm = []
    for h in range(HEADS):
        perm.extend(range(HD * h, HD * (h + 1)))          # q_h
        perm.extend(range(DIM + HD * h, DIM + HD * (h + 1)))  # k_h
    perm.extend(range(2 * DIM, 3 * DIM))                  # v unchanged
    perm = np.array(perm)
    qkv_w2 = qkv_w2[perm]
    dw_w2 = dw_w2[perm]
    qkv_b = np.asarray(qkv_b, np.float32)[perm]
    dw_b = np.asarray(dw_b, np.float32)[perm]

    # qkv weights x32, hi/lo, laid out [96, 2, 576] -> [96, 1152]
    w32 = 32.0 * qkv_w2  # [576, 192]
    whi = q8(w32).astype(np.float32)
    wlo = q8(w32 - whi).astype(np.float32)

    def wq_layout(w):  # [576(out), 192(in)] -> [96, 2*576] fp8
        wt = np.ascontiguousarray(w.T)          # [192, 576]
        return np.ascontiguousarray(
            wt.reshape(2, 96, C3).transpose(1, 0, 2).reshape(96, 2 * C3)
        ).astype(F8NP)

    wq8hi_h = wq_layout(whi)
    wq8lo_h = wq_layout(wlo)

    qkvb8_h = np.zeros((128, 5), np.float32)
    for mi, (mo, mc) in enumerate(MTILES):
        qkvb8_h[:mc, mi] = 8.0 * qkv_b[mo:mo + mc]

    # dw diag tiles (whi|wlo) per tap, x32
    d32 = 32.0 * dw_w2  # [576, 27]
    dhi = q8(d32).astype(np.float32)
    dlo = q8(d32 - dhi).astype(np.float32)
    dgd_h = []
    for mi, (mo, mc) in enumerate(MTILES):
        t = np.zeros((mc, 27 * 2 * mc), np.float32)
        idx = np.arange(mc)
        for ti in range(27):
            t[idx, ti * 2 * mc + idx] = dhi[mo:mo + mc, ti]
            t[idx, ti * 2 * mc + mc + idx] = dlo[mo:mo + mc, ti]
        dgd_h.append(t.astype(F8NP))
    dgp_h = {}
    for mi in V_MIS:
        mo, mc = MTILES[mi]
        t = np.zeros((mc, NPAIRS * 2 * mc), np.float32)
        idx = np.arange(mc)
        for pos, (ta, tb) in enumerate(_PAIR_KEYS):
            t[idx, pos * 2 * mc + idx] = dhi[mo:mo + mc, ta]
            if tb is not None:
                t[idx, pos * 2 * mc + mc + idx] = dhi[mo:mo + mc, tb]
        dgp_h[mi] = t.astype(F8NP)

    dwbqk_h = np.zeros((128, 3), np.float32)
    for mi in QK_MIS:
        mo, mc = MTILES[mi]
        dwbqk_h[:mc, mi] = 32.0 * dw_b[mo:mo + mc]
    dwbv_h = np.zeros((128, 2), np.float32)
    for j, mi in enumerate(V_MIS):
        mo, mc = MTILES[mi]
        dwbv_h[:mc, j] = dw_b[mo:mo + mc]

    projwT_h = np.ascontiguousarray(proj_w2.T).astype(np.float16)
    projb_h = np.zeros((128, 2), np.float32)
    projb_h[:128, 0] = np.asarray(proj_b, np.float32)[0:128]
    projb_h[:64, 1] = np.asarray(proj_b, np.float32)[128:192]
    temp_h = np.tile(np.asarray(temperature, np.float32).reshape(HEADS),
                     2).reshape(16, 1)  # g = b*8+h

    in_maps = []
    for i in range(NCORES):
        # padded slab [b, t10, h10, w66], h rows 8i-1 .. 8i+9 clamped->zero
        xs = np.zeros((b_, TP, HP, WP, c_), np.float32)
        hlo, hhi = 8 * i - 1, 8 * i + 9
        slo, shi = max(0, hlo), min(h_, hhi)
        xt = x[:, :, :, slo:shi, :].transpose(0, 2, 3, 4, 1)
        xs[:, 1:9, (slo - hlo):(slo - hlo) + (shi - slo), 1:65, :] = xt
        xT = np.ascontiguousarray(
            xs.reshape(b_ * TP * SLAB, c_).T)  # [192, 13200] f32
        xhi = q8(xT).astype(np.float32)
        xlo_ = q8(xT - xhi)

        def x_layout(a):  # [192, NPADTOK] -> [96, 2*NPADTOK] fp8
            return np.ascontiguousarray(
                np.asarray(a, np.float32).reshape(2, 96, NPADTOK)
                .transpose(1, 0, 2).reshape(96, 2 * NPADTOK)).astype(F8NP)

        m = {
            "x8hi": x_layout(xhi), "x8lo": x_layout(xlo_),
            "wq8hi": wq8hi_h, "wq8lo": wq8lo_h, "qkvb8": qkvb8_h,
            "dwbqk": dwbqk_h, "dwbv": dwbv_h,
            "projwT": projwT_h, "projb": projb_h, "temp": temp_h,
        }
        for mi in range(5):
            m[f"dg{mi}"] = dgd_h[mi]
        for mi in V_MIS:
            m[f"dgp{mi}"] = dgp_h[mi]
        in_maps.append(m)
    return in_maps


def _get_runner():
    """Build once; return a persistent sharded-jit callable (the per-call
    closure in bass2jax.run_bass_via_pjrt defeats jax's jit cache)."""
    if "runner" in _CACHE:
        return _CACHE["runner"]
    import jax
    for flag, val in [("jax_compilation_cache_dir", "/tmp/jax_kernel_cache"),
                      ("jax_persistent_cache_min_compile_time_secs", 1.0),
                      ("jax_persistent_cache_min_entry_size_bytes", 0)]:
        try:
            jax.config.update(flag, val)
        except Exception:
            pass
    from jax.sharding import Mesh, PartitionSpec
    from jax.experimental.shard_map import shard_map
    import concourse.mybir as mybir
    from concourse import bass2jax

    nc = _build()
    bass2jax.install_neuronx_cc_hook()

    partition_name = (nc.partition_id_tensor.name
                      if nc.partition_id_tensor else None)
    in_names, out_names, out_avals, zero_shapes = [], [], [], []
    for alloc in nc.m.functions[0].allocations:
        if not isinstance(alloc, mybir.MemoryLocationSet):
            continue
        name = alloc.memorylocations[0].name
        if alloc.kind == "ExternalInput":
            if name != partition_name:
                in_names.append(name)
        elif alloc.kind == "ExternalOutput":
            shape = tuple(alloc.tensor_shape)
            dtype = mybir.dt.np(alloc.dtype)
            out_names.append(name)
            out_avals.append(jax.core.ShapedArray(shape, dtype))
            zero_shapes.append((shape, dtype))
    n_params = len(in_names)
    all_names = in_names + out_names
    if partition_name is not None:
        all_names.append(partition_name)

    def _body(*args):
        operands = list(args)
        if partition_name is not None:
            operands.append(bass2jax.partition_id_tensor())
        outs = bass2jax._bass_exec_p.bind(
            *operands, out_avals=tuple(out_avals), in_names=tuple(all_names),
            out_names=tuple(out_names), lowering_input_output_aliases=(),
            sim_require_finite=True, sim_require_nnan=True, nc=nc)
        return tuple(outs)

    devices = jax.devices()[:NCORES]
    mesh = Mesh(np.asarray(devices), ("core",))
    n_outs = len(out_names)
    sharded = jax.jit(
        shard_map(_body, mesh=mesh,
                  in_specs=(PartitionSpec("core"),) * (n_params + n_outs),
                  out_specs=(PartitionSpec("core"),) * n_outs,
                  check_rep=False),
        donate_argnums=tuple(range(n_params, n_params + n_outs)),
        keep_unused=True)

    def run(in_maps):
        concat_in = [np.concatenate([in_maps[c][nm] for c in range(NCORES)],
                                    axis=0) for nm in in_names]
        concat_zeros = [np.zeros((NCORES * s[0], *s[1:]), dt)
                        for s, dt in zero_shapes]
        out_arrs = sharded(*concat_in, *concat_zeros)
        return [
            {nm: np.asarray(out_arrs[i]).reshape(NCORES, *out_avals[i].shape)[c]
             for i, nm in enumerate(out_names)}
            for c in range(NCORES)]

    _CACHE["runner"] = run
    return run


def kernel(x, qkv_w, qkv_b, dw_w, dw_b, temperature, proj_w, proj_b):
    run = _get_runner()
    in_maps = _prep_inputs(x, qkv_w, qkv_b, dw_w, dw_b, temperature,
                           proj_w, proj_b)
    results = run(in_maps)
    b_, c_, t_, h_, w_ = np.asarray(x).shape
    outf = np.empty((b_, c_, t_, h_, w_), np.float32)
    for i in range(NCORES):
        o = results[i]["out"].reshape(c_, b_, t_, H, w_)
        outf[:, :, :, 8 * i:8 * i + 8, :] = o.transpose(1, 0, 2, 3, 4)
    return outf


# revision 27
# speedup vs baseline: 1.4078x; 1.0430x over previous
"""nn_AttentionC Trainium2 kernel (8 NeuronCores, SPMD) — fp8 DoubleRow.

Sharding: h-axis (64) split into 8 chunks of 8 rows, one per core; each core's
x slab is host-padded to [b2, t10, h10, w66] tokens (conv zero-padding baked
in). Only cross-core traffic: AllReduce of per-(b,head) [48,48] q/k gram
matrices (110 KB).

Precision scheme (validated vs fp32 reference, rel_err ~2e-3):
  - weights everywhere: fp8e4m3 hi+lo (pre-scaled x32; coherent quant noise
    does not average out, so 2 words needed)
  - qkv input x: fp8 hi+lo from host (v-path needs the lo word)
  - q,k dw slabs + gram operands: single fp8 (x8 / x32 pre-scale) — random
    per-element noise washes out in the 32k-token gram contraction
  - v path slabs: fp8 hi+lo (x8 pre-scale); v output, attn, proj: fp16
All fp8 matmuls use DoubleRow perf mode (2 K-tiles/instr, 0.5 cyc/row):
  qkv: (whi|wlo)@xhi [+ whi@xlo for v tiles], contraction 192 split 96+96
  dw:  per tap (whi|wlo)-diag @ stride-0-paired window; v adds whi@xlo with
       taps paired two-per-instr via manually built two-offset APs
  gram: two 128-token tiles per instr.
"""
import numpy as np

DIM = 192
HEADS = 8
HD = DIM // HEADS  # 24
B, T, H, W = 2, 8, 8, 64  # per-core owned h rows = 8
HP, WP, TP = 10, 66, 10
SLAB = HP * WP  # 660
SLAB_AL = 664  # allocated slab stride (aligned)
NTOK = B * T * H * W  # 8192 owned tokens per core
NCORES = 8
C3 = 3 * DIM
NPADTOK = B * TP * SLAB  # 13200
NPT = TP * SLAB_AL  # 6640 padded cols per (b) big slab tile

_CACHE = {}

MTILES = [(0, 128), (128, 128), (256, 128), (384, 128), (512, 64)]
QK_MIS = (0, 1, 2)
V_MIS = (3, 4)
TAPS = [(dt, dh, dw) for dt in (-1, 0, 1) for dh in (-1, 0, 1)
        for dw in (-1, 0, 1)]


def _tap_idx(dt, i):
    return (dt + 1) * 9 + i


# xlo-pass tap pairings (pairs of tap indices; second None = zero k-tile)
def _xlo_pairs(kind):
    if kind == "int":
        prs = [(_tap_idx(-1, i), _tap_idx(0, i)) for i in range(9)]
        prs += [(_tap_idx(1, 2 * k), _tap_idx(1, 2 * k + 1)) for k in range(4)]
        prs += [(_tap_idx(1, 8), None)]
    elif kind == "e0":
        prs = [(_tap_idx(0, i), _tap_idx(1, i)) for i in range(9)]
    else:  # e7
        prs = [(_tap_idx(-1, i), _tap_idx(0, i)) for i in range(9)]
    return prs


# ordered unique pair keys across all chunk kinds
_PAIR_KEYS = []
for _kind in ("int", "e0", "e7"):
    for _p in _xlo_pairs(_kind):
        if _p not in _PAIR_KEYS:
            _PAIR_KEYS.append(_p)
_PAIR_POS = {p: i for i, p in enumerate(_PAIR_KEYS)}
NPAIRS = len(_PAIR_KEYS)


def _win_off(t_slab, dh, dw, half):
    """Column offset of a 264-wide tap window in the per-b big slab tile."""
    return SLAB_AL * t_slab + WP * (1 + dh + 4 * half) + (1 + dw)


def _build():
    import concourse.bacc as bacc
    import concourse.mybir as mybir
    import concourse.tile as tile
    from concourse import masks
    from concourse.ap import AP as APc

    F32 = mybir.dt.float32
    F16 = mybir.dt.float16
    F8 = mybir.dt.float8e4
    AL = mybir.AluOpType
    AF = mybir.ActivationFunctionType
    AX = mybir.AxisListType
    PM = mybir.MatmulPerfMode

    NW = 264  # tap window width (4 h-rows x 66)

    def pair_ap(full, off_a, off_b, n=NW):
        """[p, 2, n] AP over `full` (a [p, X] AP) with windows at two offsets."""
        base = full[:, off_a:off_a + n]
        return APc(base.tensor, base.offset,
                   [list(d) for d in base.ap[:1]] + [[off_b - off_a, 2]] +
                   [list(d) for d in base.ap[1:]])

    nc = bacc.Bacc("TRN2", target_bir_lowering=False, debug=False,
                   num_devices=NCORES)

    # ---- dram inputs ----
    x8hi = nc.dram_tensor("x8hi", [96, 2 * NPADTOK], F8,
                          kind="ExternalInput").ap()
    x8lo = nc.dram_tensor("x8lo", [96, 2 * NPADTOK], F8,
                          kind="ExternalInput").ap()
    wq8hi = nc.dram_tensor("wq8hi", [96, 2 * C3], F8,
                           kind="ExternalInput").ap()
    wq8lo = nc.dram_tensor("wq8lo", [96, 2 * C3], F8,
                           kind="ExternalInput").ap()
    qkvb8 = nc.dram_tensor("qkvb8", [128, 5], F32, kind="ExternalInput").ap()
    # prebuilt diag tiles: per mtile, 27 x [mc, 2mc] (whi|wlo)
    dgd = [nc.dram_tensor(f"dg{mi}", [mc, 27 * 2 * mc], F8,
                          kind="ExternalInput").ap()
           for mi, (mo, mc) in enumerate(MTILES)]
    # xlo-pass pair diags for v mtiles: NPAIRS x [mc, 2mc] (whi_a|whi_b)
    dgp = {mi: nc.dram_tensor(f"dgp{mi}", [MTILES[mi][1],
                                           NPAIRS * 2 * MTILES[mi][1]], F8,
                              kind="ExternalInput").ap()
           for mi in V_MIS}
    dwbqk = nc.dram_tensor("dwbqk", [128, 3], F32, kind="ExternalInput").ap()
    dwbv = nc.dram_tensor("dwbv", [128, 2], F32, kind="ExternalInput").ap()
    projwT = nc.dram_tensor("projwT", [DIM, DIM], F16,
                            kind="ExternalInput").ap()
    projb = nc.dram_tensor("projb", [128, 2], F32, kind="ExternalInput").ap()
    temp = nc.dram_tensor("temp", [16, 1], F32, kind="ExternalInput").ap()
    out = nc.dram_tensor("out", [DIM, NTOK], F32, kind="ExternalOutput").ap()

    gram_in = nc.dram_tensor("gram_in", [16, 48, 48], F32).ap()
    gram_out = nc.dram_tensor("gram_out", [16, 48, 48], F32,
                              addr_space="Shared").ap()
    attn_dram = nc.dram_tensor("attn_dram", [16, HD, HD], F16).ap()

    KTILES = [(0, 128), (128, 64)]  # fp16 contraction split (proj/attn@v)

    with tile.TileContext(nc) as tc:
        with (
            tc.tile_pool(name="wp", bufs=1) as wp,
            tc.tile_pool(name="xp", bufs=3) as xp,
            tc.tile_pool(name="slb", bufs=1) as slb,
            tc.tile_pool(name="qk", bufs=1) as qkpool,
            tc.tile_pool(name="ev", bufs=4) as ev,
            tc.tile_pool(name="evh", bufs=2) as evh,
            tc.tile_pool(name="small", bufs=1) as sp,
            tc.tile_pool(name="ps", bufs=5, space="PSUM") as psp,
            tc.tile_pool(name="pst", bufs=2, space="PSUM") as pst,
            tc.tile_pool(name="psg", bufs=1, space="PSUM") as psg,
        ):
            # ---------------- weights ----------------
            # qkv weights on the SP queue (needed first); the 6MB of diag
            # tiles go on the idle Pool DGE queue so the x-slab DMAs are not
            # stuck behind them.
            wqhi = wp.tile([96, 2 * C3], F8, tag="wqhi")
            nc.sync.dma_start(out=wqhi[:], in_=wq8hi)
            wqlo = wp.tile([96, 2 * C3], F8, tag="wqlo")
            nc.sync.dma_start(out=wqlo[:], in_=wq8lo)
            qkvb_s = wp.tile([128, 5], F32, tag="qkvb")
            nc.sync.dma_start(out=qkvb_s[:], in_=qkvb8)
            dg_s = []
            for mi, (mo, mc) in enumerate(MTILES):
                t = wp.tile([mc, 27 * 2 * mc], F8, tag=f"dg{mi}")
                nc.gpsimd.dma_start(out=t[:], in_=dgd[mi])
                dg_s.append(t)
            dgp_s = {}
            for mi in V_MIS:
                mc = MTILES[mi][1]
                t = wp.tile([mc, NPAIRS * 2 * mc], F8, tag=f"dgp{mi}")
                nc.gpsimd.dma_start(out=t[:], in_=dgp[mi])
                dgp_s[mi] = t
            dwbqk_s = wp.tile([128, 3], F32, tag="dwbqk")
            nc.gpsimd.dma_start(out=dwbqk_s[:], in_=dwbqk)
            dwbv_s = wp.tile([128, 2], F32, tag="dwbv")
            nc.gpsimd.dma_start(out=dwbv_s[:], in_=dwbv)
            wproj = []
            for ki, (ko, kc) in enumerate(KTILES):
                t = wp.tile([kc, DIM], F16, tag=f"wproj{ki}")
                nc.sync.dma_start(out=t[:], in_=projwT[ko:ko + kc, :])
                wproj.append(t)
            projb_s = wp.tile([128, 2], F32, tag="projb")
            nc.sync.dma_start(out=projb_s[:], in_=projb)
            temp_s = wp.tile([16, 1], F32, tag="temp")
            nc.sync.dma_start(out=temp_s[:], in_=temp)

            ident8 = wp.tile([128, 128], F8, tag="ident8")
            masks.make_identity(nc, ident8[:])

            def dg_ap(mi, ti):
                mc = MTILES[mi][1]
                return dg_s[mi][:, ti * 2 * mc:(ti + 1) * 2 * mc].rearrange(
                    "p (two m) -> p two m", two=2)

            def dgp_ap(mi, pos):
                mc = MTILES[mi][1]
                return dgp_s[mi][:, pos * 2 * mc:(pos + 1) * 2 * mc].rearrange(
                    "p (two m) -> p two m", two=2)

            # dw outputs: q~,k~ (384 ch) fp8 x32, one batch at a time since
            # transposes+grams drain per batch; v (192 ch) fp16 both batches
            NTOKB = NTOK // B  # 4096
            qk_t = [qkpool.tile([128, NTOKB], F8, tag=f"qk{i}", name=f"qk{i}")
                    for i in range(3)]
            v_t = [qkpool.tile([vc, NTOK], F16, tag=f"v{i}", name=f"v{i}")
                   for i, vc in enumerate([128, 64])]

            # per-b big slab tiles (all 10 t-slabs contiguous, stride 664)
            sl_hi = [slb.tile([MTILES[mi][1], NPT], F8, tag=f"slhi{mi}",
                              name=f"slhi{mi}") for mi in range(5)]
            sl_lo = {mi: slb.tile([MTILES[mi][1], NPT], F8, tag=f"sllo{mi}",
                                  name=f"sllo{mi}") for mi in V_MIS}

            # ---------------- qkv conv ----------------
            def qkv_slab(b, t_):
                xhi = xp.tile([96, 2 * SLAB_AL], F8, tag="xhi")
                xlo = xp.tile([96, 2 * SLAB_AL], F8, tag="xlo")
                off = (b * TP + t_) * SLAB
                for src, dst in ((x8hi, xhi), (x8lo, xlo)):
                    nc.sync.dma_start(
                        out=dst[:].rearrange("p (two n) -> p two n",
                                             two=2)[:, :, 0:SLAB],
                        in_=src.rearrange("p (two n) -> p two n",
                                          two=2)[:, :, off:off + SLAB])
                xhi2 = xhi[:].rearrange("p (two n) -> p two n", two=2)
                xlo2 = xlo[:].rearrange("p (two n) -> p two n", two=2)
                for mi, (mo, mc) in enumerate(MTILES):
                    whi2 = wqhi[:].rearrange("p (two m) -> p two m",
                                             two=2)[:, :, mo:mo + mc]
                    wlo2 = wqlo[:].rearrange("p (two m) -> p two m",
                                             two=2)[:, :, mo:mo + mc]
                    for half in range(2):
                        cs = 330 * half
                        ce = cs + 330
                        ps = psp.tile([128, 512], F32, tag="mm")
                        three = mi in V_MIS
                        nc.tensor.matmul(ps[:mc, :330], whi2,
                                         xhi2[:, :, cs:ce], start=True,
                                         stop=False, perf_mode=PM.DoubleRow)
                        nc.tensor.matmul(ps[:mc, :330], wlo2,
                                         xhi2[:, :, cs:ce], start=False,
                                         stop=not three,
                                         perf_mode=PM.DoubleRow)
                        if three:
                            nc.tensor.matmul(ps[:mc, :330], whi2,
                                             xlo2[:, :, cs:ce], start=False,
                                             stop=True,
                                             perf_mode=PM.DoubleRow)
                        dst = SLAB_AL * t_ + cs
                        # hi slab: 8*(qkv) = psum/4 + 8b
                        nc.scalar.activation(
                            sl_hi[mi][:, dst:dst + 330], ps[:mc, :330],
                            AF.Identity, bias=qkvb_s[:mc, mi:mi + 1],
                            scale=0.25)
                        if three:
                            half32 = evh.tile([mc, 330], F32, tag=f"h32_{mi}")
                            nc.scalar.activation(
                                half32[:], ps[:mc, :330], AF.Identity,
                                bias=qkvb_s[:mc, mi:mi + 1], scale=0.25)
                            nc.vector.tensor_sub(
                                sl_lo[mi][:, dst:dst + 330], half32[:],
                                sl_hi[mi][:, dst:dst + 330])

            # ---------------- depthwise ----------------
            def dw_chunk(b, t_o):
                if t_o == 0:
                    kind, dts = "e0", (0, 1)
                elif t_o == T - 1:
                    kind, dts = "e7", (-1, 0)
                else:
                    kind, dts = "int", (-1, 0, 1)
                keep = [(dt + 1) * 9 + i for dt in dts for i in range(9)]
                xpairs = _xlo_pairs(kind)
                for half in range(2):
                    co = 256 * ((b * T + t_o) * 2 + half)      # v tiles
                    cob = 256 * (t_o * 2 + half)               # qk (per-batch)
                    for mi, (mo, mc) in enumerate(MTILES):
                        ps = psp.tile([128, 512], F32, tag="mm")
                        last_hi = (mi not in V_MIS)
                        for j, ti in enumerate(keep):
                            dt, dh, dw = TAPS[ti]
                            off = _win_off(t_o + 1 + dt, dh, dw, half)
                            rhs = pair_ap(sl_hi[mi][:], off, off)
                            nc.tensor.matmul(
                                ps[:mc, :NW], dg_ap(mi, ti), rhs,
                                start=(j == 0),
                                stop=(last_hi and j == len(keep) - 1),
                                perf_mode=PM.DoubleRow)
                        if not last_hi:
                            for j, (ta, tb) in enumerate(xpairs):
                                dta, dha, dwa = TAPS[ta]
                                offa = _win_off(t_o + 1 + dta, dha, dwa, half)
                                if tb is None:
                                    offb = offa
                                else:
                                    dtb, dhb, dwb_ = TAPS[tb]
                                    offb = _win_off(t_o + 1 + dtb, dhb, dwb_,
                                                    half)
                                rhs = pair_ap(sl_lo[mi][:], offa, offb)
                                nc.tensor.matmul(
                                    ps[:mc, :NW],
                                    dgp_ap(mi, _PAIR_POS[(ta, tb)]), rhs,
                                    start=False, stop=(j == len(xpairs) - 1),
                                    perf_mode=PM.DoubleRow)
                        pv = ps[:, :NW].rearrange(
                            "p (h w) -> p h w", h=4)[:, :, 0:64]
                        if mi in QK_MIS:
                            # qk8 = 32*(dw+b) = psum/8 + 32b
                            nc.scalar.activation(
                                qk_t[mi][:, cob:cob + 256].rearrange(
                                    "p (h w) -> p h w", h=4),
                                pv[:mc], AF.Identity,
                                bias=dwbqk_s[:mc, mi:mi + 1], scale=0.125)
                        elif mi == 3:
                            nc.scalar.activation(
                                v_t[0][:, co:co + 256].rearrange(
                                    "p (h w) -> p h w", h=4),
                                pv[:mc], AF.Identity,
                                bias=dwbv_s[:mc, 0:1], scale=1.0 / 256)
                        else:
                            nc.scalar.activation(
                                v_t[1][:, co:co + 256].rearrange(
                                    "p (h w) -> p h w", h=4),
                                pv[:64], AF.Identity,
                                bias=dwbv_s[:64, 1:2], scale=1.0 / 256)

            gps = psg.tile([48, 384], F32, tag="gram")  # reused across b
            dm = sp.tile([8, 576], F32, tag="dm")
            nc.gpsimd.memset(dm[:], 0.0)
            nc.gpsimd.affine_select(
                out=dm[:], in_=dm[:], compare_op=AL.not_equal, fill=1.0,
                base=0, pattern=[[1, 24], [-1, 24]], channel_multiplier=0)
            bd = {}

            def gram_pairs(b, pr0, pr1):
                # transposes + grams for c64 pairs [pr0, pr1)
                for pr in range(pr0, pr1):
                    c64a = 2 * pr  # qk_t holds the current batch only
                    qkT = ev.tile([128, 768], F8, tag="qkT")
                    for s in range(2):
                        for i in range(3):
                            # fp8 transpose requires output element step 2
                            tps = pst.tile([128, 256], F8, tag="trps")
                            nc.tensor.transpose(
                                tps[:, 0:256:2],
                                qk_t[i][:, 128 * (c64a + s):
                                        128 * (c64a + s + 1)],
                                ident8[:])
                            nc.vector.tensor_copy(
                                qkT[:, 384 * s + 128 * i:
                                    384 * s + 128 * (i + 1)],
                                tps[:, 0:256:2])
                    z2 = qkT[:].rearrange("p (two m) -> p two m", two=2)
                    for h in range(HEADS):
                        z = z2[:, :, 48 * h:48 * (h + 1)]
                        nc.tensor.matmul(
                            gps[:, 48 * h:48 * (h + 1)], z, z,
                            start=(pr == 0 and h == 0),
                            stop=(pr == 15 and h == HEADS - 1),
                            perf_mode=PM.DoubleRow)

            def gram_allreduce(b):
                gs = ev.tile([48, 384], F32, tag="gs")
                nc.vector.tensor_copy(gs[:], gps[:])
                nc.gpsimd.dma_start(
                    out=gram_in[8 * b:8 * (b + 1)].rearrange(
                        "g c d -> c g d"),
                    in_=gs[:].rearrange("c (g d) -> c g d", g=8))
                nc.gpsimd.collective_compute(
                    "AllReduce", AL.add,
                    replica_groups=[list(range(NCORES))],
                    ins=[gram_in[8 * b:8 * (b + 1)]],
                    outs=[gram_out[8 * b:8 * (b + 1)]])

            def softmax_section(b):
                # norms + softmax for one batch ([8, .]); grams carry a 1024x
                # scale (32q)(32k) which cancels exactly in the normalized
                # correlation.
                qq_f = sp.tile([8, 576], F32, tag="qqf")
                kk_f = sp.tile([8, 576], F32, tag="kkf")
                qk_f = sp.tile([8, 576], F32, tag="qkf")
                go = gram_out[8 * b:8 * (b + 1)]
                nc.sync.dma_start(
                    out=qq_f[:].rearrange("p (c d) -> p c d", c=24),
                    in_=go[:, 0:24, 0:24])
                nc.sync.dma_start(
                    out=kk_f[:].rearrange("p (c d) -> p c d", c=24),
                    in_=go[:, 24:48, 24:48])
                nc.sync.dma_start(
                    out=qk_f[:].rearrange("p (c d) -> p c d", c=24),
                    in_=go[:, 0:24, 24:48])

                def diag_rsqrt(src, tag):
                    t1 = sp.tile([8, 576], F32, tag="tmp576")
                    nc.vector.tensor_mul(t1[:], src[:], dm[:])
                    n2 = sp.tile([8, 24], F32, tag=tag + "b")
                    nc.vector.tensor_reduce(
                        n2[:], t1[:].rearrange("p (c d) -> p c d", c=24),
                        axis=AX.X, op=AL.add)
                    nrm = sp.tile([8, 24], F32, tag=tag + "c")
                    nc.scalar.sqrt(nrm[:], n2[:])
                    r = sp.tile([8, 24], F32, tag=tag + "d")
                    nc.vector.reciprocal(r[:], nrm[:])
                    return r

                rq = diag_rsqrt(qq_f, "rq")
                rk = diag_rsqrt(kk_f, "rk")
                # fold temperature into rq ([8,24] op instead of [8,576])
                nc.vector.tensor_scalar_mul(rq[:], rq[:], temp_s[:8])

                a1 = sp.tile([8, 576], F32, tag="a1")
                nc.vector.tensor_mul(
                    a1[:].rearrange("p (c d) -> p c d", c=24),
                    qk_f[:].rearrange("p (c d) -> p c d", c=24),
                    rq[:].rearrange("p (c one) -> p c one",
                                    one=1).broadcast_to((8, 24, 24)))
                nc.vector.tensor_mul(
                    a1[:].rearrange("p (c d) -> p c d", c=24),
                    a1[:].rearrange("p (c d) -> p c d", c=24),
                    rk[:].rearrange("p (one d) -> p one d",
                                    one=1).broadcast_to((8, 24, 24)))
                # logits are normalized correlations * temp, |x| <= ~1:
                # exp() is safe without the max-subtract pass
                ex = sp.tile([8, 576], F32, tag="ex")
                nc.scalar.activation(ex[:], a1[:], AF.Exp)
                sm = sp.tile([8, 24], F32, tag="sm")
                nc.vector.tensor_reduce(
                    sm[:], ex[:].rearrange("p (c d) -> p c d", c=24),
                    axis=AX.X, op=AL.add)
                rs = sp.tile([8, 24], F32, tag="rs")
                nc.vector.reciprocal(rs[:], sm[:])
                at16 = sp.tile([8, 576], F16, tag="at16")
                nc.vector.tensor_mul(
                    at16[:].rearrange("p (c d) -> p c d", c=24),
                    ex[:].rearrange("p (c d) -> p c d", c=24),
                    rs[:].rearrange("p (c one) -> p c one",
                                    one=1).broadcast_to((8, 24, 24)))
                nc.sync.dma_start(
                    out=attn_dram[8 * b:8 * (b + 1)],
                    in_=at16[:].rearrange("p (c d) -> p c d", c=24))

                # block-diag attn^T, split into k-tiles 128+64; spread the 9
                # small loads across DGE queues to cut serial issue latency
                bts = []
                for ki, (ko, kc) in enumerate(KTILES):
                    bdt = sp.tile([kc, DIM], F16, tag=f"bd{b}_{ki}")
                    nc.vector.memset(bdt[:], 0.0)
                    bts.append(bdt)
                qs = [nc.sync, nc.gpsimd, nc.scalar]
                for h in range(HEADS):
                    q = qs[h % 3]
                    src_a = attn_dram[8 * b + h].rearrange("c d -> d c")
                    r0, r1 = HD * h, HD * (h + 1)
                    if r1 <= 128:
                        q.dma_start(out=bts[0][r0:r1, r0:r1], in_=src_a)
                    elif r0 >= 128:
                        q.dma_start(
                            out=bts[1][r0 - 128:r1 - 128, r0:r1], in_=src_a)
                    else:  # h == 5: rows 120..144 straddle the k-tile split
                        q.dma_start(
                            out=bts[0][r0:128, r0:r1], in_=src_a[0:128 - r0])
                        q.dma_start(
                            out=bts[1][0:r1 - 128, r0:r1],
                            in_=src_a[128 - r0:HD])
                bd[b] = bts

            def av_chunks(b, lo, hi, use_act=False):
                # two-phase per 4-chunk block: all attn@v groups first (PSUM
                # evicted to aos), then all proj groups — hides the aos copy
                # latency behind the other chunks' matmuls. use_act routes
                # PSUM evictions to the ACT engine (idle during the tail,
                # while DVE runs the softmax chain).
                def evict(dst, src, bias=None):
                    if use_act:
                        if bias is None:
                            nc.scalar.activation(dst, src, AF.Identity)
                        else:
                            nc.scalar.activation(dst, src, AF.Identity,
                                                 bias=bias, scale=1.0)
                    else:
                        if bias is None:
                            nc.vector.tensor_copy(dst, src)
                        else:
                            nc.vector.tensor_scalar(dst, src, bias, None,
                                                    AL.add)
                for blk in range(lo, hi, 4):
                    chunks = list(range(b * T + blk,
                                        b * T + min(blk + 4, hi)))
                    aom = {}
                    for chunk in chunks:
                        co = 512 * chunk
                        aos = [ev.tile([kc, 512], F16, tag=f"ao{ki}",
                                       name=f"ao{ki}")
                               for ki, (ko, kc) in enumerate(KTILES)]
                        for mi, (mo, mc) in enumerate(KTILES):
                            ps = psp.tile([128, 512], F32, tag="mm")
                            for ki in range(2):
                                nc.tensor.matmul(
                                    ps[:mc, :], bd[b][ki][:, mo:mo + mc],
                                    v_t[ki][:, co:co + 512],
                                    start=(ki == 0), stop=(ki == 1))
                            evict(aos[mi][:, :], ps[:mc, :])
                        aom[chunk] = aos
                    for chunk in chunks:
                        co = 512 * chunk
                        for mi, (mo, mc) in enumerate(KTILES):
                            ps = psp.tile([128, 512], F32, tag="mm")
                            for ki in range(2):
                                nc.tensor.matmul(
                                    ps[:mc, :], wproj[ki][:, mo:mo + mc],
                                    aom[chunk][ki][:, :],
                                    start=(ki == 0), stop=(ki == 1))
                            of = ev.tile([128, 512], F32, tag="of")
                            evict(of[:mc, :], ps[:mc, :],
                                  bias=projb_s[:mc, mi:mi + 1])
                            nc.sync.dma_start(
                                out=out[mo:mo + mc, co:co + 512],
                                in_=of[:mc, :])

            for b in range(B):
                for t_ in (1, 2, 3):
                    qkv_slab(b, t_)
                for t_o in range(T):
                    # qkv first: its ACT evictions land ahead of dw's in the
                    # in-order ACT queue and are ready earlier (they depend on
                    # qkv matmuls, which also run first on PE) — avoids
                    # head-of-line blocking of the dw PSUM evictions.
                    if t_o + 4 <= T:
                        qkv_slab(b, t_o + 4)
                    dw_chunk(b, t_o)
                    if t_o >= 1:
                        # grams for chunk t_o-1 (its evictions are done)
                        gram_pairs(b, 2 * (t_o - 1), 2 * t_o)
                    if b == 1:
                        # hide batch 0's attention tail inside batch 1's dw
                        if t_o == 0:
                            softmax_section(0)
                        elif t_o == 2:
                            av_chunks(0, 0, 4)
                gram_pairs(b, 14, 16)
                gram_allreduce(b)
            # tail: batch 0's last 4 chunks fill the PE while batch 1's
            # softmax chain runs on DVE/ACT/DMA
            av_chunks(0, 4, 8, use_act=True)
            softmax_section(1)
            av_chunks(1, 0, T, use_act=True)
    nc.compile()
    return nc


def _prep_inputs(x, qkv_w, qkv_b, dw_w, dw_b, temperature, proj_w, proj_b):
    """Host-side prep: fp8 hi/lo inputs, prebuilt fp8 diag tiles, fp16 proj."""
    import ml_dtypes
    F8NP = ml_dtypes.float8_e4m3

    def q8(a):
        return np.asarray(a, np.float32).astype(F8NP)

    x = np.asarray(x, np.float32)
    b_, c_, t_, h_, w_ = x.shape  # 2, 192, 8, 64, 64
    qkv_w2 = np.asarray(qkv_w, np.float32).reshape(C3, DIM)
    dw_w2 = np.asarray(dw_w, np.float32).reshape(C3, 27)
    proj_w2 = np.asarray(proj_w, np.float32).reshape(DIM, DIM)
    # permute qkv channels: [q_h0, k_h0, q_h1, k_h1, ..., v] so each head's
    # (q,k) columns are adjacent after transpose (contiguous gram operands)
    perm = []
    for h in range(HEADS):
        perm.extend(range(HD * h, HD * (h + 1)))          # q_h
        perm.extend(range(DIM + HD * h, DIM + HD * (h + 1)))  # k_h
    perm.extend(range(2 * DIM, 3 * DIM))                  # v unchanged
    perm = np.array(perm)
    qkv_w2 = qkv_w2[perm]
    dw_w2 = dw_w2[perm]
    qkv_b = np.asarray(qkv_b, np.float32)[perm]
    dw_b = np.asarray(dw_b, np.float32)[perm]

    # qkv weights x32, hi/lo, laid out [96, 2, 576] -> [96, 1152]
    w32 = 32.0 * qkv_w2  # [576, 192]
    whi = q8(w32).astype(np.float32)
    wlo = q8(w32 - whi).astype(np.float32)

    def wq_layout(w):  # [576(out), 192(in)] -> [96, 2*576] fp8
        wt = np.ascontiguousarray(w.T)          # [192, 576]
        return np.ascontiguousarray(
            wt.reshape(2, 96, C3).transpose(1, 0, 2).reshape(96, 2 * C3)
        ).astype(F8NP)

    wq8hi_h = wq_layout(whi)
    wq8lo_h = wq_layout(wlo)

    qkvb8_h = np.zeros((128, 5), np.float32)
    for mi, (mo, mc) in enumerate(MTILES):
        qkvb8_h[:mc, mi] = 8.0 * qkv_b[mo:mo + mc]

    # dw diag tiles (whi|wlo) per tap, x32
    d32 = 32.0 * dw_w2  # [576, 27]
    dhi = q8(d32).astype(np.float32)
    dlo = q8(d32 - dhi).astype(np.float32)
    dgd_h = []
    for mi, (mo, mc) in enumerate(MTILES):
        t = np.zeros((mc, 27 * 2 * mc), np.float32)
        idx = np.arange(mc)
        for ti in range(27):
            t[idx, ti * 2 * mc + idx] = dhi[mo:mo + mc, ti]
            t[idx, ti * 2 * mc + mc + idx] = dlo[mo:mo + mc, ti]
        dgd_h.append(t.astype(F8NP))
    dgp_h = {}
    for mi in V_MIS:
        mo, mc = MTILES[mi]
        t = np.zeros((mc, NPAIRS * 2 * mc), np.float32)
        idx = np.arange(mc)
        for pos, (ta, tb) in enumerate(_PAIR_KEYS):
            t[idx, pos * 2 * mc + idx] = dhi[mo:mo + mc, ta]
            if tb is not None:
                t[idx, pos * 2 * mc + mc + idx] = dhi[mo:mo + mc, tb]
        dgp_h[mi] = t.astype(F8NP)

    dwbqk_h = np.zeros((128, 3), np.float32)
    for mi in QK_MIS:
        mo, mc = MTILES[mi]
        dwbqk_h[:mc, mi] = 32.0 * dw_b[mo:mo + mc]
    dwbv_h = np.zeros((128, 2), np.float32)
    for j, mi in enumerate(V_MIS):
        mo, mc = MTILES[mi]
        dwbv_h[:mc, j] = dw_b[mo:mo + mc]

    projwT_h = np.ascontiguousarray(proj_w2.T).astype(np.float16)
    projb_h = np.zeros((128, 2), np.float32)
    projb_h[:128, 0] = np.asarray(proj_b, np.float32)[0:128]
    projb_h[:64, 1] = np.asarray(proj_b, np.float32)[128:192]
    temp_h = np.tile(np.asarray(temperature, np.float32).reshape(HEADS),
                     2).reshape(16, 1)  # g = b*8+h

    in_maps = []
    for i in range(NCORES):
        # padded slab [b, t10, h10, w66], h rows 8i-1 .. 8i+9 clamped->zero
        xs = np.zeros((b_, TP, HP, WP, c_), np.float32)
        hlo, hhi = 8 * i - 1, 8 * i + 9
        slo, shi = max(0, hlo), min(h_, hhi)
        xt = x[:, :, :, slo:shi, :].transpose(0, 2, 3, 4, 1)
        xs[:, 1:9, (slo - hlo):(slo - hlo) + (shi - slo), 1:65, :] = xt
        xT = np.ascontiguousarray(
            xs.reshape(b_ * TP * SLAB, c_).T)  # [192, 13200] f32
        xhi = q8(xT).astype(np.float32)
        xlo_ = q8(xT - xhi)

        def x_layout(a):  # [192, NPADTOK] -> [96, 2*NPADTOK] fp8
            return np.ascontiguousarray(
                np.asarray(a, np.float32).reshape(2, 96, NPADTOK)
                .transpose(1, 0, 2).reshape(96, 2 * NPADTOK)).astype(F8NP)

        m = {
            "x8hi": x_layout(xhi), "x8lo": x_layout(xlo_),
            "wq8hi": wq8hi_h, "wq8lo": wq8lo_h, "qkvb8": qkvb8_h,
            "dwbqk": dwbqk_h, "dwbv": dwbv_h,
            "projwT": projwT_h, "projb": projb_h, "temp": temp_h,
        }
        for mi in range(5):
            m[f"dg{mi}"] = dgd_h[mi]
        for mi in V_MIS:
            m[f"dgp{mi}"] = dgp_h[mi]
        in_maps.append(m)
    return in_maps


def _get_runner():
    """Build once; return a persistent sharded-jit callable (the per-call
    closure in bass2jax.run_bass_via_pjrt defeats jax's jit cache)."""
    if "runner" in _CACHE:
        return _CACHE["runner"]
    import jax
    for flag, val in [("jax_compilation_cache_dir", "/tmp/jax_kernel_cache"),
                      ("jax_persistent_cache_min_compile_time_secs", 1.0),
                      ("jax_persistent_cache_min_entry_size_bytes", 0)]:
        try:
            jax.config.update(flag, val)
        except Exception:
            pass
    from jax.sharding import Mesh, PartitionSpec
    from jax.experimental.shard_map import shard_map
    import concourse.mybir as mybir
    from concourse import bass2jax

    nc = _build()
    bass2jax.install_neuronx_cc_hook()

    partition_name = (nc.partition_id_tensor.name
                      if nc.partition_id_tensor else None)
    in_names, out_names, out_avals, zero_shapes = [], [], [], []
    for alloc in nc.m.functions[0].allocations:
        if not isinstance(alloc, mybir.MemoryLocationSet):
            continue
        name = alloc.memorylocations[0].name
        if alloc.kind == "ExternalInput":
            if name != partition_name:
                in_names.append(name)
        elif alloc.kind == "ExternalOutput":
            shape = tuple(alloc.tensor_shape)
            dtype = mybir.dt.np(alloc.dtype)
            out_names.append(name)
            out_avals.append(jax.core.ShapedArray(shape, dtype))
            zero_shapes.append((shape, dtype))
    n_params = len(in_names)
    all_names = in_names + out_names
    if partition_name is not None:
        all_names.append(partition_name)

    def _body(*args):
        operands = list(args)
        if partition_name is not None:
            operands.append(bass2jax.partition_id_tensor())
        outs = bass2jax._bass_exec_p.bind(
            *operands, out_avals=tuple(out_avals), in_names=tuple(all_names),
            out_names=tuple(out_names), lowering_input_output_aliases=(),
            sim_require_finite=True, sim_require_nnan=True, nc=nc)
        return tuple(outs)

    devices = jax.devices()[:NCORES]
    mesh = Mesh(np.asarray(devices), ("core",))
    n_outs = len(out_names)
    sharded = jax.jit(
        shard_map(_body, mesh=mesh,
                  in_specs=(PartitionSpec("core"),) * (n_params + n_outs),
                  out_specs=(PartitionSpec("core"),) * n_outs,
                  check_rep=False),
        donate_argnums=tuple(range(n_params, n_params + n_outs)),
        keep_unused=True)

    def run(in_maps):
        concat_in = [np.concatenate([in_maps[c][nm] for c in range(NCORES)],
                                    axis=0) for nm in in_names]
        concat_zeros = [np.zeros((NCORES * s[0], *s[1:]), dt)
                        for s, dt in zero_shapes]
        out_arrs = sharded(*concat_in, *concat_zeros)
        return [
            {nm: np.asarray(out_arrs[i]).reshape(NCORES, *out_avals[i].shape)[c]
             for i, nm in enumerate(out_names)}
            for c in range(NCORES)]

    _CACHE["runner"] = run
    return run


def kernel(x, qkv_w, qkv_b, dw_w, dw_b, temperature, proj_w, proj_b):
    run = _get_runner()
    in_maps = _prep_inputs(x, qkv_w, qkv_b, dw_w, dw_b, temperature,
                           proj_w, proj_b)
    results = run(in_maps)
    b_, c_, t_, h_, w_ = np.asarray(x).shape
    outf = np.empty((b_, c_, t_, h_, w_), np.float32)
    for i in range(NCORES):
        o = results[i]["out"].reshape(c_, b_, t_, H, w_)
        outf[:, :, :, 8 * i:8 * i + 8, :] = o.transpose(1, 0, 2, 3, 4)
    return outf


# revision 43
# speedup vs baseline: 1.4839x; 1.0541x over previous
"""nn_AttentionC Trainium2 kernel (8 NeuronCores, SPMD) — fp8 DoubleRow.

Sharding: h-axis (64) split into 8 chunks of 8 rows, one per core; each core's
x slab is host-padded to [b2, t10, h10, w66] tokens (conv zero-padding baked
in). Only cross-core traffic: AllReduce of per-(b,head) [48,48] q/k gram
matrices (110 KB).

Precision scheme (validated vs fp32 reference, rel_err ~2e-3):
  - weights everywhere: fp8e4m3 hi+lo (pre-scaled x32; coherent quant noise
    does not average out, so 2 words needed)
  - qkv input x: fp8 hi+lo from host (v-path needs the lo word)
  - q,k dw slabs + gram operands: single fp8 (x8 / x32 pre-scale) — random
    per-element noise washes out in the 32k-token gram contraction
  - v path slabs: fp8 hi+lo (x8 pre-scale); v output, attn, proj: fp16
All fp8 matmuls use DoubleRow perf mode (2 K-tiles/instr, 0.5 cyc/row):
  qkv: (whi|wlo)@xhi [+ whi@xlo for v tiles], contraction 192 split 96+96
  dw:  per tap (whi|wlo)-diag @ stride-0-paired window; v adds whi@xlo with
       taps paired two-per-instr via manually built two-offset APs
  gram: two 128-token tiles per instr.
"""
import numpy as np

DIM = 192
HEADS = 8
HD = DIM // HEADS  # 24
B, T, H, W = 2, 8, 8, 64  # per-core owned h rows = 8
HP, WP, TP = 10, 66, 10
SLAB = HP * WP  # 660
SLAB_AL = 664  # allocated slab stride (aligned)
NTOK = B * T * H * W  # 8192 owned tokens per core
NTOKH = NTOK // 2  # v2 (64-ch v tile) holds half0 rows in p0:64, half1 in p64:
NCORES = 8
C3 = 3 * DIM
CW = C3 + 64  # qkv weight cols incl. duplicated mi4 block (half-pairing)
NPADTOK = B * TP * SLAB  # 13200
NPT = TP * SLAB_AL  # 6640 padded cols per (b) big slab tile

_CACHE = {}

MTILES = [(0, 128), (128, 128), (256, 128), (384, 128), (512, 64)]
QK_MIS = (0, 1, 2)
V_MIS = (3, 4)
TAPS = [(dt, dh, dw) for dt in (-1, 0, 1) for dh in (-1, 0, 1)
        for dw in (-1, 0, 1)]


def _tap_idx(dt, i):
    return (dt + 1) * 9 + i


# xlo-pass tap pairings (pairs of tap indices; second None = zero k-tile)
def _xlo_pairs(kind):
    if kind == "int":
        prs = [(_tap_idx(-1, i), _tap_idx(0, i)) for i in range(9)]
        prs += [(_tap_idx(1, 2 * k), _tap_idx(1, 2 * k + 1)) for k in range(4)]
        prs += [(_tap_idx(1, 8), None)]
    elif kind == "e0":
        prs = [(_tap_idx(0, i), _tap_idx(1, i)) for i in range(9)]
    else:  # e7
        prs = [(_tap_idx(-1, i), _tap_idx(0, i)) for i in range(9)]
    return prs


# ordered unique pair keys across all chunk kinds
_PAIR_KEYS = []
for _kind in ("int", "e0", "e7"):
    for _p in _xlo_pairs(_kind):
        if _p not in _PAIR_KEYS:
            _PAIR_KEYS.append(_p)
_PAIR_POS = {p: i for i, p in enumerate(_PAIR_KEYS)}
NPAIRS = len(_PAIR_KEYS)


def _win_off(t_slab, dh, dw, half):
    """Column offset of a 264-wide tap window in the per-b big slab tile."""
    return SLAB_AL * t_slab + WP * (1 + dh + 4 * half) + (1 + dw)


def _build():
    import concourse.bacc as bacc
    import concourse.mybir as mybir
    import concourse.tile as tile
    from concourse import masks
    from concourse.ap import AP as APc

    F32 = mybir.dt.float32
    F16 = mybir.dt.float16
    F8 = mybir.dt.float8e4
    AL = mybir.AluOpType
    AF = mybir.ActivationFunctionType
    AX = mybir.AxisListType
    PM = mybir.MatmulPerfMode

    NW = 264  # tap window width (4 h-rows x 66)

    def pair_ap(full, off_a, off_b, n=NW):
        """[p, 2, n] AP over `full` (a [p, X] AP) with windows at two offsets."""
        base = full[:, off_a:off_a + n]
        return APc(base.tensor, base.offset,
                   [list(d) for d in base.ap[:1]] + [[off_b - off_a, 2]] +
                   [list(d) for d in base.ap[1:]])

    nc = bacc.Bacc("TRN2", target_bir_lowering=False, debug=False,
                   num_devices=NCORES)

    # ---- dram inputs ----
    x8hi = nc.dram_tensor("x8hi", [96, 2 * NPADTOK], F8,
                          kind="ExternalInput").ap()
    x8lo = nc.dram_tensor("x8lo", [96, 2 * NPADTOK], F8,
                          kind="ExternalInput").ap()
    wq8hi = nc.dram_tensor("wq8hi", [96, 2 * CW], F8,
                           kind="ExternalInput").ap()
    wq8lo = nc.dram_tensor("wq8lo", [96, 2 * CW], F8,
                           kind="ExternalInput").ap()
    qkvb8 = nc.dram_tensor("qkvb8", [128, 5], F32, kind="ExternalInput").ap()
    # prebuilt diag tiles: per mtile, 27 x [mcw, 2mcw] (whi|wlo); mi4 is
    # half-paired: 128 partitions with the 64 channels' diag values repeated
    DMCW = [128, 128, 128, 128, 128]
    dgd = [nc.dram_tensor(f"dg{mi}", [DMCW[mi], 27 * 2 * DMCW[mi]], F8,
                          kind="ExternalInput").ap()
           for mi in range(5)]
    # xlo-pass pair diags for v mtiles: NPAIRS x [mcw, 2mcw] (whi_a|whi_b)
    dgp = {mi: nc.dram_tensor(f"dgp{mi}", [DMCW[mi],
                                           NPAIRS * 2 * DMCW[mi]], F8,
                              kind="ExternalInput").ap()
           for mi in V_MIS}
    dwbqk = nc.dram_tensor("dwbqk", [128, 3], F32, kind="ExternalInput").ap()
    dwbv = nc.dram_tensor("dwbv", [128, 2], F32, kind="ExternalInput").ap()
    projwT = nc.dram_tensor("projwT", [DIM, DIM], F16,
                            kind="ExternalInput").ap()
    projb = nc.dram_tensor("projb", [128, 2], F32, kind="ExternalInput").ap()
    temp = nc.dram_tensor("temp", [16, 1], F32, kind="ExternalInput").ap()
    out = nc.dram_tensor("out", [DIM, NTOK], F32, kind="ExternalOutput").ap()

    gram_in = nc.dram_tensor("gram_in", [16, 48, 48], F32).ap()
    gram_out = nc.dram_tensor("gram_out", [16, 48, 48], F32,
                              addr_space="Shared").ap()
    attn_dram = nc.dram_tensor("attn_dram", [16, HD, HD], F16).ap()

    KTILES = [(0, 128), (128, 64)]  # fp16 contraction split (proj/attn@v)

    with tile.TileContext(nc) as tc:
        with (
            tc.tile_pool(name="wp", bufs=1) as wp,
            tc.tile_pool(name="xp", bufs=3) as xp,
            tc.tile_pool(name="slb", bufs=1) as slb,
            tc.tile_pool(name="qk", bufs=1) as qkpool,
            tc.tile_pool(name="ev", bufs=4) as ev,
            tc.tile_pool(name="evh", bufs=2) as evh,
            tc.tile_pool(name="small", bufs=1) as sp,
            tc.tile_pool(name="ps", bufs=5, space="PSUM") as psp,
            tc.tile_pool(name="pst", bufs=2, space="PSUM") as pst,
            tc.tile_pool(name="psg", bufs=1, space="PSUM") as psg,
        ):
            # ---------------- weights ----------------
            # qkv weights on the SP queue (needed first); the 6MB of diag
            # tiles go on the idle Pool DGE queue so the x-slab DMAs are not
            # stuck behind them.
            wqhi = wp.tile([96, 2 * CW], F8, tag="wqhi")
            nc.sync.dma_start(out=wqhi[:], in_=wq8hi)
            wqlo = wp.tile([96, 2 * CW], F8, tag="wqlo")
            nc.sync.dma_start(out=wqlo[:], in_=wq8lo)
            qkvb_s = wp.tile([128, 5], F32, tag="qkvb")
            nc.sync.dma_start(out=qkvb_s[:], in_=qkvb8)
            dg_s = []
            for mi in range(5):
                mcw = DMCW[mi]
                t = wp.tile([mcw, 27 * 2 * mcw], F8, tag=f"dg{mi}")
                nc.gpsimd.dma_start(out=t[:], in_=dgd[mi])
                dg_s.append(t)
            dgp_s = {}
            for mi in V_MIS:
                mcw = DMCW[mi]
                t = wp.tile([mcw, NPAIRS * 2 * mcw], F8, tag=f"dgp{mi}")
                nc.gpsimd.dma_start(out=t[:], in_=dgp[mi])
                dgp_s[mi] = t
            dwbqk_s = wp.tile([128, 3], F32, tag="dwbqk")
            nc.gpsimd.dma_start(out=dwbqk_s[:], in_=dwbqk)
            dwbv_s = wp.tile([128, 2], F32, tag="dwbv")
            nc.gpsimd.dma_start(out=dwbv_s[:], in_=dwbv)
            wproj = []
            for ki, (ko, kc) in enumerate(KTILES):
                t = wp.tile([kc, DIM], F16, tag=f"wproj{ki}")
                nc.sync.dma_start(out=t[:], in_=projwT[ko:ko + kc, :])
                wproj.append(t)
            projb_s = wp.tile([128, 2], F32, tag="projb")
            nc.sync.dma_start(out=projb_s[:], in_=projb)
            temp_s = wp.tile([16, 1], F32, tag="temp")
            nc.sync.dma_start(out=temp_s[:], in_=temp)

            ident8 = wp.tile([128, 128], F8, tag="ident8")
            masks.make_identity(nc, ident8[:])

            def dg_ap(mi, ti):
                mc = DMCW[mi]
                return dg_s[mi][:, ti * 2 * mc:(ti + 1) * 2 * mc].rearrange(
                    "p (two m) -> p two m", two=2)

            def dgp_ap(mi, pos):
                mc = DMCW[mi]
                return dgp_s[mi][:, pos * 2 * mc:(pos + 1) * 2 * mc].rearrange(
                    "p (two m) -> p two m", two=2)

            # dw outputs: q~,k~ (384 ch) fp8 x32, one batch at a time since
            # transposes+grams drain per batch; v (192 ch) fp16 both batches
            NTOKB = NTOK // B  # 4096
            qk_t = [qkpool.tile([128, NTOKB], F8, tag=f"qk{i}", name=f"qk{i}")
                    for i in range(3)]
            # v tile 0: 128 channels, both halves along tokens; v tile 1
            # (64 ch) is half-paired: p0:64 half0, p64:128 half1 per chunk
            v_t = [qkpool.tile([128, NTOK], F16, tag="v0", name="v0"),
                   qkpool.tile([128, NTOKH], F16, tag="v1", name="v1")]
            # base-0 copy of v2's odd-half partitions (the BIR path rejects
            # base-64 matmuls; DMA legally shifts partitions)
            v2b = qkpool.tile([64, NTOKH], F16, tag="v2b", name="v2b")

            # per-b big slab tiles (all 10 t-slabs contiguous, stride 664);
            # mi4 half-paired: p64:128 hold slab cols shifted by -264
            sl_hi = [slb.tile([DMCW[mi], NPT], F8, tag=f"slhi{mi}",
                              name=f"slhi{mi}") for mi in range(5)]
            sl_lo = {mi: slb.tile([DMCW[mi], NPT], F8, tag=f"sllo{mi}",
                                  name=f"sllo{mi}") for mi in V_MIS}
            # zero the never-written tails read by the widest tap windows
            # (their products land in trimmed output columns): the 4 gap
            # cols after each slab, and mi4's shifted-half tail [396, 664)
            for t_z in range(1, 9):
                for tl in (sl_hi[4], sl_lo[4]):
                    nc.vector.memset(
                        tl[64:128, SLAB_AL * t_z + 396:SLAB_AL * (t_z + 1)],
                        0.0)
                for tl in (sl_hi[0], sl_hi[1], sl_hi[2], sl_hi[3],
                           sl_lo[3]):
                    nc.vector.memset(
                        tl[:, SLAB_AL * t_z + 660:SLAB_AL * (t_z + 1)], 0.0)
                for tl in (sl_hi[4], sl_lo[4]):
                    nc.vector.memset(
                        tl[0:64, SLAB_AL * t_z + 660:SLAB_AL * (t_z + 1)],
                        0.0)

            # ---------------- qkv conv ----------------
            def qkv_slab(b, t_):
                xhi = xp.tile([96, 2 * SLAB_AL], F8, tag="xhi")
                xlo = xp.tile([96, 2 * SLAB_AL], F8, tag="xlo")
                off = (b * TP + t_) * SLAB
                for src, dst in ((x8hi, xhi), (x8lo, xlo)):
                    nc.sync.dma_start(
                        out=dst[:].rearrange("p (two n) -> p two n",
                                             two=2)[:, :, 0:SLAB],
                        in_=src.rearrange("p (two n) -> p two n",
                                          two=2)[:, :, off:off + SLAB])
                xhi2 = xhi[:].rearrange("p (two n) -> p two n", two=2)
                xlo2 = xlo[:].rearrange("p (two n) -> p two n", two=2)
                base = SLAB_AL * t_
                for mi in range(5):
                    mo = MTILES[mi][0]
                    mc = DMCW[mi]
                    whi2 = wqhi[:].rearrange("p (two m) -> p two m",
                                             two=2)[:, :, mo:mo + mc]
                    wlo2 = wqlo[:].rearrange("p (two m) -> p two m",
                                             two=2)[:, :, mo:mo + mc]
                    for half in range(2):
                        cs = 330 * half
                        ce = cs + 330
                        ps = psp.tile([128, 512], F32, tag="mm")
                        three = mi in V_MIS
                        nc.tensor.matmul(ps[:mc, :330], whi2,
                                         xhi2[:, :, cs:ce], start=True,
                                         stop=False, perf_mode=PM.DoubleRow)
                        nc.tensor.matmul(ps[:mc, :330], wlo2,
                                         xhi2[:, :, cs:ce], start=False,
                                         stop=not three,
                                         perf_mode=PM.DoubleRow)
                        if three:
                            nc.tensor.matmul(ps[:mc, :330], whi2,
                                             xlo2[:, :, cs:ce], start=False,
                                             stop=True,
                                             perf_mode=PM.DoubleRow)
                        dst = base + cs
                        # hi slab: 8*(qkv) = psum/4 + 8b
                        nhi = 64 if mi == 4 else mc
                        nc.scalar.activation(
                            sl_hi[mi][:nhi, dst:dst + 330], ps[:nhi, :330],
                            AF.Identity, bias=qkvb_s[:nhi, mi:mi + 1],
                            scale=0.25)
                        if mi == 4:
                            # shifted copy for half-paired layout: p64:128
                            # col j holds slab col j+264
                            if half == 0:
                                nc.scalar.activation(
                                    sl_hi[4][64:128, base:base + 66],
                                    ps[64:128, 264:330], AF.Identity,
                                    bias=qkvb_s[64:128, 4:5], scale=0.25)
                            else:
                                nc.scalar.activation(
                                    sl_hi[4][64:128, base + 66:base + 396],
                                    ps[64:128, 0:330], AF.Identity,
                                    bias=qkvb_s[64:128, 4:5], scale=0.25)
                        if three:
                            half32 = evh.tile([mc, 330], F32, tag=f"h32_{mi}")
                            nc.scalar.activation(
                                half32[:], ps[:mc, :330], AF.Identity,
                                bias=qkvb_s[:mc, mi:mi + 1], scale=0.25)
                            nc.vector.tensor_sub(
                                sl_lo[mi][:nhi, dst:dst + 330],
                                half32[:nhi], sl_hi[mi][:nhi, dst:dst + 330])
                            if mi == 4:
                                if half == 0:
                                    nc.vector.tensor_sub(
                                        sl_lo[4][64:128, base:base + 66],
                                        half32[64:128, 264:330],
                                        sl_hi[4][64:128, base:base + 66])
                                else:
                                    nc.vector.tensor_sub(
                                        sl_lo[4][64:128,
                                                 base + 66:base + 396],
                                        half32[64:128, 0:330],
                                        sl_hi[4][64:128,
                                                 base + 66:base + 396])

            # ---------------- depthwise ----------------
            def dw_chunk(b, t_o):
                if t_o == 0:
                    kind, dts = "e0", (0, 1)
                elif t_o == T - 1:
                    kind, dts = "e7", (-1, 0)
                else:
                    kind, dts = "int", (-1, 0, 1)
                keep = [(dt + 1) * 9 + i for dt in dts for i in range(9)]
                xpairs = _xlo_pairs(kind)

                def dw_group(mi, half):
                    # one accumulation group: all taps of (chunk, mtile).
                    # mi4 runs once per chunk (half-paired partitions).
                    mc = DMCW[mi]
                    ps = psp.tile([128, 512], F32, tag="mm")
                    last_hi = (mi not in V_MIS)
                    for j, ti in enumerate(keep):
                        dt, dh, dw = TAPS[ti]
                        off = _win_off(t_o + 1 + dt, dh, dw, half)
                        rhs = pair_ap(sl_hi[mi][:], off, off)
                        nc.tensor.matmul(
                            ps[:mc, :NW], dg_ap(mi, ti), rhs,
                            start=(j == 0),
                            stop=(last_hi and j == len(keep) - 1),
                            perf_mode=PM.DoubleRow)
                    if not last_hi:
                        for j, (ta, tb) in enumerate(xpairs):
                            dta, dha, dwa = TAPS[ta]
                            offa = _win_off(t_o + 1 + dta, dha, dwa, half)
                            if tb is None:
                                offb = offa
                            else:
                                dtb, dhb, dwb_ = TAPS[tb]
                                offb = _win_off(t_o + 1 + dtb, dhb, dwb_,
                                                half)
                            rhs = pair_ap(sl_lo[mi][:], offa, offb)
                            nc.tensor.matmul(
                                ps[:mc, :NW],
                                dgp_ap(mi, _PAIR_POS[(ta, tb)]), rhs,
                                start=False, stop=(j == len(xpairs) - 1),
                                perf_mode=PM.DoubleRow)
                    return ps

                for half in range(2):
                    co = 256 * ((b * T + t_o) * 2 + half)      # v tiles
                    cob = 256 * (t_o * 2 + half)               # qk (per-batch)
                    for mi in range(4):
                        ps = dw_group(mi, half)
                        pv = ps[:, :NW].rearrange(
                            "p (h w) -> p h w", h=4)[:, :, 0:64]
                        if mi in QK_MIS:
                            # qk8 = 32*(dw+b) = psum/8 + 32b
                            nc.scalar.activation(
                                qk_t[mi][:, cob:cob + 256].rearrange(
                                    "p (h w) -> p h w", h=4),
                                pv[:128], AF.Identity,
                                bias=dwbqk_s[:128, mi:mi + 1], scale=0.125)
                        else:
                            nc.scalar.activation(
                                v_t[0][:, co:co + 256].rearrange(
                                    "p (h w) -> p h w", h=4),
                                pv[:128], AF.Identity,
                                bias=dwbv_s[:128, 0:1], scale=1.0 / 256)
                # mi4: both halves in one pass (p0:64 half0, p64:128 half1)
                cv = 256 * (b * T + t_o)
                ps = dw_group(4, 0)
                pv = ps[:, :NW].rearrange("p (h w) -> p h w", h=4)[:, :, 0:64]
                nc.scalar.activation(
                    v_t[1][0:64, cv:cv + 256].rearrange(
                        "p (h w) -> p h w", h=4),
                    pv[0:64], AF.Identity,
                    bias=dwbv_s[0:64, 1:2], scale=1.0 / 256)
                nc.scalar.activation(
                    v_t[1][64:128, cv:cv + 256].rearrange(
                        "p (h w) -> p h w", h=4),
                    pv[64:128], AF.Identity,
                    bias=dwbv_s[64:128, 1:2], scale=1.0 / 256)

            gps = psg.tile([48, 384], F32, tag="gram")  # reused across b
            dm = sp.tile([8, 576], F32, tag="dm")
            nc.gpsimd.memset(dm[:], 0.0)
            nc.gpsimd.affine_select(
                out=dm[:], in_=dm[:], compare_op=AL.not_equal, fill=1.0,
                base=0, pattern=[[1, 24], [-1, 24]], channel_multiplier=0)
            bd = {}

            def gram_pairs(b, pr0, pr1):
                # transposes + grams for c64 pairs [pr0, pr1)
                for pr in range(pr0, pr1):
                    c64a = 2 * pr  # qk_t holds the current batch only
                    qkT = ev.tile([128, 768], F8, tag="qkT")
                    for s in range(2):
                        for i in range(3):
                            # fp8 transpose requires output element step 2
                            tps = pst.tile([128, 256], F8, tag="trps")
                            nc.tensor.transpose(
                                tps[:, 0:256:2],
                                qk_t[i][:, 128 * (c64a + s):
                                        128 * (c64a + s + 1)],
                                ident8[:])
                            nc.vector.tensor_copy(
                                qkT[:, 384 * s + 128 * i:
                                    384 * s + 128 * (i + 1)],
                                tps[:, 0:256:2])
                    z2 = qkT[:].rearrange("p (two m) -> p two m", two=2)
                    for h in range(HEADS):
                        z = z2[:, :, 48 * h:48 * (h + 1)]
                        nc.tensor.matmul(
                            gps[:, 48 * h:48 * (h + 1)], z, z,
                            start=(pr == 0 and h == 0),
                            stop=(pr == 15 and h == HEADS - 1),
                            perf_mode=PM.DoubleRow)

            def gram_allreduce(b):
                gs = ev.tile([48, 384], F32, tag="gs")
                nc.vector.tensor_copy(gs[:], gps[:])
                nc.gpsimd.dma_start(
                    out=gram_in[8 * b:8 * (b + 1)].rearrange(
                        "g c d -> c g d"),
                    in_=gs[:].rearrange("c (g d) -> c g d", g=8))
                nc.gpsimd.collective_compute(
                    "AllReduce", AL.add,
                    replica_groups=[list(range(NCORES))],
                    ins=[gram_in[8 * b:8 * (b + 1)]],
                    outs=[gram_out[8 * b:8 * (b + 1)]])

            def softmax_section(b):
                # norms + softmax for one batch ([8, .]); grams carry a 1024x
                # scale (32q)(32k) which cancels exactly in the normalized
                # correlation.
                qq_f = sp.tile([8, 576], F32, tag="qqf")
                kk_f = sp.tile([8, 576], F32, tag="kkf")
                qk_f = sp.tile([8, 576], F32, tag="qkf")
                go = gram_out[8 * b:8 * (b + 1)]
                nc.sync.dma_start(
                    out=qq_f[:].rearrange("p (c d) -> p c d", c=24),
                    in_=go[:, 0:24, 0:24])
                nc.sync.dma_start(
                    out=kk_f[:].rearrange("p (c d) -> p c d", c=24),
                    in_=go[:, 24:48, 24:48])
                nc.sync.dma_start(
                    out=qk_f[:].rearrange("p (c d) -> p c d", c=24),
                    in_=go[:, 0:24, 24:48])

                def diag_rsqrt(src, tag):
                    t1 = sp.tile([8, 576], F32, tag="tmp576")
                    nc.vector.tensor_mul(t1[:], src[:], dm[:])
                    n2 = sp.tile([8, 24], F32, tag=tag + "b")
                    nc.vector.tensor_reduce(
                        n2[:], t1[:].rearrange("p (c d) -> p c d", c=24),
                        axis=AX.X, op=AL.add)
                    nrm = sp.tile([8, 24], F32, tag=tag + "c")
                    nc.scalar.sqrt(nrm[:], n2[:])
                    r = sp.tile([8, 24], F32, tag=tag + "d")
                    nc.vector.reciprocal(r[:], nrm[:])
                    return r

                rq = diag_rsqrt(qq_f, "rq")
                rk = diag_rsqrt(kk_f, "rk")
                # fold temperature into rq ([8,24] op instead of [8,576])
                nc.vector.tensor_scalar_mul(rq[:], rq[:], temp_s[:8])

                a1 = sp.tile([8, 576], F32, tag="a1")
                nc.vector.tensor_mul(
                    a1[:].rearrange("p (c d) -> p c d", c=24),
                    qk_f[:].rearrange("p (c d) -> p c d", c=24),
                    rq[:].rearrange("p (c one) -> p c one",
                                    one=1).broadcast_to((8, 24, 24)))
                nc.vector.tensor_mul(
                    a1[:].rearrange("p (c d) -> p c d", c=24),
                    a1[:].rearrange("p (c d) -> p c d", c=24),
                    rk[:].rearrange("p (one d) -> p one d",
                                    one=1).broadcast_to((8, 24, 24)))
                # logits are normalized correlations * temp, |x| <= ~1:
                # exp() is safe without the max-subtract pass
                ex = sp.tile([8, 576], F32, tag="ex")
                nc.scalar.activation(ex[:], a1[:], AF.Exp)
                sm = sp.tile([8, 24], F32, tag="sm")
                nc.vector.tensor_reduce(
                    sm[:], ex[:].rearrange("p (c d) -> p c d", c=24),
                    axis=AX.X, op=AL.add)
                rs = sp.tile([8, 24], F32, tag="rs")
                nc.vector.reciprocal(rs[:], sm[:])
                at16 = sp.tile([8, 576], F16, tag="at16")
                nc.vector.tensor_mul(
                    at16[:].rearrange("p (c d) -> p c d", c=24),
                    ex[:].rearrange("p (c d) -> p c d", c=24),
                    rs[:].rearrange("p (c one) -> p c one",
                                    one=1).broadcast_to((8, 24, 24)))
                nc.sync.dma_start(
                    out=attn_dram[8 * b:8 * (b + 1)],
                    in_=at16[:].rearrange("p (c d) -> p c d", c=24))

                # block-diag attn^T, split into k-tiles 128 + 64(dup at p64:
                # for the half-paired v2); spread the small loads across DGE
                # queues to cut serial issue latency
                bts = [sp.tile([128, DIM], F16, tag=f"bd{b}_0",
                               name=f"bd{b}_0"),
                       sp.tile([128, DIM], F16, tag=f"bd{b}_1",
                               name=f"bd{b}_1")]
                nc.vector.memset(bts[0][:], 0.0)
                nc.vector.memset(bts[1][:], 0.0)
                qs = [nc.sync, nc.gpsimd, nc.scalar]
                for h in range(HEADS):
                    q = qs[h % 3]
                    src_a = attn_dram[8 * b + h].rearrange("c d -> d c")
                    r0, r1 = HD * h, HD * (h + 1)
                    if r1 <= 128:
                        q.dma_start(out=bts[0][r0:r1, r0:r1], in_=src_a)
                    elif r0 >= 128:
                        q.dma_start(
                            out=bts[1][r0 - 128:r1 - 128, r0:r1], in_=src_a)
                        q.dma_start(
                            out=bts[1][r0 - 64:r1 - 64, r0:r1], in_=src_a)
                    else:  # h == 5: rows 120..144 straddle the k-tile split
                        q.dma_start(
                            out=bts[0][r0:128, r0:r1], in_=src_a[0:128 - r0])
                        q.dma_start(
                            out=bts[1][0:r1 - 128, r0:r1],
                            in_=src_a[128 - r0:HD])
                        q.dma_start(
                            out=bts[1][64:r1 - 64, r0:r1],
                            in_=src_a[128 - r0:HD])
                bd[b] = bts

            def av_chunks(b, lo, hi, use_act=False):
                # two-phase per 4-chunk block: all attn@v groups first (PSUM
                # evicted to aos), then all proj groups — hides the aos copy
                # latency behind the other chunks' matmuls. use_act routes
                # PSUM evictions to the ACT engine (idle during the tail,
                # while DVE runs the softmax chain).
                def evict(dst, src, bias=None):
                    if use_act:
                        if bias is None:
                            nc.scalar.activation(dst, src, AF.Identity)
                        else:
                            nc.scalar.activation(dst, src, AF.Identity,
                                                 bias=bias, scale=1.0)
                    else:
                        if bias is None:
                            nc.vector.tensor_copy(dst, src)
                        else:
                            nc.vector.tensor_scalar(dst, src, bias, None,
                                                    AL.add)
                for blk in range(lo, hi, 4):
                    chunks = list(range(b * T + blk,
                                        b * T + min(blk + 4, hi)))
                    aom = {}
                    for chunk in chunks:
                        co = 512 * chunk
                        cv = 256 * chunk
                        aos = [ev.tile([kc, 512], F16, tag=f"ao{ki}",
                                       name=f"ao{ki}")
                               for ki, (ko, kc) in enumerate(KTILES)]
                        for mi, (mo, mc) in enumerate(KTILES):
                            ps = psp.tile([128, 512], F32, tag="mm")
                            nc.tensor.matmul(
                                ps[:mc, :], bd[b][0][:, mo:mo + mc],
                                v_t[0][:, co:co + 512],
                                start=True, stop=False)
                            # v2 is half-paired: half0 tokens from v_t[1]
                            # p0:64, half1 from the base-0 copy v2b
                            nc.tensor.matmul(
                                ps[:mc, 0:256], bd[b][1][0:64, mo:mo + mc],
                                v_t[1][0:64, cv:cv + 256],
                                start=False, stop=False)
                            nc.tensor.matmul(
                                ps[:mc, 256:512],
                                bd[b][1][0:64, mo:mo + mc],
                                v2b[:, cv:cv + 256],
                                start=False, stop=True)
                            evict(aos[mi][:, :], ps[:mc, :])
                        aom[chunk] = aos
                    for chunk in chunks:
                        co = 512 * chunk
                        for mi, (mo, mc) in enumerate(KTILES):
                            ps = psp.tile([128, 512], F32, tag="mm")
                            for ki in range(2):
                                nc.tensor.matmul(
                                    ps[:mc, :], wproj[ki][:, mo:mo + mc],
                                    aom[chunk][ki][:, :],
                                    start=(ki == 0), stop=(ki == 1))
                            of = ev.tile([128, 512], F32, tag="of")
                            evict(of[:mc, :], ps[:mc, :],
                                  bias=projb_s[:mc, mi:mi + 1])
                            nc.sync.dma_start(
                                out=out[mo:mo + mc, co:co + 512],
                                in_=of[:mc, :])

            for b in range(B):
                for t_ in (1, 2, 3):
                    qkv_slab(b, t_)
                for t_o in range(T):
                    # qkv first: its ACT evictions land ahead of dw's in the
                    # in-order ACT queue and are ready earlier (they depend on
                    # qkv matmuls, which also run first on PE) — avoids
                    # head-of-line blocking of the dw PSUM evictions.
                    if t_o + 4 <= T:
                        qkv_slab(b, t_o + 4)
                    dw_chunk(b, t_o)
                    if t_o >= 1:
                        # grams for chunk t_o-1 (its evictions are done)
                        gram_pairs(b, 2 * (t_o - 1), 2 * t_o)
                    if t_o == T - 1:
                        # stage this batch's odd-half v2 rows at base 0
                        nc.gpsimd.dma_start(
                            out=v2b[:, 2048 * b:2048 * (b + 1)],
                            in_=v_t[1][64:128, 2048 * b:2048 * (b + 1)])
                    if b == 1:
                        # hide batch 0's attention tail inside batch 1's dw
                        if t_o == 0:
                            softmax_section(0)
                        elif t_o == 2:
                            av_chunks(0, 0, 4)
                gram_pairs(b, 14, 16)
                gram_allreduce(b)
            # tail: batch 0's last 4 chunks fill the PE while batch 1's
            # softmax chain runs on DVE/ACT/DMA
            av_chunks(0, 4, 8, use_act=True)
            softmax_section(1)
            av_chunks(1, 0, T, use_act=True)
    nc.compile()
    return nc


def _prep_inputs(x, qkv_w, qkv_b, dw_w, dw_b, temperature, proj_w, proj_b):
    """Host-side prep: fp8 hi/lo inputs, prebuilt fp8 diag tiles, fp16 proj."""
    import ml_dtypes
    F8NP = ml_dtypes.float8_e4m3

    def q8(a):
        return np.asarray(a, np.float32).astype(F8NP)

    x = np.asarray(x, np.float32)
    b_, c_, t_, h_, w_ = x.shape  # 2, 192, 8, 64, 64
    qkv_w2 = np.asarray(qkv_w, np.float32).reshape(C3, DIM)
    dw_w2 = np.asarray(dw_w, np.float32).reshape(C3, 27)
    proj_w2 = np.asarray(proj_w, np.float32).reshape(DIM, DIM)
    # permute qkv channels: [q_h0, k_h0, q_h1, k_h1, ..., v] so each head's
    # (q,k) columns are adjacent after transpose (contiguous gram operands)
    perm = []
    for h in range(HEADS):
        perm.extend(range(HD * h, HD * (h + 1)))          # q_h
        perm.extend(range(DIM + HD * h, DIM + HD * (h + 1)))  # k_h
    perm.extend(range(2 * DIM, 3 * DIM))                  # v unchanged
    perm = np.array(perm)
    qkv_w2 = qkv_w2[perm]
    dw_w2 = dw_w2[perm]
    qkv_b = np.asarray(qkv_b, np.float32)[perm]
    dw_b = np.asarray(dw_b, np.float32)[perm]

    # qkv weights x32, hi/lo, laid out [96, 2, 576] -> [96, 1152]
    w32 = 32.0 * qkv_w2  # [576, 192]
    whi = q8(w32).astype(np.float32)
    wlo = q8(w32 - whi).astype(np.float32)

    def wq_layout(w):  # [576(out), 192(in)] -> [96, 2*CW] fp8
        # append a duplicate of out-channels 512:576 (mi4 half-pairing)
        wd = np.concatenate([w, w[512:576]], axis=0)  # [640, 192]
        wt = np.ascontiguousarray(wd.T)               # [192, 640]
        return np.ascontiguousarray(
            wt.reshape(2, 96, CW).transpose(1, 0, 2).reshape(96, 2 * CW)
        ).astype(F8NP)

    wq8hi_h = wq_layout(whi)
    wq8lo_h = wq_layout(wlo)

    qkvb8_h = np.zeros((128, 5), np.float32)
    for mi, (mo, mc) in enumerate(MTILES):
        qkvb8_h[:mc, mi] = 8.0 * qkv_b[mo:mo + mc]
    qkvb8_h[64:128, 4] = qkvb8_h[0:64, 4]  # mi4 duplicated channels

    # dw diag tiles (whi|wlo) per tap, x32; mi4 replicated to 128 partitions
    d32 = 32.0 * dw_w2  # [576, 27]
    dhi = q8(d32).astype(np.float32)
    dlo = q8(d32 - dhi).astype(np.float32)
    DMCW = [128, 128, 128, 128, 128]

    def _dsel(mi, idx, src, ti):
        mo = MTILES[mi][0]
        return src[mo + (idx % 64), ti] if mi == 4 else src[mo + idx, ti]

    dgd_h = []
    for mi in range(5):
        mc = DMCW[mi]
        t = np.zeros((mc, 27 * 2 * mc), np.float32)
        idx = np.arange(mc)
        for ti in range(27):
            t[idx, ti * 2 * mc + idx] = _dsel(mi, idx, dhi, ti)
            t[idx, ti * 2 * mc + mc + idx] = _dsel(mi, idx, dlo, ti)
        dgd_h.append(t.astype(F8NP))
    dgp_h = {}
    for mi in V_MIS:
        mc = DMCW[mi]
        t = np.zeros((mc, NPAIRS * 2 * mc), np.float32)
        idx = np.arange(mc)
        for pos, (ta, tb) in enumerate(_PAIR_KEYS):
            t[idx, pos * 2 * mc + idx] = _dsel(mi, idx, dhi, ta)
            if tb is not None:
                t[idx, pos * 2 * mc + mc + idx] = _dsel(mi, idx, dhi, tb)
        dgp_h[mi] = t.astype(F8NP)

    dwbqk_h = np.zeros((128, 3), np.float32)
    for mi in QK_MIS:
        mo, mc = MTILES[mi]
        dwbqk_h[:mc, mi] = 32.0 * dw_b[mo:mo + mc]
    dwbv_h = np.zeros((128, 2), np.float32)
    dwbv_h[:128, 0] = dw_b[384:512]
    dwbv_h[0:64, 1] = dw_b[512:576]
    dwbv_h[64:128, 1] = dw_b[512:576]

    projwT_h = np.ascontiguousarray(proj_w2.T).astype(np.float16)
    projb_h = np.zeros((128, 2), np.float32)
    projb_h[:128, 0] = np.asarray(proj_b, np.float32)[0:128]
    projb_h[:64, 1] = np.asarray(proj_b, np.float32)[128:192]
    temp_h = np.tile(np.asarray(temperature, np.float32).reshape(HEADS),
                     2).reshape(16, 1)  # g = b*8+h

    in_maps = []
    for i in range(NCORES):
        # padded slab [b, t10, h10, w66], h rows 8i-1 .. 8i+9 clamped->zero
        xs = np.zeros((b_, TP, HP, WP, c_), np.float32)
        hlo, hhi = 8 * i - 1, 8 * i + 9
        slo, shi = max(0, hlo), min(h_, hhi)
        xt = x[:, :, :, slo:shi, :].transpose(0, 2, 3, 4, 1)
        xs[:, 1:9, (slo - hlo):(slo - hlo) + (shi - slo), 1:65, :] = xt
        xT = np.ascontiguousarray(
            xs.reshape(b_ * TP * SLAB, c_).T)  # [192, 13200] f32
        xhi = q8(xT).astype(np.float32)
        xlo_ = q8(xT - xhi)

        def x_layout(a):  # [192, NPADTOK] -> [96, 2*NPADTOK] fp8
            return np.ascontiguousarray(
                np.asarray(a, np.float32).reshape(2, 96, NPADTOK)
                .transpose(1, 0, 2).reshape(96, 2 * NPADTOK)).astype(F8NP)

        m = {
            "x8hi": x_layout(xhi), "x8lo": x_layout(xlo_),
            "wq8hi": wq8hi_h, "wq8lo": wq8lo_h, "qkvb8": qkvb8_h,
            "dwbqk": dwbqk_h, "dwbv": dwbv_h,
            "projwT": projwT_h, "projb": projb_h, "temp": temp_h,
        }
        for mi in range(5):
            m[f"dg{mi}"] = dgd_h[mi]
        for mi in V_MIS:
            m[f"dgp{mi}"] = dgp_h[mi]
        in_maps.append(m)
    return in_maps


def _get_runner():
    """Build once; return a persistent sharded-jit callable (the per-call
    closure in bass2jax.run_bass_via_pjrt defeats jax's jit cache)."""
    if "runner" in _CACHE:
        return _CACHE["runner"]
    import jax
    for flag, val in [("jax_compilation_cache_dir", "/tmp/jax_kernel_cache"),
                      ("jax_persistent_cache_min_compile_time_secs", 1.0),
                      ("jax_persistent_cache_min_entry_size_bytes", 0)]:
        try:
            jax.config.update(flag, val)
        except Exception:
            pass
    from jax.sharding import Mesh, PartitionSpec
    from jax.experimental.shard_map import shard_map
    import concourse.mybir as mybir
    from concourse import bass2jax

    nc = _build()
    bass2jax.install_neuronx_cc_hook()

    partition_name = (nc.partition_id_tensor.name
                      if nc.partition_id_tensor else None)
    in_names, out_names, out_avals, zero_shapes = [], [], [], []
    for alloc in nc.m.functions[0].allocations:
        if not isinstance(alloc, mybir.MemoryLocationSet):
            continue
        name = alloc.memorylocations[0].name
        if alloc.kind == "ExternalInput":
            if name != partition_name:
                in_names.append(name)
        elif alloc.kind == "ExternalOutput":
            shape = tuple(alloc.tensor_shape)
            dtype = mybir.dt.np(alloc.dtype)
            out_names.append(name)
            out_avals.append(jax.core.ShapedArray(shape, dtype))
            zero_shapes.append((shape, dtype))
    n_params = len(in_names)
    all_names = in_names + out_names
    if partition_name is not None:
        all_names.append(partition_name)

    def _body(*args):
        operands = list(args)
        if partition_name is not None:
            operands.append(bass2jax.partition_id_tensor())
        outs = bass2jax._bass_exec_p.bind(
            *operands, out_avals=tuple(out_avals), in_names=tuple(all_names),
            out_names=tuple(out_names), lowering_input_output_aliases=(),
            sim_require_finite=True, sim_require_nnan=True, nc=nc)
        return tuple(outs)

    devices = jax.devices()[:NCORES]
    mesh = Mesh(np.asarray(devices), ("core",))
    n_outs = len(out_names)
    sharded = jax.jit(
        shard_map(_body, mesh=mesh,
                  in_specs=(PartitionSpec("core"),) * (n_params + n_outs),
                  out_specs=(PartitionSpec("core"),) * n_outs,
                  check_rep=False),
        donate_argnums=tuple(range(n_params, n_params + n_outs)),
        keep_unused=True)

    def run(in_maps):
        concat_in = [np.concatenate([in_maps[c][nm] for c in range(NCORES)],
                                    axis=0) for nm in in_names]
        concat_zeros = [np.zeros((NCORES * s[0], *s[1:]), dt)
                        for s, dt in zero_shapes]
        out_arrs = sharded(*concat_in, *concat_zeros)
        return [
            {nm: np.asarray(out_arrs[i]).reshape(NCORES, *out_avals[i].shape)[c]
             for i, nm in enumerate(out_names)}
            for c in range(NCORES)]

    _CACHE["runner"] = run
    return run


def kernel(x, qkv_w, qkv_b, dw_w, dw_b, temperature, proj_w, proj_b):
    run = _get_runner()
    in_maps = _prep_inputs(x, qkv_w, qkv_b, dw_w, dw_b, temperature,
                           proj_w, proj_b)
    results = run(in_maps)
    b_, c_, t_, h_, w_ = np.asarray(x).shape
    outf = np.empty((b_, c_, t_, h_, w_), np.float32)
    for i in range(NCORES):
        o = results[i]["out"].reshape(c_, b_, t_, H, w_)
        outf[:, :, :, 8 * i:8 * i + 8, :] = o.transpose(1, 0, 2, 3, 4)
    return outf


# revision 47
# speedup vs baseline: 1.5224x; 1.0259x over previous
"""nn_AttentionC Trainium2 kernel (8 NeuronCores, SPMD) — fp8 DoubleRow.

Sharding: h-axis (64) split into 8 chunks of 8 rows, one per core; each core's
x slab is host-padded to [b2, t10, h10, w66] tokens (conv zero-padding baked
in). Only cross-core traffic: AllReduce of per-(b,head) [48,48] q/k gram
matrices (110 KB).

Precision scheme (validated vs fp32 reference, rel_err ~2e-3):
  - weights everywhere: fp8e4m3 hi+lo (pre-scaled x32; coherent quant noise
    does not average out, so 2 words needed)
  - qkv input x: fp8 hi+lo from host (v-path needs the lo word)
  - q,k dw slabs + gram operands: single fp8 (x8 / x32 pre-scale) — random
    per-element noise washes out in the 32k-token gram contraction
  - v path slabs: fp8 hi+lo (x8 pre-scale); v output, attn, proj: fp16
All fp8 matmuls use DoubleRow perf mode (2 K-tiles/instr, 0.5 cyc/row):
  qkv: (whi|wlo)@xhi [+ whi@xlo for v tiles], contraction 192 split 96+96
  dw:  per tap (whi|wlo)-diag @ stride-0-paired window; v adds whi@xlo with
       taps paired two-per-instr via manually built two-offset APs
  gram: two 128-token tiles per instr.
"""
import numpy as np

DIM = 192
HEADS = 8
HD = DIM // HEADS  # 24
B, T, H, W = 2, 8, 8, 64  # per-core owned h rows = 8
HP, WP, TP = 10, 66, 10
SLAB = HP * WP  # 660
SLAB_AL = 664  # allocated slab stride (aligned)
NTOK = B * T * H * W  # 8192 owned tokens per core
NTOKH = NTOK // 2  # v2 (64-ch v tile) holds half0 rows in p0:64, half1 in p64:
NCORES = 8
C3 = 3 * DIM
CW = C3 + 64  # qkv weight cols incl. duplicated mi4 block (half-pairing)
NPADTOK = B * TP * SLAB  # 13200
NPT = TP * SLAB_AL  # 6640 padded cols per (b) big slab tile

_CACHE = {}

MTILES = [(0, 128), (128, 128), (256, 128), (384, 128), (512, 64)]
QK_MIS = (0, 1, 2)
V_MIS = (3, 4)
TAPS = [(dt, dh, dw) for dt in (-1, 0, 1) for dh in (-1, 0, 1)
        for dw in (-1, 0, 1)]


def _tap_idx(dt, i):
    return (dt + 1) * 9 + i


# xlo-pass tap pairings (pairs of tap indices; second None = zero k-tile)
def _xlo_pairs(kind):
    if kind == "int":
        prs = [(_tap_idx(-1, i), _tap_idx(0, i)) for i in range(9)]
        prs += [(_tap_idx(1, 2 * k), _tap_idx(1, 2 * k + 1)) for k in range(4)]
        prs += [(_tap_idx(1, 8), None)]
    elif kind == "e0":
        prs = [(_tap_idx(0, i), _tap_idx(1, i)) for i in range(9)]
    else:  # e7
        prs = [(_tap_idx(-1, i), _tap_idx(0, i)) for i in range(9)]
    return prs


# ordered unique pair keys across all chunk kinds
_PAIR_KEYS = []
for _kind in ("int", "e0", "e7"):
    for _p in _xlo_pairs(_kind):
        if _p not in _PAIR_KEYS:
            _PAIR_KEYS.append(_p)
_PAIR_POS = {p: i for i, p in enumerate(_PAIR_KEYS)}
NPAIRS = len(_PAIR_KEYS)


def _win_off(t_slab, dh, dw, half):
    """Column offset of a 264-wide tap window in the per-b big slab tile."""
    return SLAB_AL * t_slab + WP * (1 + dh + 4 * half) + (1 + dw)


def _build():
    import concourse.bacc as bacc
    import concourse.mybir as mybir
    import concourse.tile as tile
    from concourse import masks
    from concourse.ap import AP as APc

    F32 = mybir.dt.float32
    F16 = mybir.dt.float16
    F8 = mybir.dt.float8e4
    AL = mybir.AluOpType
    AF = mybir.ActivationFunctionType
    AX = mybir.AxisListType
    PM = mybir.MatmulPerfMode

    NW = 264  # tap window width (4 h-rows x 66)

    def pair_ap(full, off_a, off_b, n=NW):
        """[p, 2, n] AP over `full` (a [p, X] AP) with windows at two offsets."""
        base = full[:, off_a:off_a + n]
        return APc(base.tensor, base.offset,
                   [list(d) for d in base.ap[:1]] + [[off_b - off_a, 2]] +
                   [list(d) for d in base.ap[1:]])

    nc = bacc.Bacc("TRN2", target_bir_lowering=False, debug=False,
                   num_devices=NCORES)

    # ---- dram inputs ----
    x8hi = nc.dram_tensor("x8hi", [96, 2 * NPADTOK], F8,
                          kind="ExternalInput").ap()
    x8lo = nc.dram_tensor("x8lo", [96, 2 * NPADTOK], F8,
                          kind="ExternalInput").ap()
    wq8hi = nc.dram_tensor("wq8hi", [96, 2 * CW], F8,
                           kind="ExternalInput").ap()
    wq8lo = nc.dram_tensor("wq8lo", [96, 2 * CW], F8,
                           kind="ExternalInput").ap()
    qkvb8 = nc.dram_tensor("qkvb8", [128, 5], F32, kind="ExternalInput").ap()
    # prebuilt diag tiles: per mtile, 27 x [mcw, 2mcw] (whi|wlo); mi4 is
    # half-paired: 128 partitions with the 64 channels' diag values repeated
    DMCW = [128, 128, 128, 128, 128]
    dgd = [nc.dram_tensor(f"dg{mi}", [DMCW[mi], 27 * 2 * DMCW[mi]], F8,
                          kind="ExternalInput").ap()
           for mi in range(5)]
    # xlo-pass pair diags for v mtiles: NPAIRS x [mcw, 2mcw] (whi_a|whi_b)
    dgp = {mi: nc.dram_tensor(f"dgp{mi}", [DMCW[mi],
                                           NPAIRS * 2 * DMCW[mi]], F8,
                              kind="ExternalInput").ap()
           for mi in V_MIS}
    dwbqk = nc.dram_tensor("dwbqk", [128, 3], F32, kind="ExternalInput").ap()
    dwbv = nc.dram_tensor("dwbv", [128, 2], F32, kind="ExternalInput").ap()
    projwT = nc.dram_tensor("projwT", [DIM, DIM], F16,
                            kind="ExternalInput").ap()
    projb = nc.dram_tensor("projb", [128, 2], F32, kind="ExternalInput").ap()
    temp = nc.dram_tensor("temp", [16, 1], F32, kind="ExternalInput").ap()
    out = nc.dram_tensor("out", [DIM, NTOK], F32, kind="ExternalOutput").ap()

    gram_in = nc.dram_tensor("gram_in", [16, 48, 48], F32).ap()
    gram_out = nc.dram_tensor("gram_out", [16, 48, 48], F32,
                              addr_space="Shared").ap()
    attn_dram = nc.dram_tensor("attn_dram", [16, HD, HD], F16).ap()

    KTILES = [(0, 128), (128, 64)]  # fp16 contraction split (proj/attn@v)

    with tile.TileContext(nc) as tc:
        with (
            tc.tile_pool(name="wp", bufs=1) as wp,
            tc.tile_pool(name="xp", bufs=3) as xp,
            tc.tile_pool(name="slb", bufs=1) as slb,
            tc.tile_pool(name="qk", bufs=1) as qkpool,
            tc.tile_pool(name="ev", bufs=4) as ev,
            tc.tile_pool(name="evh", bufs=2) as evh,
            tc.tile_pool(name="small", bufs=1) as sp,
            tc.tile_pool(name="ps", bufs=5, space="PSUM") as psp,
            tc.tile_pool(name="pst", bufs=2, space="PSUM") as pst,
            tc.tile_pool(name="psg", bufs=1, space="PSUM") as psg,
        ):
            # ---------------- weights ----------------
            # qkv weights on the SP queue (needed first); the 6MB of diag
            # tiles go on the idle Pool DGE queue so the x-slab DMAs are not
            # stuck behind them.
            wqhi = wp.tile([96, 2 * CW], F8, tag="wqhi")
            nc.sync.dma_start(out=wqhi[:], in_=wq8hi)
            wqlo = wp.tile([96, 2 * CW], F8, tag="wqlo")
            nc.sync.dma_start(out=wqlo[:], in_=wq8lo)
            qkvb_s = wp.tile([128, 5], F32, tag="qkvb")
            nc.sync.dma_start(out=qkvb_s[:], in_=qkvb8)
            dg_s = []
            for mi in range(5):
                mcw = DMCW[mi]
                t = wp.tile([mcw, 27 * 2 * mcw], F8, tag=f"dg{mi}")
                nc.gpsimd.dma_start(out=t[:], in_=dgd[mi])
                dg_s.append(t)
            dgp_s = {}
            for mi in V_MIS:
                mcw = DMCW[mi]
                t = wp.tile([mcw, NPAIRS * 2 * mcw], F8, tag=f"dgp{mi}")
                nc.gpsimd.dma_start(out=t[:], in_=dgp[mi])
                dgp_s[mi] = t
            dwbqk_s = wp.tile([128, 3], F32, tag="dwbqk")
            nc.gpsimd.dma_start(out=dwbqk_s[:], in_=dwbqk)
            dwbv_s = wp.tile([128, 2], F32, tag="dwbv")
            nc.gpsimd.dma_start(out=dwbv_s[:], in_=dwbv)
            wproj = []
            for ki, (ko, kc) in enumerate(KTILES):
                t = wp.tile([kc, DIM], F16, tag=f"wproj{ki}")
                nc.sync.dma_start(out=t[:], in_=projwT[ko:ko + kc, :])
                wproj.append(t)
            projb_s = wp.tile([128, 2], F32, tag="projb")
            nc.sync.dma_start(out=projb_s[:], in_=projb)
            temp_s = wp.tile([16, 1], F32, tag="temp")
            nc.sync.dma_start(out=temp_s[:], in_=temp)

            ident8 = wp.tile([128, 128], F8, tag="ident8")
            masks.make_identity(nc, ident8[:])

            def dg_ap(mi, ti):
                mc = DMCW[mi]
                return dg_s[mi][:, ti * 2 * mc:(ti + 1) * 2 * mc].rearrange(
                    "p (two m) -> p two m", two=2)

            def dgp_ap(mi, pos):
                mc = DMCW[mi]
                return dgp_s[mi][:, pos * 2 * mc:(pos + 1) * 2 * mc].rearrange(
                    "p (two m) -> p two m", two=2)

            # dw outputs: q~,k~ (384 ch) fp8 x32, one batch at a time since
            # transposes+grams drain per batch; v (192 ch) fp16 both batches
            NTOKB = NTOK // B  # 4096
            qk_t = [qkpool.tile([128, NTOKB], F8, tag=f"qk{i}", name=f"qk{i}")
                    for i in range(3)]
            # v tile 0: 128 channels, both halves along tokens; v tile 1
            # (64 ch) is half-paired: p0:64 half0, p64:128 half1 per chunk
            v_t = [qkpool.tile([128, NTOK], F16, tag="v0", name="v0"),
                   qkpool.tile([128, NTOKH], F16, tag="v1", name="v1")]
            # base-0 copy of v2's odd-half partitions (the BIR path rejects
            # base-64 matmuls; DMA legally shifts partitions)
            v2b = qkpool.tile([64, NTOKH], F16, tag="v2b", name="v2b")

            # per-b big slab tiles (all 10 t-slabs contiguous, stride 664);
            # mi4 half-paired: p64:128 hold slab cols shifted by -264
            sl_hi = [slb.tile([DMCW[mi], NPT], F8, tag=f"slhi{mi}",
                              name=f"slhi{mi}") for mi in range(5)]
            sl_lo = {mi: slb.tile([DMCW[mi], NPT], F8, tag=f"sllo{mi}",
                                  name=f"sllo{mi}") for mi in V_MIS}
            # zero the never-written tails read by the widest tap windows
            # (their products land in trimmed output columns): the 4 gap
            # cols after each slab, and mi4's shifted-half tail [396, 664)
            for t_z in range(1, 9):
                for tl in (sl_hi[4], sl_lo[4]):
                    nc.vector.memset(
                        tl[64:128, SLAB_AL * t_z + 396:SLAB_AL * (t_z + 1)],
                        0.0)
                for tl in (sl_hi[0], sl_hi[1], sl_hi[2], sl_hi[3],
                           sl_lo[3]):
                    nc.vector.memset(
                        tl[:, SLAB_AL * t_z + 660:SLAB_AL * (t_z + 1)], 0.0)
                for tl in (sl_hi[4], sl_lo[4]):
                    nc.vector.memset(
                        tl[0:64, SLAB_AL * t_z + 660:SLAB_AL * (t_z + 1)],
                        0.0)

            # ---------------- qkv conv ----------------
            def qkv_slab(b, t_):
                xhi = xp.tile([96, 2 * SLAB_AL], F8, tag="xhi")
                xlo = xp.tile([96, 2 * SLAB_AL], F8, tag="xlo")
                off = (b * TP + t_) * SLAB
                for src, dst in ((x8hi, xhi), (x8lo, xlo)):
                    nc.sync.dma_start(
                        out=dst[:].rearrange("p (two n) -> p two n",
                                             two=2)[:, :, 0:SLAB],
                        in_=src.rearrange("p (two n) -> p two n",
                                          two=2)[:, :, off:off + SLAB])
                xhi2 = xhi[:].rearrange("p (two n) -> p two n", two=2)
                xlo2 = xlo[:].rearrange("p (two n) -> p two n", two=2)
                base = SLAB_AL * t_
                for mi in range(5):
                    mo = MTILES[mi][0]
                    mc = DMCW[mi]
                    whi2 = wqhi[:].rearrange("p (two m) -> p two m",
                                             two=2)[:, :, mo:mo + mc]
                    wlo2 = wqlo[:].rearrange("p (two m) -> p two m",
                                             two=2)[:, :, mo:mo + mc]
                    for half in range(2):
                        cs = 330 * half
                        ce = cs + 330
                        ps = psp.tile([128, 512], F32, tag="mm")
                        three = mi in V_MIS
                        nc.tensor.matmul(ps[:mc, :330], whi2,
                                         xhi2[:, :, cs:ce], start=True,
                                         stop=False, perf_mode=PM.DoubleRow)
                        nc.tensor.matmul(ps[:mc, :330], wlo2,
                                         xhi2[:, :, cs:ce], start=False,
                                         stop=not three,
                                         perf_mode=PM.DoubleRow)
                        if three:
                            nc.tensor.matmul(ps[:mc, :330], whi2,
                                             xlo2[:, :, cs:ce], start=False,
                                             stop=True,
                                             perf_mode=PM.DoubleRow)
                        dst = base + cs
                        # hi slab: 8*(qkv) = psum/4 + 8b
                        nhi = 64 if mi == 4 else mc
                        nc.scalar.activation(
                            sl_hi[mi][:nhi, dst:dst + 330], ps[:nhi, :330],
                            AF.Identity, bias=qkvb_s[:nhi, mi:mi + 1],
                            scale=0.25)
                        if mi == 4:
                            # shifted copy for half-paired layout: p64:128
                            # col j holds slab col j+264
                            if half == 0:
                                nc.scalar.activation(
                                    sl_hi[4][64:128, base:base + 66],
                                    ps[64:128, 264:330], AF.Identity,
                                    bias=qkvb_s[64:128, 4:5], scale=0.25)
                            else:
                                nc.scalar.activation(
                                    sl_hi[4][64:128, base + 66:base + 396],
                                    ps[64:128, 0:330], AF.Identity,
                                    bias=qkvb_s[64:128, 4:5], scale=0.25)
                        if three:
                            half32 = evh.tile([mc, 330], F32, tag=f"h32_{mi}")
                            nc.scalar.activation(
                                half32[:], ps[:mc, :330], AF.Identity,
                                bias=qkvb_s[:mc, mi:mi + 1], scale=0.25)
                            nc.vector.tensor_sub(
                                sl_lo[mi][:nhi, dst:dst + 330],
                                half32[:nhi], sl_hi[mi][:nhi, dst:dst + 330])
                            if mi == 4:
                                if half == 0:
                                    nc.vector.tensor_sub(
                                        sl_lo[4][64:128, base:base + 66],
                                        half32[64:128, 264:330],
                                        sl_hi[4][64:128, base:base + 66])
                                else:
                                    nc.vector.tensor_sub(
                                        sl_lo[4][64:128,
                                                 base + 66:base + 396],
                                        half32[64:128, 0:330],
                                        sl_hi[4][64:128,
                                                 base + 66:base + 396])

            # ---------------- depthwise ----------------
            def dw_chunk(b, t_o):
                if t_o == 0:
                    kind, dts = "e0", (0, 1)
                elif t_o == T - 1:
                    kind, dts = "e7", (-1, 0)
                else:
                    kind, dts = "int", (-1, 0, 1)
                keep = [(dt + 1) * 9 + i for dt in dts for i in range(9)]
                xpairs = _xlo_pairs(kind)

                def dw_group(mi, half):
                    # one accumulation group: all taps of (chunk, mtile).
                    # mi4 runs once per chunk (half-paired partitions).
                    mc = DMCW[mi]
                    ps = psp.tile([128, 512], F32, tag="mm")
                    last_hi = (mi not in V_MIS)
                    for j, ti in enumerate(keep):
                        dt, dh, dw = TAPS[ti]
                        off = _win_off(t_o + 1 + dt, dh, dw, half)
                        rhs = pair_ap(sl_hi[mi][:], off, off)
                        nc.tensor.matmul(
                            ps[:mc, :NW], dg_ap(mi, ti), rhs,
                            start=(j == 0),
                            stop=(last_hi and j == len(keep) - 1),
                            perf_mode=PM.DoubleRow)
                    if not last_hi:
                        for j, (ta, tb) in enumerate(xpairs):
                            dta, dha, dwa = TAPS[ta]
                            offa = _win_off(t_o + 1 + dta, dha, dwa, half)
                            if tb is None:
                                offb = offa
                            else:
                                dtb, dhb, dwb_ = TAPS[tb]
                                offb = _win_off(t_o + 1 + dtb, dhb, dwb_,
                                                half)
                            rhs = pair_ap(sl_lo[mi][:], offa, offb)
                            nc.tensor.matmul(
                                ps[:mc, :NW],
                                dgp_ap(mi, _PAIR_POS[(ta, tb)]), rhs,
                                start=False, stop=(j == len(xpairs) - 1),
                                perf_mode=PM.DoubleRow)
                    return ps

                for half in range(2):
                    co = 256 * ((b * T + t_o) * 2 + half)      # v tiles
                    cob = 256 * (t_o * 2 + half)               # qk (per-batch)
                    for mi in range(4):
                        ps = dw_group(mi, half)
                        pv = ps[:, :NW].rearrange(
                            "p (h w) -> p h w", h=4)[:, :, 0:64]
                        if mi in QK_MIS:
                            # qk8 = 32*(dw+b) = psum/8 + 32b, on DVE to
                            # keep the ACT queue short for the v evictions
                            nc.vector.tensor_scalar(
                                qk_t[mi][:, cob:cob + 256].rearrange(
                                    "p (h w) -> p h w", h=4),
                                pv[:128], 0.125,
                                dwbqk_s[:128, mi:mi + 1],
                                AL.mult, AL.add)
                        else:
                            nc.scalar.activation(
                                v_t[0][:, co:co + 256].rearrange(
                                    "p (h w) -> p h w", h=4),
                                pv[:128], AF.Identity,
                                bias=dwbv_s[:128, 0:1], scale=1.0 / 256)
                # mi4: both halves in one pass (p0:64 half0, p64:128 half1)
                cv = 256 * (b * T + t_o)
                ps = dw_group(4, 0)
                pv = ps[:, :NW].rearrange("p (h w) -> p h w", h=4)[:, :, 0:64]
                nc.scalar.activation(
                    v_t[1][0:64, cv:cv + 256].rearrange(
                        "p (h w) -> p h w", h=4),
                    pv[0:64], AF.Identity,
                    bias=dwbv_s[0:64, 1:2], scale=1.0 / 256)
                nc.scalar.activation(
                    v_t[1][64:128, cv:cv + 256].rearrange(
                        "p (h w) -> p h w", h=4),
                    pv[64:128], AF.Identity,
                    bias=dwbv_s[64:128, 1:2], scale=1.0 / 256)

            gps = psg.tile([48, 384], F32, tag="gram")  # reused across b
            dm = sp.tile([8, 576], F32, tag="dm")
            nc.gpsimd.memset(dm[:], 0.0)
            nc.gpsimd.affine_select(
                out=dm[:], in_=dm[:], compare_op=AL.not_equal, fill=1.0,
                base=0, pattern=[[1, 24], [-1, 24]], channel_multiplier=0)
            bd = {}

            def gram_pairs(b, pr0, pr1):
                # transposes + grams for c64 pairs [pr0, pr1)
                for pr in range(pr0, pr1):
                    c64a = 2 * pr  # qk_t holds the current batch only
                    qkT = ev.tile([128, 768], F8, tag="qkT")
                    for s in range(2):
                        for i in range(3):
                            # fp8 transpose requires output element step 2
                            tps = pst.tile([128, 256], F8, tag="trps")
                            nc.tensor.transpose(
                                tps[:, 0:256:2],
                                qk_t[i][:, 128 * (c64a + s):
                                        128 * (c64a + s + 1)],
                                ident8[:])
                            nc.vector.tensor_copy(
                                qkT[:, 384 * s + 128 * i:
                                    384 * s + 128 * (i + 1)],
                                tps[:, 0:256:2])
                    z2 = qkT[:].rearrange("p (two m) -> p two m", two=2)
                    for h in range(HEADS):
                        z = z2[:, :, 48 * h:48 * (h + 1)]
                        nc.tensor.matmul(
                            gps[:, 48 * h:48 * (h + 1)], z, z,
                            start=(pr == 0 and h == 0),
                            stop=(pr == 15 and h == HEADS - 1),
                            perf_mode=PM.DoubleRow)

            def gram_allreduce(b):
                gs = ev.tile([48, 384], F32, tag="gs")
                nc.vector.tensor_copy(gs[:], gps[:])
                nc.gpsimd.dma_start(
                    out=gram_in[8 * b:8 * (b + 1)].rearrange(
                        "g c d -> c g d"),
                    in_=gs[:].rearrange("c (g d) -> c g d", g=8))
                nc.gpsimd.collective_compute(
                    "AllReduce", AL.add,
                    replica_groups=[list(range(NCORES))],
                    ins=[gram_in[8 * b:8 * (b + 1)]],
                    outs=[gram_out[8 * b:8 * (b + 1)]])

            def softmax_section(b):
                # norms + softmax for one batch ([8, .]); grams carry a 1024x
                # scale (32q)(32k) which cancels exactly in the normalized
                # correlation.
                qq_f = sp.tile([8, 576], F32, tag="qqf")
                kk_f = sp.tile([8, 576], F32, tag="kkf")
                qk_f = sp.tile([8, 576], F32, tag="qkf")
                go = gram_out[8 * b:8 * (b + 1)]
                nc.sync.dma_start(
                    out=qq_f[:].rearrange("p (c d) -> p c d", c=24),
                    in_=go[:, 0:24, 0:24])
                nc.sync.dma_start(
                    out=kk_f[:].rearrange("p (c d) -> p c d", c=24),
                    in_=go[:, 24:48, 24:48])
                nc.sync.dma_start(
                    out=qk_f[:].rearrange("p (c d) -> p c d", c=24),
                    in_=go[:, 0:24, 24:48])

                def diag_rsqrt(src, tag):
                    t1 = sp.tile([8, 576], F32, tag="tmp576")
                    nc.vector.tensor_mul(t1[:], src[:], dm[:])
                    n2 = sp.tile([8, 24], F32, tag=tag + "b")
                    nc.vector.tensor_reduce(
                        n2[:], t1[:].rearrange("p (c d) -> p c d", c=24),
                        axis=AX.X, op=AL.add)
                    nrm = sp.tile([8, 24], F32, tag=tag + "c")
                    nc.scalar.sqrt(nrm[:], n2[:])
                    r = sp.tile([8, 24], F32, tag=tag + "d")
                    nc.vector.reciprocal(r[:], nrm[:])
                    return r

                rq = diag_rsqrt(qq_f, "rq")
                rk = diag_rsqrt(kk_f, "rk")
                # fold temperature into rq ([8,24] op instead of [8,576])
                nc.vector.tensor_scalar_mul(rq[:], rq[:], temp_s[:8])

                a1 = sp.tile([8, 576], F32, tag="a1")
                nc.vector.tensor_mul(
                    a1[:].rearrange("p (c d) -> p c d", c=24),
                    qk_f[:].rearrange("p (c d) -> p c d", c=24),
                    rq[:].rearrange("p (c one) -> p c one",
                                    one=1).broadcast_to((8, 24, 24)))
                nc.vector.tensor_mul(
                    a1[:].rearrange("p (c d) -> p c d", c=24),
                    a1[:].rearrange("p (c d) -> p c d", c=24),
                    rk[:].rearrange("p (one d) -> p one d",
                                    one=1).broadcast_to((8, 24, 24)))
                # logits are normalized correlations * temp, |x| <= ~1:
                # exp() is safe without the max-subtract pass
                ex = sp.tile([8, 576], F32, tag="ex")
                nc.scalar.activation(ex[:], a1[:], AF.Exp)
                sm = sp.tile([8, 24], F32, tag="sm")
                nc.vector.tensor_reduce(
                    sm[:], ex[:].rearrange("p (c d) -> p c d", c=24),
                    axis=AX.X, op=AL.add)
                rs = sp.tile([8, 24], F32, tag="rs")
                nc.vector.reciprocal(rs[:], sm[:])
                at16 = sp.tile([8, 576], F16, tag="at16")
                nc.vector.tensor_mul(
                    at16[:].rearrange("p (c d) -> p c d", c=24),
                    ex[:].rearrange("p (c d) -> p c d", c=24),
                    rs[:].rearrange("p (c one) -> p c one",
                                    one=1).broadcast_to((8, 24, 24)))
                nc.sync.dma_start(
                    out=attn_dram[8 * b:8 * (b + 1)],
                    in_=at16[:].rearrange("p (c d) -> p c d", c=24))

                # block-diag attn^T, split into k-tiles 128 + 64(dup at p64:
                # for the half-paired v2); spread the small loads across DGE
                # queues to cut serial issue latency
                bts = [sp.tile([128, DIM], F16, tag=f"bd{b}_0",
                               name=f"bd{b}_0"),
                       sp.tile([128, DIM], F16, tag=f"bd{b}_1",
                               name=f"bd{b}_1")]
                nc.vector.memset(bts[0][:], 0.0)
                nc.vector.memset(bts[1][:], 0.0)
                qs = [nc.sync, nc.gpsimd, nc.scalar]
                for h in range(HEADS):
                    q = qs[h % 3]
                    src_a = attn_dram[8 * b + h].rearrange("c d -> d c")
                    r0, r1 = HD * h, HD * (h + 1)
                    if r1 <= 128:
                        q.dma_start(out=bts[0][r0:r1, r0:r1], in_=src_a)
                    elif r0 >= 128:
                        q.dma_start(
                            out=bts[1][r0 - 128:r1 - 128, r0:r1], in_=src_a)
                        q.dma_start(
                            out=bts[1][r0 - 64:r1 - 64, r0:r1], in_=src_a)
                    else:  # h == 5: rows 120..144 straddle the k-tile split
                        q.dma_start(
                            out=bts[0][r0:128, r0:r1], in_=src_a[0:128 - r0])
                        q.dma_start(
                            out=bts[1][0:r1 - 128, r0:r1],
                            in_=src_a[128 - r0:HD])
                        q.dma_start(
                            out=bts[1][64:r1 - 64, r0:r1],
                            in_=src_a[128 - r0:HD])
                bd[b] = bts

            def av_chunks(b, lo, hi, use_act=False):
                # two-phase per 4-chunk block: all attn@v groups first (PSUM
                # evicted to aos), then all proj groups — hides the aos copy
                # latency behind the other chunks' matmuls. use_act routes
                # PSUM evictions to the ACT engine (idle during the tail,
                # while DVE runs the softmax chain).
                def evict(dst, src, bias=None):
                    if use_act:
                        if bias is None:
                            nc.scalar.activation(dst, src, AF.Identity)
                        else:
                            nc.scalar.activation(dst, src, AF.Identity,
                                                 bias=bias, scale=1.0)
                    else:
                        if bias is None:
                            nc.vector.tensor_copy(dst, src)
                        else:
                            nc.vector.tensor_scalar(dst, src, bias, None,
                                                    AL.add)
                for blk in range(lo, hi, 4):
                    chunks = list(range(b * T + blk,
                                        b * T + min(blk + 4, hi)))
                    aom = {}
                    for chunk in chunks:
                        co = 512 * chunk
                        cv = 256 * chunk
                        aos = [ev.tile([kc, 512], F16, tag=f"ao{ki}",
                                       name=f"ao{ki}")
                               for ki, (ko, kc) in enumerate(KTILES)]
                        for mi, (mo, mc) in enumerate(KTILES):
                            ps = psp.tile([128, 512], F32, tag="mm")
                            nc.tensor.matmul(
                                ps[:mc, :], bd[b][0][:, mo:mo + mc],
                                v_t[0][:, co:co + 512],
                                start=True, stop=False)
                            # v2 is half-paired: half0 tokens from v_t[1]
                            # p0:64, half1 from the base-0 copy v2b
                            nc.tensor.matmul(
                                ps[:mc, 0:256], bd[b][1][0:64, mo:mo + mc],
                                v_t[1][0:64, cv:cv + 256],
                                start=False, stop=False)
                            nc.tensor.matmul(
                                ps[:mc, 256:512],
                                bd[b][1][0:64, mo:mo + mc],
                                v2b[:, cv:cv + 256],
                                start=False, stop=True)
                            evict(aos[mi][:, :], ps[:mc, :])
                        aom[chunk] = aos
                    for chunk in chunks:
                        co = 512 * chunk
                        for mi, (mo, mc) in enumerate(KTILES):
                            ps = psp.tile([128, 512], F32, tag="mm")
                            for ki in range(2):
                                nc.tensor.matmul(
                                    ps[:mc, :], wproj[ki][:, mo:mo + mc],
                                    aom[chunk][ki][:, :],
                                    start=(ki == 0), stop=(ki == 1))
                            of = ev.tile([128, 512], F32, tag="of")
                            evict(of[:mc, :], ps[:mc, :],
                                  bias=projb_s[:mc, mi:mi + 1])
                            nc.sync.dma_start(
                                out=out[mo:mo + mc, co:co + 512],
                                in_=of[:mc, :])

            for b in range(B):
                for t_ in (1, 2, 3):
                    qkv_slab(b, t_)
                for t_o in range(T):
                    # qkv first: its ACT evictions land ahead of dw's in the
                    # in-order ACT queue and are ready earlier (they depend on
                    # qkv matmuls, which also run first on PE) — avoids
                    # head-of-line blocking of the dw PSUM evictions.
                    if t_o + 4 <= T:
                        qkv_slab(b, t_o + 4)
                    dw_chunk(b, t_o)
                    if t_o >= 1:
                        # grams for chunk t_o-1 (its evictions are done)
                        gram_pairs(b, 2 * (t_o - 1), 2 * t_o)
                    if t_o == T - 1:
                        # stage this batch's odd-half v2 rows at base 0
                        nc.gpsimd.dma_start(
                            out=v2b[:, 2048 * b:2048 * (b + 1)],
                            in_=v_t[1][64:128, 2048 * b:2048 * (b + 1)])
                    if b == 1 and t_o == 0:
                        # batch 0's softmax runs during batch 1's dw phase
                        softmax_section(0)
                gram_pairs(b, 14, 16)
                gram_allreduce(b)
            # tail: all of batch 0's attention fills the PE while batch 1's
            # softmax chain runs on DVE/ACT/DMA
            av_chunks(0, 0, T, use_act=True)
            softmax_section(1)
            av_chunks(1, 0, T, use_act=True)
    nc.compile()
    return nc


def _prep_inputs(x, qkv_w, qkv_b, dw_w, dw_b, temperature, proj_w, proj_b):
    """Host-side prep: fp8 hi/lo inputs, prebuilt fp8 diag tiles, fp16 proj."""
    import ml_dtypes
    F8NP = ml_dtypes.float8_e4m3

    def q8(a):
        return np.asarray(a, np.float32).astype(F8NP)

    x = np.asarray(x, np.float32)
    b_, c_, t_, h_, w_ = x.shape  # 2, 192, 8, 64, 64
    qkv_w2 = np.asarray(qkv_w, np.float32).reshape(C3, DIM)
    dw_w2 = np.asarray(dw_w, np.float32).reshape(C3, 27)
    proj_w2 = np.asarray(proj_w, np.float32).reshape(DIM, DIM)
    # permute qkv channels: [q_h0, k_h0, q_h1, k_h1, ..., v] so each head's
    # (q,k) columns are adjacent after transpose (contiguous gram operands)
    perm = []
    for h in range(HEADS):
        perm.extend(range(HD * h, HD * (h + 1)))          # q_h
        perm.extend(range(DIM + HD * h, DIM + HD * (h + 1)))  # k_h
    perm.extend(range(2 * DIM, 3 * DIM))                  # v unchanged
    perm = np.array(perm)
    qkv_w2 = qkv_w2[perm]
    dw_w2 = dw_w2[perm]
    qkv_b = np.asarray(qkv_b, np.float32)[perm]
    dw_b = np.asarray(dw_b, np.float32)[perm]

    # qkv weights x32, hi/lo, laid out [96, 2, 576] -> [96, 1152]
    w32 = 32.0 * qkv_w2  # [576, 192]
    whi = q8(w32).astype(np.float32)
    wlo = q8(w32 - whi).astype(np.float32)

    def wq_layout(w):  # [576(out), 192(in)] -> [96, 2*CW] fp8
        # append a duplicate of out-channels 512:576 (mi4 half-pairing)
        wd = np.concatenate([w, w[512:576]], axis=0)  # [640, 192]
        wt = np.ascontiguousarray(wd.T)               # [192, 640]
        return np.ascontiguousarray(
            wt.reshape(2, 96, CW).transpose(1, 0, 2).reshape(96, 2 * CW)
        ).astype(F8NP)

    wq8hi_h = wq_layout(whi)
    wq8lo_h = wq_layout(wlo)

    qkvb8_h = np.zeros((128, 5), np.float32)
    for mi, (mo, mc) in enumerate(MTILES):
        qkvb8_h[:mc, mi] = 8.0 * qkv_b[mo:mo + mc]
    qkvb8_h[64:128, 4] = qkvb8_h[0:64, 4]  # mi4 duplicated channels

    # dw diag tiles (whi|wlo) per tap, x32; mi4 replicated to 128 partitions
    d32 = 32.0 * dw_w2  # [576, 27]
    dhi = q8(d32).astype(np.float32)
    dlo = q8(d32 - dhi).astype(np.float32)
    DMCW = [128, 128, 128, 128, 128]

    def _dsel(mi, idx, src, ti):
        mo = MTILES[mi][0]
        return src[mo + (idx % 64), ti] if mi == 4 else src[mo + idx, ti]

    dgd_h = []
    for mi in range(5):
        mc = DMCW[mi]
        t = np.zeros((mc, 27 * 2 * mc), np.float32)
        idx = np.arange(mc)
        for ti in range(27):
            t[idx, ti * 2 * mc + idx] = _dsel(mi, idx, dhi, ti)
            t[idx, ti * 2 * mc + mc + idx] = _dsel(mi, idx, dlo, ti)
        dgd_h.append(t.astype(F8NP))
    dgp_h = {}
    for mi in V_MIS:
        mc = DMCW[mi]
        t = np.zeros((mc, NPAIRS * 2 * mc), np.float32)
        idx = np.arange(mc)
        for pos, (ta, tb) in enumerate(_PAIR_KEYS):
            t[idx, pos * 2 * mc + idx] = _dsel(mi, idx, dhi, ta)
            if tb is not None:
                t[idx, pos * 2 * mc + mc + idx] = _dsel(mi, idx, dhi, tb)
        dgp_h[mi] = t.astype(F8NP)

    dwbqk_h = np.zeros((128, 3), np.float32)
    for mi in QK_MIS:
        mo, mc = MTILES[mi]
        dwbqk_h[:mc, mi] = 32.0 * dw_b[mo:mo + mc]
    dwbv_h = np.zeros((128, 2), np.float32)
    dwbv_h[:128, 0] = dw_b[384:512]
    dwbv_h[0:64, 1] = dw_b[512:576]
    dwbv_h[64:128, 1] = dw_b[512:576]

    projwT_h = np.ascontiguousarray(proj_w2.T).astype(np.float16)
    projb_h = np.zeros((128, 2), np.float32)
    projb_h[:128, 0] = np.asarray(proj_b, np.float32)[0:128]
    projb_h[:64, 1] = np.asarray(proj_b, np.float32)[128:192]
    temp_h = np.tile(np.asarray(temperature, np.float32).reshape(HEADS),
                     2).reshape(16, 1)  # g = b*8+h

    in_maps = []
    for i in range(NCORES):
        # padded slab [b, t10, h10, w66], h rows 8i-1 .. 8i+9 clamped->zero
        xs = np.zeros((b_, TP, HP, WP, c_), np.float32)
        hlo, hhi = 8 * i - 1, 8 * i + 9
        slo, shi = max(0, hlo), min(h_, hhi)
        xt = x[:, :, :, slo:shi, :].transpose(0, 2, 3, 4, 1)
        xs[:, 1:9, (slo - hlo):(slo - hlo) + (shi - slo), 1:65, :] = xt
        xT = np.ascontiguousarray(
            xs.reshape(b_ * TP * SLAB, c_).T)  # [192, 13200] f32
        xhi = q8(xT).astype(np.float32)
        xlo_ = q8(xT - xhi)

        def x_layout(a):  # [192, NPADTOK] -> [96, 2*NPADTOK] fp8
            return np.ascontiguousarray(
                np.asarray(a, np.float32).reshape(2, 96, NPADTOK)
                .transpose(1, 0, 2).reshape(96, 2 * NPADTOK)).astype(F8NP)

        m = {
            "x8hi": x_layout(xhi), "x8lo": x_layout(xlo_),
            "wq8hi": wq8hi_h, "wq8lo": wq8lo_h, "qkvb8": qkvb8_h,
            "dwbqk": dwbqk_h, "dwbv": dwbv_h,
            "projwT": projwT_h, "projb": projb_h, "temp": temp_h,
        }
        for mi in range(5):
            m[f"dg{mi}"] = dgd_h[mi]
        for mi in V_MIS:
            m[f"dgp{mi}"] = dgp_h[mi]
        in_maps.append(m)
    return in_maps


def _get_runner():
    """Build once; return a persistent sharded-jit callable (the per-call
    closure in bass2jax.run_bass_via_pjrt defeats jax's jit cache)."""
    if "runner" in _CACHE:
        return _CACHE["runner"]
    import jax
    for flag, val in [("jax_compilation_cache_dir", "/tmp/jax_kernel_cache"),
                      ("jax_persistent_cache_min_compile_time_secs", 1.0),
                      ("jax_persistent_cache_min_entry_size_bytes", 0)]:
        try:
            jax.config.update(flag, val)
        except Exception:
            pass
    from jax.sharding import Mesh, PartitionSpec
    from jax.experimental.shard_map import shard_map
    import concourse.mybir as mybir
    from concourse import bass2jax

    nc = _build()
    bass2jax.install_neuronx_cc_hook()

    partition_name = (nc.partition_id_tensor.name
                      if nc.partition_id_tensor else None)
    in_names, out_names, out_avals, zero_shapes = [], [], [], []
    for alloc in nc.m.functions[0].allocations:
        if not isinstance(alloc, mybir.MemoryLocationSet):
            continue
        name = alloc.memorylocations[0].name
        if alloc.kind == "ExternalInput":
            if name != partition_name:
                in_names.append(name)
        elif alloc.kind == "ExternalOutput":
            shape = tuple(alloc.tensor_shape)
            dtype = mybir.dt.np(alloc.dtype)
            out_names.append(name)
            out_avals.append(jax.core.ShapedArray(shape, dtype))
            zero_shapes.append((shape, dtype))
    n_params = len(in_names)
    all_names = in_names + out_names
    if partition_name is not None:
        all_names.append(partition_name)

    def _body(*args):
        operands = list(args)
        if partition_name is not None:
            operands.append(bass2jax.partition_id_tensor())
        outs = bass2jax._bass_exec_p.bind(
            *operands, out_avals=tuple(out_avals), in_names=tuple(all_names),
            out_names=tuple(out_names), lowering_input_output_aliases=(),
            sim_require_finite=True, sim_require_nnan=True, nc=nc)
        return tuple(outs)

    devices = jax.devices()[:NCORES]
    mesh = Mesh(np.asarray(devices), ("core",))
    n_outs = len(out_names)
    sharded = jax.jit(
        shard_map(_body, mesh=mesh,
                  in_specs=(PartitionSpec("core"),) * (n_params + n_outs),
                  out_specs=(PartitionSpec("core"),) * n_outs,
                  check_rep=False),
        donate_argnums=tuple(range(n_params, n_params + n_outs)),
        keep_unused=True)

    def run(in_maps):
        concat_in = [np.concatenate([in_maps[c][nm] for c in range(NCORES)],
                                    axis=0) for nm in in_names]
        concat_zeros = [np.zeros((NCORES * s[0], *s[1:]), dt)
                        for s, dt in zero_shapes]
        out_arrs = sharded(*concat_in, *concat_zeros)
        return [
            {nm: np.asarray(out_arrs[i]).reshape(NCORES, *out_avals[i].shape)[c]
             for i, nm in enumerate(out_names)}
            for c in range(NCORES)]

    _CACHE["runner"] = run
    return run


def kernel(x, qkv_w, qkv_b, dw_w, dw_b, temperature, proj_w, proj_b):
    run = _get_runner()
    in_maps = _prep_inputs(x, qkv_w, qkv_b, dw_w, dw_b, temperature,
                           proj_w, proj_b)
    results = run(in_maps)
    b_, c_, t_, h_, w_ = np.asarray(x).shape
    outf = np.empty((b_, c_, t_, h_, w_), np.float32)
    for i in range(NCORES):
        o = results[i]["out"].reshape(c_, b_, t_, H, w_)
        outf[:, :, :, 8 * i:8 * i + 8, :] = o.transpose(1, 0, 2, 3, 4)
    return outf
